# revision 1
# baseline (speedup 1.0000x reference)
"""Trainium2 Bass kernel for the multiphase CFD fractional-step solver
(predictor + divergence + 2 multigrid V-cycles + projection) on a
64x64x512 grid, sharded along x across 8 NeuronCores.

Self-contained: hardcodes shapes/sharding; reads stencil coefficient
VALUES from the runtime weight inputs and compiles a specialized graph
(cached per coefficient set).

Device layout (level l):
  partitions p = zh*ny + y   (zh in {0,1} z-halves; l=6 has P=1)
  free       j = z'*xc + x   (z' in [0, zr): rows 0 and zr-1 are z-ghosts;
                              x in [0, xc): cols 0 and xc-1 are x-ghosts)
Volume passes run on the row-trimmed flat range [xc, F-xc) so all
+-1 / +-xc shifted reads stay inside the [P, F] tile.
y-axis stencil taps (partition axis) are done on the TensorEngine as
[K,M] matmuls with per-field boundary rows baked into the matrices.
x halos between cores travel through one AllGather per exchange round;
each core unpacks its two neighbor planes with rank-dependent slot
indices passed in as data (value_load -> bass.ds dynamic DMA offsets).
Multigrid levels 0-1 stay sharded; level 2 residual is allgathered and
levels 2-6 are computed replicated on every core.
"""
import sys
sys.path.insert(0, '/opt/trn_rl_repo')
import numpy as np
import concourse.bass as bass
import concourse.bacc as bacc
import concourse.mybir as mybir
from concourse.bass_utils import run_bass_kernel_spmd
from concourse.tile import TileContext

F32 = mybir.dt.float32
I32 = mybir.dt.int32
OP = mybir.AluOpType

DT, DX, G_Z = 0.002, 0.04, -10.0
RHO_L, RHO_G, NU = 1000.0, 1.0, 1e-3
NZ, NY, NX = 64, 64, 512
NC_ = 8
XL = NX // NC_  # 64 local x

# level: (P, ny, zr, xc, sharded, gw) -- gw = x-ghost cols per side
GEOM = {
    0: (128, 64, 34, 72, True, 4),
    1: (64, 32, 18, 36, True, 2),
    2: (16, 16, 18, 130, False, 1),
    3: (8, 8, 10, 66, False, 1),
    4: (4, 4, 6, 34, False, 1),
    5: (2, 2, 4, 18, False, 1),
    6: (1, 1, 3, 10, False, 1),
}
# BC per field: axis -> (lo, hi), 'n' neumann (ghost=adjacent), 'd' dirichlet (ghost=0)
BC_U = {'z': ('n', 'n'), 'y': ('n', 'n'), 'x': ('d', 'd')}
BC_V = {'z': ('n', 'n'), 'y': ('d', 'd'), 'x': ('n', 'n')}
BC_W = {'z': ('d', 'd'), 'y': ('n', 'n'), 'x': ('n', 'n')}
BC_PD = {'z': ('n', 'd'), 'y': ('n', 'n'), 'x': ('n', 'n')}


# ---------------------------------------------------------------- host-side
def _yblock(ny, cm, cc, cp, bc):
    """[ny, ny] matrix M with out[y] = sum_k M[k, y] in[k]:
    tridiag with sub=cm (coeff of in[y-1]), diag=cc, super=cp (in[y+1]),
    Neumann BC folds the ghost coeff into the boundary diagonal."""
    m = np.zeros((ny, ny), np.float32)
    for y in range(ny):
        m[y, y] += cc
        if y > 0:
            m[y - 1, y] += cm
        elif bc[0] == 'n':
            m[y, y] += cm
        if y < ny - 1:
            m[y + 1, y] += cp
        elif bc[1] == 'n':
            m[y, y] += cp
    return m


def _blkdiag2(b):
    n = b.shape[0]
    m = np.zeros((2 * n, 2 * b.shape[1]), np.float32)
    m[:n, :b.shape[1]] = b
    m[n:, b.shape[1]:] = b
    return m


def _halve(ny):
    m = np.zeros((ny, ny // 2), np.float32)
    for y in range(ny):
        m[y, y // 2] = 0.5
    return m


def _double(nyc, nyf):
    m = np.zeros((nyc, nyf), np.float32)
    for y in range(nyf):
        m[y // 2, y] = 1.0
    return m


def build_mats(C):
    """Concatenated [128, sum M] lhsT matrices + column offset map."""
    cols = {}
    parts = []
    total = 0

    def add(name, m, K):
        nonlocal total
        assert m.shape[0] == K and K <= 128 and m.shape[1] <= 128
        buf = np.zeros((128, m.shape[1]), np.float32)
        buf[:K] = m
        cols[name] = (total, m.shape[1], K)
        parts.append(buf)
        total += m.shape[1]

    # predictor diffusion y-taps + center (K=M=128, blockdiag over zh)
    for nm, bc in (('u', BC_U), ('v', BC_V), ('w', BC_W)):
        b = _yblock(64, DT * C['wd_ym'], 1.0 + DT * C['wd_c'], DT * C['wd_yp'], bc['y'])
        add('MD_' + nm, _blkdiag2(b), 128)
    # advection / gradient y-difference (raw tap values)
    for nm, bc in (('u', BC_U), ('v', BC_V), ('w', BC_W), ('pd', BC_PD)):
        b = _yblock(64, C['aym'], 0.0, C['ayp'], bc['y'])
        add('DY_' + nm, _blkdiag2(b), 128)
    # residual y-taps + center at L0
    b = _yblock(64, C['wA_ym'], C['wA_c'], C['wA_yp'], BC_PD['y'])
    add('AY0', _blkdiag2(b), 128)
    # jacobi y matrices per level 0..5 (levels 0-1 are (zh,y) block-diag)
    for l in range(6):
        P, ny = GEOM[l][0], GEOM[l][1]
        b = _yblock(ny, C['jym'], 0.0, C['jyp'], BC_PD['y'])
        add(f'JY{l}', _blkdiag2(b) if l <= 1 else b, P)
    # restrict y-halving matrices (R{l}: level l -> l+1)
    add('R0', _blkdiag2(_halve(64)), 128)
    add('R1', _blkdiag2(_halve(32)), 64)   # stays (zh,y); zh dissolved in AG unpack
    for l in range(2, 6):
        add(f'R{l}', _halve(GEOM[l][1]), GEOM[l][0])
    # prolong y-doubling: PR{l} maps level l+1 -> l
    add('PR0', _blkdiag2(_double(32, 64)), 64)
    m = _double(16, 32)
    add('PR1', np.concatenate([m, m], axis=1), 16)  # s=1 coarse -> (zh,y) fine
    for l in range(2, 6):
        add(f'PR{l}', _double(GEOM[l + 1][1], GEOM[l][1]), GEOM[l + 1][0])

    return np.concatenate(parts, axis=1), cols


def extract_consts(w_diff, w_xadv, w_yadv, w_zadv, w_A, w_res):
    g = lambda a, i, j, k: float(np.asarray(a)[0, 0, i, j, k])
    C = {}
    C['wd_c'] = g(w_diff, 1, 1, 1)
    C['wd_zm'], C['wd_zp'] = g(w_diff, 0, 1, 1), g(w_diff, 2, 1, 1)
    C['wd_ym'], C['wd_yp'] = g(w_diff, 1, 0, 1), g(w_diff, 1, 2, 1)
    C['wd_xm'], C['wd_xp'] = g(w_diff, 1, 1, 0), g(w_diff, 1, 1, 2)
    C['wA_c'] = g(w_A, 1, 1, 1)
    C['wA_zm'], C['wA_zp'] = g(w_A, 0, 1, 1), g(w_A, 2, 1, 1)
    C['wA_ym'], C['wA_yp'] = g(w_A, 1, 0, 1), g(w_A, 1, 2, 1)
    C['wA_xm'], C['wA_xp'] = g(w_A, 1, 1, 0), g(w_A, 1, 1, 2)
    C['axp'], C['axm'] = g(w_xadv, 1, 1, 2), g(w_xadv, 1, 1, 0)
    C['ayp'], C['aym'] = g(w_yadv, 1, 2, 1), g(w_yadv, 1, 0, 1)
    C['azp'], C['azm'] = g(w_zadv, 2, 1, 1), g(w_zadv, 0, 1, 1)
    wr = np.asarray(w_res).ravel()
    assert np.allclose(wr, wr[0]), "nonuniform w_res unsupported"
    C['wres'] = float(wr[0])
    # fast paths used by the kernel
    assert abs(C['axm'] + C['axp']) < 1e-12 * max(1, abs(C['axp']))
    assert abs(C['azm'] + C['azp']) < 1e-12 * max(1, abs(C['azp']))
    # z coefficient symmetry (pool engine computes unscaled zp+zm sums)
    assert abs(C['wd_zm'] - C['wd_zp']) < 1e-12 * max(1, abs(C['wd_zp']))
    assert abs(C['wA_zm'] - C['wA_zp']) < 1e-12 * max(1, abs(C['wA_zp']))
    diag = C['wA_c']
    C['diag'] = diag
    C['jzm'], C['jzp'] = -C['wA_zm'] / diag, -C['wA_zp'] / diag
    C['jym'], C['jyp'] = -C['wA_ym'] / diag, -C['wA_yp'] / diag
    C['jxm'], C['jxp'] = -C['wA_xm'] / diag, -C['wA_xp'] / diag
    C['rb'] = 1.0 / diag
    return C


# ---------------------------------------------------------------- builder
class Fld:
    def __init__(self, t, lvl):
        self.t, self.lvl = t, lvl
        P, ny, zr, xc, _, gw = GEOM[lvl]
        self.P, self.zr, self.xc, self.F, self.gw = P, zr, xc, zr * xc, gw


class B:
    """Builder context."""

    def __init__(self, C, mats_np, mat_cols, dbg_name=None, stage='full', dbg_init=False):
        self.C = C
        self.stage = stage
        self.dbg_init = dbg_init
        self.dbg_name = dbg_name
        self.nc = bacc.Bacc()
        nc = self.nc
        self.mat_cols = mat_cols
        self.MC = mats_np.shape[1]
        # params
        self.p_in = {}
        for nm in ('alpha', 'values_u', 'values_v', 'values_w', 'values_pd'):
            self.p_in[nm] = nc.declare_dram_parameter(nm, [NZ, NY, XL], F32, isOutput=False)
        self.p_mats = nc.declare_dram_parameter('mats', [128, self.MC], F32, isOutput=False)
        self.p_masks = nc.declare_dram_parameter('masks', [128, 4], F32, isOutput=False)
        self.p_hoffs = nc.declare_dram_parameter('hoffs', [1, 9], I32, isOutput=False)
        self.p_out = nc.declare_dram_parameter('out', [4, NZ, NY, XL], F32, isOutput=True)
        if dbg_name:
            self.p_dbg = nc.declare_dram_parameter('dbg', [128, GEOM[0][2] * GEOM[0][3]], F32, isOutput=True)
        self.dbg_written = False

    # --- tile helpers -----------------------------------------------------
    def fld(self, name, lvl, tag=None):
        g = GEOM[lvl]
        t = self.pool.tile([g[0], g[2] * g[3]], F32, tag=(tag or name), name=name)
        if self.dbg_init:
            self.nc.vector.memset(t[:, :], 0.0)
        return Fld(t, lvl)

    def sub(self, f, lvl):
        g = GEOM[lvl]
        return Fld(f.t[0:g[0], 0:g[2] * g[3]], lvl)

    def T(self, f, s=0):
        """row-trimmed shifted flat view [P, F-2*xc]"""
        return f.t[:, f.xc + s: f.F - f.xc + s]

    def V(self, f):
        return f.t[:, 0:f.F]

    def D3(self, f):
        return f.t[:, 0:f.F].rearrange("p (z x) -> p z x", x=f.xc)

    def mat(self, name):
        off, M, K = self.mat_cols[name]
        return self.mats_t[0:K, off:off + M]

    def mm(self, name, rhs_f, Pout, psum_w=None):
        """psum[Pout, F] = mats[name].T @ V(rhs)  (chunked, full width)"""
        nc = self.nc
        F = psum_w or rhs_f.F
        ps = self.psum_pool.tile([Pout, F], F32, tag="psA", name=f"ps_{name}_{nc.next_id()}")
        rhs = rhs_f.t[:, 0:F]
        lhsT = self.mat(name)
        for c0 in range(0, F, 512):
            w = min(512, F - c0)
            nc.tensor.matmul(ps[:, c0:c0 + w], lhsT, rhs[:, c0:c0 + w], start=True, stop=True)
        return ps

    # --- ghost prep -------------------------------------------------------
    def prep_z(self, f, bc):
        """fill z ghost rows: global BC rows (+ inter-half swap on levels 0-1)"""
        nc, d3 = self.nc, self.D3(f)
        P, zr = f.P, f.zr
        split = f.lvl <= 1
        lo = slice(0, P // 2) if split else slice(0, P)
        hi = slice(P // 2, P) if split else slice(0, P)
        if bc['z'][0] == 'n':
            nc.scalar.copy(d3[lo, 0, :], d3[lo, 1, :])
        else:
            nc.gpsimd.memset(d3[lo, 0, :], 0.0)
        if bc['z'][1] == 'n':
            nc.scalar.copy(d3[hi, zr - 1, :], d3[hi, zr - 2, :])
        else:
            nc.gpsimd.memset(d3[hi, zr - 1, :], 0.0)
        if split:
            nc.sync.dma_start(d3[lo, zr - 1, :], d3[hi, 1, :])
            nc.sync.dma_start(d3[hi, 0, :], d3[lo, zr - 2, :])

    def prep_x_bc(self, f, bc):
        """replicated levels: plain BC on both x faces"""
        nc, d3 = self.nc, self.D3(f)
        xc = f.xc
        if bc['x'][0] == 'n':
            nc.scalar.copy(d3[:, :, 0], d3[:, :, 1])
        else:
            nc.gpsimd.memset(d3[:, :, 0], 0.0)
        if bc['x'][1] == 'n':
            nc.scalar.copy(d3[:, :, xc - 1], d3[:, :, xc - 2])
        else:
            nc.gpsimd.memset(d3[:, :, xc - 1], 0.0)

    def edge_fix(self, f, bc):
        """after halo unpack: overwrite ghost cols on the 2 edge cores by BC,
        via per-core mask inputs (mL,nmL,mR,nmR)."""
        nc, d3 = self.nc, self.D3(f)
        P, zr, xc, gw = f.P, f.zr, f.xc, f.gw
        mL, nmL = self.masks_t[0:P, 0:1], self.masks_t[0:P, 1:2]
        mR, nmR = self.masks_t[0:P, 2:3], self.masks_t[0:P, 3:4]
        for (lo, side, m, nm) in ((True, gw - 1, mL, nmL), (False, xc - gw, mR, nmR)):
            gcol = d3[:, :, side]
            if bc['x'][0 if lo else 1] == 'd':
                nc.vector.tensor_scalar_mul(gcol, gcol, nm)
            else:
                icol = d3[:, :, gw if lo else xc - gw - 1]
                tmp = self.ebc_t[0:P, 0:zr]
                nc.vector.tensor_scalar_mul(tmp, icol, m)
                nc.vector.scalar_tensor_tensor(gcol, gcol, nm, tmp, OP.mult, OP.add)

    def exchange(self, fields_bcs, fam, wd=1, fix=True):
        """AllGather halo exchange of `wd` ghost rings for (Fld, bc) pairs."""
        nc = self.nc
        nf = len(fields_bcs)
        f0 = fields_bcs[0][0]
        P, zr, xc, gw = f0.P, f0.zr, f0.xc, f0.gw
        assert wd <= gw
        agin = self.dram.tile([2 * nf, P, zr, wd], F32, tag=f"agin_{fam}", name=f"agin{nc.next_id()}")
        agout = self.dram.tile([NC_ * 2 * nf, P, zr, wd], F32, tag=f"agout_{fam}",
                               name=f"agout{nc.next_id()}", addr_space="Shared")
        for i, (f, bc) in enumerate(fields_bcs):
            d3 = self.D3(f)
            nc.sync.dma_start(agin[2 * i + 0, :, :, :], d3[:, :, gw:gw + wd])
            nc.sync.dma_start(agin[2 * i + 1, :, :, :], d3[:, :, xc - gw - wd:xc - gw])
        nc.gpsimd.collective_compute(
            "AllGather", OP.bypass, replica_groups=[list(range(NC_))],
            ins=[agin.opt()], outs=[agout.opt()])
        regL, regR = self.regs[('f3' if nf == 3 else 'f1')]
        for i, (f, bc) in enumerate(fields_bcs):
            d3 = self.D3(f)
            # left ghost <- (rank-1) slot side=1 of field i; right <- (rank+1) side=0
            nc.sync.dma_start(d3[:, :, gw - wd:gw], agout[bass.ds(regL[i], 1), :, :, :])
            nc.sync.dma_start(d3[:, :, xc - gw:xc - gw + wd], agout[bass.ds(regR[i], 1), :, :, :])
            if fix:
                self.edge_fix(f, bc)

    def prep(self, f, bc, exch_fam=None, fields=None):
        """full ghost prep: z rows then x cols."""
        self.prep_z(f, bc)
        if GEOM[f.lvl][4]:  # sharded
            if exch_fam is not None:
                self.exchange(fields or [(f, bc)], exch_fam)
        else:
            self.prep_x_bc(f, bc)

    # --- compute blocks ---------------------------------------------------
    def jacobi(self, dst, w_in, r, lvl):
        """dst = sum_taps(-wA/diag)*w_nb + (1/diag)*r ; w_in ghosts must be valid.
        z taps on gpsimd, x taps + combine on vector, y taps on PE."""
        nc, C = self.nc, self.C
        xc = w_in.xc
        ps = self.mm(f'JY{lvl}', w_in, w_in.P)
        pst = ps[:, xc: w_in.F - xc]
        gz = self.sub(self.gz0, lvl)
        nc.gpsimd.tensor_add(self.T(gz), self.T(w_in, xc), self.T(w_in, -xc))
        nc.vector.scalar_tensor_tensor(self.T(dst), self.T(w_in, 1), C['jxp'], pst, OP.mult, OP.add)
        nc.vector.scalar_tensor_tensor(self.T(dst), self.T(w_in, -1), C['jxm'], self.T(dst), OP.mult, OP.add)
        nc.vector.scalar_tensor_tensor(self.T(dst), self.T(gz), C['jzp'], self.T(dst), OP.mult, OP.add)
        nc.vector.scalar_tensor_tensor(self.T(dst), self.T(r), C['rb'], self.T(dst), OP.mult, OP.add)

    def residual(self, dst, pd, b):
        """dst = A pd - b at L0; pd ghosts valid."""
        nc, C = self.nc, self.C
        xc = pd.xc
        ps = self.mm('AY0', pd, 128)
        pst = ps[:, xc: pd.F - xc]
        gz = self.gz0
        nc.gpsimd.tensor_add(self.T(gz), self.T(pd, xc), self.T(pd, -xc))
        nc.vector.scalar_tensor_tensor(self.T(dst), self.T(pd, 1), C['wA_xp'], pst, OP.mult, OP.add)
        nc.vector.scalar_tensor_tensor(self.T(dst), self.T(pd, -1), C['wA_xm'], self.T(dst), OP.mult, OP.add)
        nc.vector.scalar_tensor_tensor(self.T(dst), self.T(b), -1.0, self.T(dst), OP.mult, OP.add)
        nc.vector.scalar_tensor_tensor(self.T(dst), self.T(gz), C['wA_zp'], self.T(dst), OP.mult, OP.add)

    def restrict(self, r_f, r_c, lf):
        """r_c (level lf+1) interior = w_res-weighted 2x2x2 sum of r_f (level lf)."""
        nc, C = self.nc, self.C
        g = GEOM[lf]
        P, zr, xc = g[0], g[2], g[3]
        F = zr * xc
        gc = GEOM[lf + 1]
        Pc = gc[0]
        ps = self.mm(f'R{lf}', r_f, Pc)
        t0 = self.gz0.t[0:Pc, 0:F]
        nc.scalar.copy(t0, ps[:, 0:F])
        t1 = self.sx0.t[0:Pc, 0:F]
        t2 = self.tx0.t[0:Pc, 0:F]
        nc.vector.tensor_add(t1[:, 0:F - 1], t0[:, 0:F - 1], t0[:, 1:F])
        nc.vector.tensor_add(t2[:, 0:F - xc - 1], t1[:, 0:F - xc - 1], t1[:, xc:F - 1])
        # strided gather: coarse cells <- fine pair sums.  For lf==0 also
        # produce the coarse x-ghost ring-1 (computable from the extended
        # fine residual) so L1 never needs its own halo exchange.
        zi = gc[2] - 2
        gwf, gwc = GEOM[lf][5], GEOM[lf + 1][5]
        d3c = self.D3(r_c)
        t23 = t2[:, 0:F].rearrange("p (z x) -> p z x", x=xc)
        if lf == 0:
            xi = gc[3] - 2 * gwc + 2        # interior + ghost ring-1 (34)
            c0, f0 = gwc - 1, gwf - 2       # coarse col 1 <- fine cols (2,3)
        else:
            xi = gc[3] - 2 * gwc
            c0, f0 = gwc, gwf
        nc.vector.tensor_scalar_mul(
            d3c[:, 1:1 + zi, c0:c0 + xi],
            t23[:, 1:1 + 2 * zi:2, f0:f0 + 2 * xi:2],
            2.0 * C['wres'])

    def prolong_mm(self, w_c, lf):
        """y-doubling matmul of level lf+1 tile -> psum [P_lf, F_{lf+1}]"""
        return self.mm(f'PR{lf}', w_c, GEOM[lf][0])

    def parity_views(self, ps, lvl_f, dst3):
        """yield (dst_quadrant, psum_quadrant) for the 4 z/x parities."""
        gf, gc = GEOM[lvl_f], GEOM[lvl_f + 1]
        zrf, xcf = gf[2], gf[3]
        zrc, xcc = gc[2], gc[3]
        ps3 = ps[:, 0:zrc * xcc].rearrange("p (z x) -> p z x", x=xcc)
        for pz in (0, 1):
            nzf = (zrf - pz + 1) // 2
            cz = 0 if pz == 0 else 1
            for px in (0, 1):
                nxf = (xcf - px + 1) // 2
                cx = 0 if px == 0 else 1
                yield (dst3[:, pz::2, px::2], ps3[:, cz:cz + nzf, cx:cx + nxf])

    def prolong_copy(self, w_c, w_f, lf):
        """w_f = prolong(w_c) including ghosts (coarse ghosts must be valid)."""
        nc = self.nc
        if lf == 1:
            # extract this core's 20-col x window (incl. both ghost rings) of
            # the replicated L2 field via a padded DRAM bounce (dynamic
            # SBUF-side DMA offsets hang on hardware), y-double via PR1, and
            # expand with per-zh coarse row offsets.  Fine cols {2k, 2k+1}
            # map to window col k.
            d3w2 = self.D3(w_c)
            win3 = self.w2win.t[:, 0:360].rearrange("p (z x) -> p z x", x=20)
            w2d = self.dram.tile([16, 18, 132], F32, tag='w2d', name=f'w2d{nc.next_id()}')
            nc.sync.dma_start(w2d[:, :, 0:130], d3w2[:, :, :])
            nc.sync.dma_start(win3[:, :, :], w2d[:, :, bass.ds(self.reg_w2, 20)])
            ps = self.mm('PR1', Fld(self.w2win.t[:, 0:360], 1), 64, psum_w=360)
            ps3 = ps[:, 0:360].rearrange("p (z x) -> p z x", x=20)
            d3 = self.D3(w_f)
            for zh in (0, 1):
                czh = 8 * zh
                psl = slice(zh * 32, (zh + 1) * 32)
                for pz in (0, 1):
                    nzf = (18 - pz + 1) // 2
                    cz = czh + (0 if pz == 0 else 1)
                    for fx0 in (0, 1):
                        nc.scalar.copy(
                            d3[psl, pz::2, fx0::2],
                            ps3[psl, cz:cz + nzf, 0:18])
            return
        ps = self.prolong_mm(w_c, lf)
        d3 = self.D3(w_f)
        for dq, pq in self.parity_views(ps, lf, d3):
            self.nc.scalar.copy(dq, pq)

    def prolong_sub(self, w_c, pd_old, pd_new, lf):
        """pd_new = pd_old - prolong(w_c), covering interior + ghost rings 1-2.
        Fine level 0 has gw=3: fine col c maps to coarse col (c-3)//2+1;
        cols {1,3,..,67} <- coarse 0..33, cols {2,4,..,68} <- coarse 0..33."""
        assert lf == 0
        ps = self.prolong_mm(w_c, lf)
        gf, gc = GEOM[lf], GEOM[lf + 1]
        zrf, xcf = gf[2], gf[3]
        zrc, xcc = gc[2], gc[3]
        ps3 = ps[:, 0:zrc * xcc].rearrange("p (z x) -> p z x", x=xcc)
        d3n, d3o = self.D3(pd_new), self.D3(pd_old)
        for pz in (0, 1):
            nzf = (zrf - pz + 1) // 2
            cz = 0 if pz == 0 else 1
            for fx0 in (0, 1):
                dq = d3n[:, pz::2, fx0::2]
                oq = d3o[:, pz::2, fx0::2]
                pq = ps3[:, cz:cz + nzf, 0:36]
                self.nc.vector.scalar_tensor_tensor(dq, pq, -1.0, oq, OP.mult, OP.add)

    def dbg_dump(self, name, f):
        if self.dbg_name == name and not self.dbg_written:
            self.nc.sync.dma_start(self.p_dbg[0:f.P, 0:f.F], self.V(f))
            self.dbg_written = True

    # --- main build -------------------------------------------------------
    def build(self):
        nc, C = self.nc, self.C
        with TileContext(nc) as tc:
            with tc.tile_pool(name="main", bufs=1) as pool, \
                 tc.tile_pool(name="psum", bufs=1, space="PSUM") as psum_pool, \
                 tc.tile_pool(name="dram", bufs=1, space="DRAM") as dram:
                self.pool, self.psum_pool, self.dram = pool, psum_pool, dram
                self._build_body(tc)
        nc.finalize()
        return nc

    def _load_fld(self, pname, name, lvl, tag=None, engs=None):
        nc = self.nc
        f = self.fld(name, lvl, tag=tag)
        d3 = self.D3(f)
        src = self.p_in[pname][:, :, :].transpose([1, 0, 2])
        engs = engs or (nc.sync, nc.sync)
        for zh in (0, 1):
            engs[zh].dma_start(d3[zh * 64:(zh + 1) * 64, 1:33, 4:68], src[:, zh * 32:(zh + 1) * 32, :])
        return f

    def _store_fld(self, f, ch, engs=None):
        nc = self.nc
        d3 = self.D3(f)
        dst = self.p_out[:, :, :, :].transpose([0, 2, 1, 3])
        engs = engs or (nc.sync, nc.sync)
        for zh in (0, 1):
            engs[zh].dma_start(dst[ch, :, zh * 32:(zh + 1) * 32, :], d3[zh * 64:(zh + 1) * 64, 1:33, 4:68])

    def _build_body(self, tc):
        nc, C = self.nc, self.C
        pool = self.pool
        if self.stage == 'io0':
            t = self._load_fld('values_u', 'u', 0)
            for ch in range(4):
                self._store_fld(t, ch)
            return
        # constants / matrices / masks
        self.mats_t = pool.tile([128, self.MC], F32, tag="mats", name="mats_t")
        nc.sync.dma_start(self.mats_t[:, :], self.p_mats[:, :])
        self.masks_t = pool.tile([128, 4], F32, tag="masks", name="masks_t")
        nc.sync.dma_start(self.masks_t[:, :], self.p_masks[:, :])
        hoffs_t = pool.tile([1, 9], I32, tag="hoffs", name="hoffs_t")
        nc.sync.dma_start(hoffs_t[:, :], self.p_hoffs[:, :])
        if self.stage == 'io1':  # preamble DMAs only, no value_loads
            t = self._load_fld('values_u', 'u', 0)
            for ch in range(4):
                self._store_fld(t, ch)
            return
        # slot index registers for halo unpack (gpsimd issues those DMAs)
        rf3L = [nc.sync.value_load(hoffs_t[0:1, i:i + 1], min_val=None, max_val=None) for i in range(3)]
        rf3R = [nc.sync.value_load(hoffs_t[0:1, 3 + i:4 + i], min_val=None, max_val=None) for i in range(3)]
        rf1L = nc.sync.value_load(hoffs_t[0:1, 6:7], min_val=None, max_val=None)
        rf1R = nc.sync.value_load(hoffs_t[0:1, 7:8], min_val=None, max_val=None)
        self.reg_w2 = nc.sync.value_load(hoffs_t[0:1, 8:9], min_val=None, max_val=None)
        self.regs = {'f3': (rf3L, rf3R), 'f1': ([rf1L], [rf1R])}

        if self.stage == 'io':
            t = self._load_fld('values_u', 'u', 0)
            for ch in range(4):
                self._store_fld(t, ch)
            return

        # ---- loads (spread across engine DMA queues)
        u = self._load_fld('values_u', 'u', 0, engs=(nc.sync, nc.scalar))
        v = self._load_fld('values_v', 'v', 0, engs=(nc.gpsimd, nc.sync))
        w = self._load_fld('values_w', 'w', 0, engs=(nc.scalar, nc.gpsimd))
        a = self._load_fld('alpha', 'a', 0, engs=(nc.sync, nc.scalar))

        # ---- rho chain (interior only needed); a -> rho -> rinv in place
        buoy = self.fld('buoy', 0)
        rP1 = self.fld('rP1', 0)
        rP2 = self.fld('rP2', 0)
        nc.vector.tensor_scalar(self.T(a), self.T(a), 0.05, 1.0, OP.max, OP.min)
        nc.vector.tensor_scalar(self.T(a), self.T(a), RHO_L - RHO_G, RHO_G, OP.mult, OP.add)
        rinv = a
        nc.vector.reciprocal(self.T(rinv), self.T(a))
        nc.vector.tensor_scalar(self.T(buoy), self.T(rinv), -DT * G_Z * RHO_L, DT * G_Z, OP.mult, OP.add)
        nc.scalar.mul(self.T(rP1), self.T(rinv), DT * C['axp'])
        nc.scalar.mul(self.T(rP2), self.T(rinv), DT)

        # scratch + scaled advection multipliers (tiles also reused later)
        self.ebc_t = pool.tile([128, 34], F32, tag='ebc', name='ebc_t')
        # scratch sized for the largest level (L0 is 34*72=2448)
        self.gz0 = Fld(pool.tile([128, 2448], F32, tag='gz0', name='gz0'), 0)
        self.sx0 = Fld(pool.tile([128, 2448], F32, tag='sx0', name='sx0'), 0)
        self.tx0 = Fld(pool.tile([128, 2448], F32, tag='tx0', name='tx0'), 0)
        if self.dbg_init:
            for t_ in (self.gz0.t, self.sx0.t, self.tx0.t, self.ebc_t):
                self.nc.vector.memset(t_[:, :], 0.0)
        ut1 = self.fld('ut1', 0)     # x-family: -DT*axp*u
        wt1 = self.fld('wt1', 0)     # z-family: -DT*azp*w
        vt2 = self.fld('vt2', 0)     # y-family: -DT*v
        nc.scalar.mul(self.T(ut1), self.T(u), -DT * C['axp'])
        nc.scalar.mul(self.T(wt1), self.T(w), -DT * C['azp'])
        nc.scalar.mul(self.T(vt2), self.T(v), -DT)

        # ---- early pd load + width-3 exchange (overlaps predictor compute)
        pdA = Fld(u.t, 0)  # u dead after predictor taps; load ordered by Tile
        # NOTE: pd must NOT reuse u's tile anymore since the pd exchange now
        # happens while u is still live; use a dedicated allocation order:
        # pd0 lives in 'pd0' tile and pdA rotation uses u's tile later.
        pd0 = self.fld('pd0', 0)
        d3pd0 = self.D3(pd0)
        srcpd = self.p_in['values_pd'][:, :, :].transpose([1, 0, 2])
        for zh in (0, 1):
            nc.gpsimd.dma_start(d3pd0[zh * 64:(zh + 1) * 64, 1:33, 4:68], srcpd[:, zh * 32:(zh + 1) * 32, :])
        self.prep_z(pd0, BC_PD)

        # ---- predictor: ghosts for u,v,w then stars
        for f, bc in ((u, BC_U), (v, BC_V), (w, BC_W)):
            self.prep_z(f, bc)
        self.exchange([(u, BC_U), (v, BC_V), (w, BC_W)], 'f3')
        self.exchange([(pd0, BC_PD)], 'f1', wd=4)

        xc = u.xc
        us, vs, ws = self.fld('us', 0), self.fld('vs', 0), self.fld('ws', 0)
        sx, tx, gz = self.sx0, self.tx0, self.gz0
        for f, bc, dst, extra in ((u, BC_U, us, None), (v, BC_V, vs, None), (w, BC_W, ws, buoy)):
            nm = 'u' if f is u else ('v' if f is v else 'w')
            # PE matmuls park eagerly into SBUF via ACT so the single PSUM
            # slot frees up and all six predictor matmuls pipeline during
            # the halo exchange (they need no x-ghosts).
            ps = self.mm('MD_' + nm, f, 128)
            nc.scalar.copy(self.T(dst), ps[:, xc: f.F - xc])
            ps2 = self.mm('DY_' + nm, f, 128)
            nc.scalar.copy(self.T(tx), ps2[:, xc: f.F - xc])
            # gpsimd: z diffusion tap sum + z advection (TT ops only on Pool)
            nc.gpsimd.tensor_add(self.T(gz), self.T(f, xc), self.T(f, -xc))
            nc.gpsimd.tensor_sub(self.T(sx), self.T(f, xc), self.T(f, -xc))
            nc.gpsimd.tensor_mul(self.T(sx), self.T(sx), self.T(wt1))
            # vector: y advection first (frees tx), then x taps/advection
            nc.vector.tensor_mul(self.T(tx), self.T(tx), self.T(vt2))
            nc.vector.tensor_add(self.T(dst), self.T(dst), self.T(tx))
            nc.vector.tensor_sub(self.T(tx), self.T(f, 1), self.T(f, -1))
            nc.vector.tensor_mul(self.T(tx), self.T(tx), self.T(ut1))
            nc.vector.tensor_add(self.T(dst), self.T(dst), self.T(tx))
            nc.vector.scalar_tensor_tensor(self.T(dst), self.T(f, 1), DT * C['wd_xp'], self.T(dst), OP.mult, OP.add)
            nc.vector.scalar_tensor_tensor(self.T(dst), self.T(f, -1), DT * C['wd_xm'], self.T(dst), OP.mult, OP.add)
            nc.vector.scalar_tensor_tensor(self.T(dst), self.T(gz), DT * C['wd_zp'], self.T(dst), OP.mult, OP.add)
            nc.vector.tensor_add(self.T(dst), self.T(dst), self.T(sx))
            if extra is not None:
                nc.vector.tensor_add(self.T(dst), self.T(dst), self.T(extra))
        self.dbg_dump('us', us)
        self.dbg_dump('vs', vs)
        self.dbg_dump('ws', ws)
        if self.stage == 'pred':
            self._store_fld(us, 0)
            self._store_fld(vs, 1)
            self._store_fld(ws, 2)
            self._store_fld(ws, 3)
            return

        # ---- divergence -> b
        for f, bc in ((us, BC_U), (vs, BC_V), (ws, BC_W)):
            self.prep_z(f, bc)
        self.exchange([(us, BC_U), (vs, BC_V), (ws, BC_W)], 'f3')
        b = Fld(buoy.t, 0)  # buoy dead after ws
        cbx = -(DX * DX / DT) * C['axp']
        cbz = -(DX * DX / DT) * C['azp']
        ps = self.mm('DY_v', vs, 128)
        pst = ps[:, xc: vs.F - xc]
        nc.vector.tensor_scalar_mul(self.T(b), pst, -(DX * DX / DT))
        nc.vector.scalar_tensor_tensor(self.T(b), self.T(us, 1), cbx, self.T(b), OP.mult, OP.add)
        nc.vector.scalar_tensor_tensor(self.T(b), self.T(us, -1), -cbx, self.T(b), OP.mult, OP.add)
        nc.gpsimd.tensor_sub(self.T(sx), self.T(ws, xc), self.T(ws, -xc))
        nc.vector.scalar_tensor_tensor(self.T(b), self.T(sx), cbz, self.T(b), OP.mult, OP.add)
        self.exchange([(b, BC_PD)], 'f1', wd=3, fix=False)
        self.dbg_dump('b', b)
        if self.stage == 'div':
            self._store_fld(us, 0)
            self._store_fld(vs, 1)
            self._store_fld(ws, 2)
            self._store_fld(b, 3)
            return

        # ---- multigrid: 2 V-cycles
        # coarse-level tiles live inside v/w (dead after the predictor)
        r1 = Fld(v.t[0:64, 0:648], 1)
        wp1_ = Fld(v.t[0:64, 648:1296], 1)
        wsm1_ = Fld(v.t[0:64, 1296:1944], 1)
        r2l = v.t[0:32, 1944:2072]
        self.w2win = Fld(v.t[0:16, 2072:2432], 1)
        r2 = Fld(w.t[0:16, 0:2340], 2)
        pdA = Fld(u.t, 0)    # u dead after predictor
        pdB = Fld(wt1.t, 0)  # wt1 dead after predictor
        pdC = Fld(rinv.t, 0)  # rinv(=a) dead after rP1/rP2/buoy
        r0 = Fld(vt2.t, 0)   # vt2 dead after predictor
        r3 = self.fld('r3', 3)
        r4 = self.fld('r4', 4)
        r5 = self.fld('r5', 5)
        r6 = self.fld('r6', 6)
        wts = {l: (self.fld(f'wp{l}', l), self.fld(f'wsm{l}', l)) for l in range(2, 6)}
        wts[1] = (wp1_, wsm1_)
        w6 = self.fld('w6', 6)

        pd_cur = pd0
        rot = [pdB, pdC, pdA]
        ri = 0
        for vc in range(2):
            # pre-smooth (vc0: pd0 already exchanged early, overlapped with
            # the predictor; vc1: exchange here)
            if vc > 0:
                self.prep_z(pd_cur, BC_PD)
                self.exchange([(pd_cur, BC_PD)], 'f1', wd=4)
            if self.stage == 'exch1' and vc == 0:
                for ch in range(4):
                    self._store_fld(pd_cur, ch)
                return
            pd1 = rot[ri % 3]; ri += 1
            self.jacobi(pd1, pd_cur, b, 0)
            self.edge_fix(pd1, BC_PD)
            if self.stage == 'jac1' and vc == 0:
                for ch in range(4):
                    self._store_fld(pd1, ch)
                return
            # residual: pd1 ghost ring-1 is valid from the extended pre-smooth
            self.prep_z(pd1, BC_PD)
            self.residual(r0, pd1, b)
            if self.stage == 'resid' and vc == 0:
                for ch in range(4):
                    self._store_fld(pd1, ch)
                return
            if self.stage == 'resid2' and vc == 0:
                nc.vector.scalar_tensor_tensor(self.T(pd1), self.T(r0), 0.0, self.T(pd1), OP.mult, OP.add)
                for ch in range(4):
                    self._store_fld(pd1, ch)
                return
            # restrict chain
            self.restrict(r0, r1, 0)
            if self.stage == 'rst1' and vc == 0:
                for ch in range(4):
                    self._store_fld(pd1, ch)
                return
            # r1 -> r2 local window, then allgather full r2
            self._restrict_r1_to_r2(r1, r2l, r2)
            if self.stage == 'rst2' and vc == 0:
                for ch in range(4):
                    self._store_fld(pd1, ch)
                return
            self.restrict(r2, r3, 2)
            self.restrict(r3, r4, 3)
            self.restrict(r4, r5, 4)
            self.restrict(r5, r6, 5)
            # coarsest: w6 = r6 / diag
            nc.scalar.mul(self.V(w6), self.V(r6), C['rb'])
            self.prep_z(w6, BC_PD)
            self.prep_x_bc(w6, BC_PD)
            if self.stage == 'crs' and vc == 0:
                for ch in range(4):
                    self._store_fld(pd1, ch)
                return
            # up-leg 5..1
            wc = w6
            for l in range(5, 0, -1):
                wp, wsm = wts[l]
                self.prolong_copy(wc, wp, l)
                self.jacobi(wsm, wp, (r1 if l == 1 else (r2, r3, r4, r5)[l - 2]), l)
                self.prep_z(wsm, BC_PD)
                if l >= 2:
                    self.prep_x_bc(wsm, BC_PD)
                wc = wsm
                if self.stage == f'up{l}' and vc == 0:
                    for ch in range(4):
                        self._store_fld(pd1, ch)
                    return
            # correction + post-smooth
            pd2 = rot[ri % 3]; ri += 1
            self.prolong_sub(wc, pd1, pd2, 0)
            if self.stage == 'corr' and vc == 0:
                for ch in range(4):
                    self._store_fld(pd2, ch)
                return
            pd3 = rot[ri % 3]; ri += 1
            self.jacobi(pd3, pd2, b, 0)
            self.edge_fix(pd3, BC_PD)
            pd_cur = pd3
            self.dbg_dump(f'pd_vc{vc}', pd3)
            if self.stage == 'vc1' and vc == 0:
                self._store_fld(us, 0)
                self._store_fld(vs, 1)
                self._store_fld(ws, 2)
                self._store_fld(pd_cur, 3)
                return
            if self.stage == 'presm' and vc == 0:
                self._store_fld(pd1, 0)
                self._store_fld(r0, 1)
                self._store_fld(r0, 2)
                self._store_fld(pd1, 3)
                return
            if self.stage == 'presmB' and vc == 0:
                for ch in range(4):
                    self._store_fld(r0, ch)
                return

        # ---- projection: pd ghost ring-1 already valid (extended post-smooth)
        self.prep_z(pd_cur, BC_PD)
        ps = self.mm('DY_pd', pd_cur, 128)
        pst = ps[:, xc: pd_cur.F - xc]
        nc.vector.tensor_mul(self.T(tx), pst, self.T(rP2))
        nc.vector.tensor_sub(self.T(vs), self.T(vs), self.T(tx))
        nc.vector.tensor_sub(self.T(sx), self.T(pd_cur, 1), self.T(pd_cur, -1))
        nc.vector.tensor_mul(self.T(sx), self.T(sx), self.T(rP1))
        nc.vector.tensor_sub(self.T(us), self.T(us), self.T(sx))
        nc.gpsimd.tensor_sub(self.T(gz), self.T(pd_cur, xc), self.T(pd_cur, -xc))
        nc.gpsimd.tensor_mul(self.T(gz), self.T(gz), self.T(rP1))
        nc.vector.tensor_sub(self.T(ws), self.T(ws), self.T(gz))

        # ---- store (spread across engine DMA queues)
        self._store_fld(us, 0, engs=(nc.sync, nc.scalar))
        self._store_fld(vs, 1, engs=(nc.gpsimd, nc.sync))
        self._store_fld(ws, 2, engs=(nc.scalar, nc.gpsimd))
        self._store_fld(pd_cur, 3, engs=(nc.sync, nc.scalar))

    def _restrict_r1_to_r2(self, r1, r2l, r2):
        """restrict sharded r1 -> local coarse window, allgather -> full r2."""
        nc, C = self.nc, self.C
        g = GEOM[1]
        P, zr, xc = g[0], g[2], g[3]
        F = zr * xc
        ps = self.mm('R1', r1, P // 2)
        t0 = self.gz0.t[0:P // 2, 0:F]
        nc.scalar.copy(t0, ps[:, 0:F])
        t1 = self.sx0.t[0:P // 2, 0:F]
        t2 = self.tx0.t[0:P // 2, 0:F]
        nc.vector.tensor_add(t1[:, 0:F - 1], t0[:, 0:F - 1], t0[:, 1:F])
        nc.vector.tensor_add(t2[:, 0:F - xc - 1], t1[:, 0:F - xc - 1], t1[:, xc:F - 1])
        t23 = t2[:, 0:F].rearrange("p (z x) -> p z x", x=xc)
        # compact local window [32p, 8z * 16x]
        r2l3 = r2l[:, 0:128].rearrange("p (z x) -> p z x", x=16)
        nc.vector.tensor_scalar_mul(r2l3[:, :, :], t23[:, 1:17:2, 2:34:2], 2.0 * C['wres'])
        agin = self.dram.tile([1, 32 * 128], F32, tag='agr2i', name=f'agr2i{nc.next_id()}')
        agout = self.dram.tile([NC_, 2, 16, 8, 16], F32, tag='agr2o', name=f'agr2o{nc.next_id()}',
                               addr_space="Shared")
        nc.sync.dma_start(agin[0:1, :], r2l[:, 0:128])
        nc.gpsimd.collective_compute(
            "AllGather", OP.bypass, replica_groups=[list(range(NC_))],
            ins=[agin.opt()], outs=[agout.opt()])
        d3 = self.D3(r2)
        for s in range(NC_):
            for zh in (0, 1):
                nc.sync.dma_start(
                    d3[:, 1 + zh * 8:9 + zh * 8, 1 + s * 16:17 + s * 16],
                    agout[s, zh, :, :, :])


# ---------------------------------------------------------------- entry
_CACHE = {}


def _get_nc(key, C, dbg_name=None, stage='full', dbg_init=False):
    ck = (key, dbg_name, stage, dbg_init)
    if ck not in _CACHE:
        mats_np, cols = build_mats(C)
        b = B(C, mats_np, cols, dbg_name=dbg_name, stage=stage, dbg_init=dbg_init)
        nc = b.build()
        _CACHE[ck] = (nc, mats_np)
    return _CACHE[ck]


def _make_in_maps(fields, mats_np):
    in_maps = []
    for r in range(NC_):
        m = {}
        for nm, arr in fields.items():
            m[nm] = np.ascontiguousarray(np.asarray(arr, np.float32)[0, 0, :, :, r * XL:(r + 1) * XL])
        m['mats'] = mats_np
        msk = np.zeros((128, 4), np.float32)
        msk[:, 0] = 1.0 if r == 0 else 0.0       # mL
        msk[:, 1] = 0.0 if r == 0 else 1.0       # nmL
        msk[:, 2] = 1.0 if r == NC_ - 1 else 0.0  # mR
        msk[:, 3] = 0.0 if r == NC_ - 1 else 1.0  # nmR
        m['masks'] = msk
        ho = np.zeros((1, 9), np.int32)
        rl = max(r - 1, 0)
        rr = min(r + 1, NC_ - 1)
        for i in range(3):
            ho[0, i] = rl * 6 + i * 2 + 1      # left ghost <- left nbr, right plane
            ho[0, 3 + i] = rr * 6 + i * 2 + 0  # right ghost <- right nbr, left plane
        ho[0, 6] = rl * 2 + 1
        ho[0, 7] = rr * 2 + 0
        ho[0, 8] = r * 16
        m['hoffs'] = ho
        in_maps.append(m)
    return in_maps


def kernel(alpha, values_u, values_v, values_w, values_pd,
           w_diff, w_xadv, w_yadv, w_zadv, w_A, w_res, _dbg=None, _stage='full', _dbg_init=False):
    C = extract_consts(w_diff, w_xadv, w_yadv, w_zadv, w_A, w_res)
    key = tuple(sorted(C.items()))
    nc, mats_np = _get_nc(key, C, dbg_name=_dbg, stage=_stage, dbg_init=_dbg_init)
    fields = {'alpha': alpha, 'values_u': values_u, 'values_v': values_v,
              'values_w': values_w, 'values_pd': values_pd}
    in_maps = _make_in_maps(fields, mats_np)
    res = run_bass_kernel_spmd(nc, in_maps, core_ids=list(range(NC_)))
    outs = [res.results[r]['out'] for r in range(NC_)]
    full = np.concatenate(outs, axis=3)  # [4, 64, 64, 512]
    if _dbg is not None:
        kernel._dbg_res = [res.results[r].get('dbg') for r in range(NC_)]
    return full[None]  # (1, 4, 64, 64, 512)



# revision 5
# speedup vs baseline: 1.8213x; 1.8213x over previous
"""Trainium2 Bass kernel for the multiphase CFD fractional-step solver
(predictor + divergence + 2 multigrid V-cycles + projection) on a
64x64x512 grid, sharded along x across 8 NeuronCores.

Self-contained: hardcodes shapes/sharding; reads stencil coefficient
VALUES from the runtime weight inputs and compiles a specialized graph
(cached per coefficient set).

Device layout (level l):
  partitions p = zh*ny + y   (zh in {0,1} z-halves; l=6 has P=1)
  free       j = z'*xc + x   (z' in [0, zr): rows 0 and zr-1 are z-ghosts;
                              x in [0, xc): cols 0 and xc-1 are x-ghosts)
Volume passes run on the row-trimmed flat range [xc, F-xc) so all
+-1 / +-xc shifted reads stay inside the [P, F] tile.
y-axis stencil taps (partition axis) are done on the TensorEngine as
[K,M] matmuls with per-field boundary rows baked into the matrices.

I/O: the host pre-pads each field into the device tile layout
[128, zr*xc] so every load/store is a single fully-contiguous DMA
(128 descriptors) instead of a strided descriptor storm.

x halos between cores travel through AllGathers; ghost columns are
packed into a contiguous staging tile with cheap engine copies so the
DMA legs are contiguous.  One merged exchange up front carries
u,v,w (wd2) + alpha (wd1) + pd (wd4), which lets the predictor produce
star velocities valid through ghost ring 1 and removes the separate
post-predictor exchange entirely.  Remaining collectives: b (wd3),
replicated-r2 gather per V-cycle, pd re-exchange before V-cycle 2.
Multigrid levels 0-1 stay sharded; level 2 residual is allgathered and
levels 2-6 are computed replicated on every core.
"""
import sys
sys.path.insert(0, '/opt/trn_rl_repo')
import numpy as np
import concourse.bass as bass
import concourse.bacc as bacc
import concourse.mybir as mybir
from concourse.bass_utils import run_bass_kernel_spmd
from concourse.tile import TileContext

F32 = mybir.dt.float32
I32 = mybir.dt.int32
OP = mybir.AluOpType

DT, DX, G_Z = 0.002, 0.04, -10.0
RHO_L, RHO_G, NU = 1000.0, 1.0, 1e-3
NZ, NY, NX = 64, 64, 512
NC_ = 8
XL = NX // NC_  # 64 local x

# level: (P, ny, zr, xc, sharded, gw) -- gw = x-ghost cols per side
GEOM = {
    0: (128, 64, 34, 72, True, 4),
    1: (64, 32, 18, 36, True, 2),
    2: (16, 16, 18, 130, False, 1),
    3: (8, 8, 10, 66, False, 1),
    4: (4, 4, 6, 34, False, 1),
    5: (2, 2, 4, 18, False, 1),
    6: (1, 1, 3, 10, False, 1),
}
# BC per field: axis -> (lo, hi), 'n' neumann (ghost=adjacent), 'd' dirichlet (ghost=0)
BC_U = {'z': ('n', 'n'), 'y': ('n', 'n'), 'x': ('d', 'd')}
BC_V = {'z': ('n', 'n'), 'y': ('d', 'd'), 'x': ('n', 'n')}
BC_W = {'z': ('d', 'd'), 'y': ('n', 'n'), 'x': ('n', 'n')}
BC_PD = {'z': ('n', 'd'), 'y': ('n', 'n'), 'x': ('n', 'n')}
BC_A = {'z': ('n', 'n'), 'y': ('n', 'n'), 'x': ('n', 'n')}


# ---------------------------------------------------------------- host-side
def _yblock(ny, cm, cc, cp, bc):
    """[ny, ny] matrix M with out[y] = sum_k M[k, y] in[k]:
    tridiag with sub=cm (coeff of in[y-1]), diag=cc, super=cp (in[y+1]),
    Neumann BC folds the ghost coeff into the boundary diagonal."""
    m = np.zeros((ny, ny), np.float32)
    for y in range(ny):
        m[y, y] += cc
        if y > 0:
            m[y - 1, y] += cm
        elif bc[0] == 'n':
            m[y, y] += cm
        if y < ny - 1:
            m[y + 1, y] += cp
        elif bc[1] == 'n':
            m[y, y] += cp
    return m


def _blkdiag2(b):
    n = b.shape[0]
    m = np.zeros((2 * n, 2 * b.shape[1]), np.float32)
    m[:n, :b.shape[1]] = b
    m[n:, b.shape[1]:] = b
    return m


def _halve(ny):
    m = np.zeros((ny, ny // 2), np.float32)
    for y in range(ny):
        m[y, y // 2] = 0.5
    return m


def _double(nyc, nyf):
    m = np.zeros((nyc, nyf), np.float32)
    for y in range(nyf):
        m[y // 2, y] = 1.0
    return m


def build_mats(C):
    """Concatenated [128, sum M] lhsT matrices + column offset map."""
    cols = {}
    parts = []
    total = 0

    def add(name, m, K):
        nonlocal total
        assert m.shape[0] == K and K <= 128 and m.shape[1] <= 128
        buf = np.zeros((128, m.shape[1]), np.float32)
        buf[:K] = m
        cols[name] = (total, m.shape[1], K)
        parts.append(buf)
        total += m.shape[1]

    # predictor diffusion y-taps + center (K=M=128, blockdiag over zh)
    for nm, bc in (('u', BC_U), ('v', BC_V), ('w', BC_W)):
        b = _yblock(64, DT * C['wd_ym'], 1.0 + DT * C['wd_c'], DT * C['wd_yp'], bc['y'])
        add('MD_' + nm, _blkdiag2(b), 128)
    # advection / gradient y-difference (raw tap values)
    for nm, bc in (('u', BC_U), ('v', BC_V), ('w', BC_W), ('pd', BC_PD)):
        b = _yblock(64, C['aym'], 0.0, C['ayp'], bc['y'])
        add('DY_' + nm, _blkdiag2(b), 128)
    # residual y-taps + center at L0
    b = _yblock(64, C['wA_ym'], C['wA_c'], C['wA_yp'], BC_PD['y'])
    add('AY0', _blkdiag2(b), 128)
    # jacobi y matrices per level 0..5 (levels 0-1 are (zh,y) block-diag)
    for l in range(6):
        P, ny = GEOM[l][0], GEOM[l][1]
        b = _yblock(ny, C['jym'], 0.0, C['jyp'], BC_PD['y'])
        add(f'JY{l}', _blkdiag2(b) if l <= 1 else b, P)
    # restrict y-halving matrices (R{l}: level l -> l+1)
    add('R0', _blkdiag2(_halve(64)), 128)
    add('R1', _blkdiag2(_halve(32)), 64)   # stays (zh,y); zh dissolved in AG unpack
    for l in range(2, 6):
        add(f'R{l}', _halve(GEOM[l][1]), GEOM[l][0])
    # prolong y-doubling: PR{l} maps level l+1 -> l
    add('PR0', _blkdiag2(_double(32, 64)), 64)
    m = _double(16, 32)
    add('PR1', np.concatenate([m, m], axis=1), 16)  # s=1 coarse -> (zh,y) fine
    for l in range(2, 6):
        add(f'PR{l}', _double(GEOM[l + 1][1], GEOM[l][1]), GEOM[l + 1][0])

    return np.concatenate(parts, axis=1), cols


def extract_consts(w_diff, w_xadv, w_yadv, w_zadv, w_A, w_res):
    g = lambda a, i, j, k: float(np.asarray(a)[0, 0, i, j, k])
    C = {}
    C['wd_c'] = g(w_diff, 1, 1, 1)
    C['wd_zm'], C['wd_zp'] = g(w_diff, 0, 1, 1), g(w_diff, 2, 1, 1)
    C['wd_ym'], C['wd_yp'] = g(w_diff, 1, 0, 1), g(w_diff, 1, 2, 1)
    C['wd_xm'], C['wd_xp'] = g(w_diff, 1, 1, 0), g(w_diff, 1, 1, 2)
    C['wA_c'] = g(w_A, 1, 1, 1)
    C['wA_zm'], C['wA_zp'] = g(w_A, 0, 1, 1), g(w_A, 2, 1, 1)
    C['wA_ym'], C['wA_yp'] = g(w_A, 1, 0, 1), g(w_A, 1, 2, 1)
    C['wA_xm'], C['wA_xp'] = g(w_A, 1, 1, 0), g(w_A, 1, 1, 2)
    C['axp'], C['axm'] = g(w_xadv, 1, 1, 2), g(w_xadv, 1, 1, 0)
    C['ayp'], C['aym'] = g(w_yadv, 1, 2, 1), g(w_yadv, 1, 0, 1)
    C['azp'], C['azm'] = g(w_zadv, 2, 1, 1), g(w_zadv, 0, 1, 1)
    wr = np.asarray(w_res).ravel()
    assert np.allclose(wr, wr[0]), "nonuniform w_res unsupported"
    C['wres'] = float(wr[0])
    # fast paths used by the kernel
    assert abs(C['axm'] + C['axp']) < 1e-12 * max(1, abs(C['axp']))
    assert abs(C['azm'] + C['azp']) < 1e-12 * max(1, abs(C['azp']))
    # z coefficient symmetry (pool engine computes unscaled zp+zm sums)
    assert abs(C['wd_zm'] - C['wd_zp']) < 1e-12 * max(1, abs(C['wd_zp']))
    assert abs(C['wA_zm'] - C['wA_zp']) < 1e-12 * max(1, abs(C['wA_zp']))
    diag = C['wA_c']
    C['diag'] = diag
    C['jzm'], C['jzp'] = -C['wA_zm'] / diag, -C['wA_zp'] / diag
    C['jym'], C['jyp'] = -C['wA_ym'] / diag, -C['wA_yp'] / diag
    C['jxm'], C['jxp'] = -C['wA_xm'] / diag, -C['wA_xp'] / diag
    C['rb'] = 1.0 / diag
    return C


# ---------------------------------------------------------------- builder
class Fld:
    def __init__(self, t, lvl):
        self.t, self.lvl = t, lvl
        P, ny, zr, xc, _, gw = GEOM[lvl]
        self.P, self.zr, self.xc, self.F, self.gw = P, zr, xc, zr * xc, gw


class B:
    """Builder context."""

    def __init__(self, C, mats_np, mat_cols, dbg_name=None, stage='full', dbg_init=False):
        self.C = C
        self.stage = stage
        self.dbg_init = dbg_init
        self.dbg_name = dbg_name
        self.nc = bacc.Bacc()
        nc = self.nc
        self.mat_cols = mat_cols
        self.MC = mats_np.shape[1]
        # params (fields are pre-padded on host into the device tile layout)
        self.p_in = {}
        for nm in ('alpha', 'values_u', 'values_v', 'values_w', 'values_pd'):
            self.p_in[nm] = nc.declare_dram_parameter(nm, [128, GEOM[0][2] * GEOM[0][3]], F32, isOutput=False)
        self.p_mats = nc.declare_dram_parameter('mats', [128, self.MC], F32, isOutput=False)
        self.p_masks = nc.declare_dram_parameter('masks', [128, 4], F32, isOutput=False)
        self.p_hoffs = nc.declare_dram_parameter('hoffs', [1, 3], I32, isOutput=False)
        self.p_out = nc.declare_dram_parameter('out', [4, 128, GEOM[0][2] * GEOM[0][3]], F32, isOutput=True)
        if dbg_name:
            self.p_dbg = nc.declare_dram_parameter('dbg', [128, GEOM[0][2] * GEOM[0][3]], F32, isOutput=True)
        self.dbg_written = False

    # --- tile helpers -----------------------------------------------------
    def fld(self, name, lvl, tag=None):
        g = GEOM[lvl]
        t = self.pool.tile([g[0], g[2] * g[3]], F32, tag=(tag or name), name=name)
        if self.dbg_init:
            self.nc.vector.memset(t[:, :], 0.0)
        return Fld(t, lvl)

    def sub(self, f, lvl):
        g = GEOM[lvl]
        return Fld(f.t[0:g[0], 0:g[2] * g[3]], lvl)

    def T(self, f, s=0):
        """row-trimmed shifted flat view [P, F-2*xc]"""
        return f.t[:, f.xc + s: f.F - f.xc + s]

    def V(self, f):
        return f.t[:, 0:f.F]

    def D3(self, f):
        return f.t[:, 0:f.F].rearrange("p (z x) -> p z x", x=f.xc)

    def mat(self, name):
        off, M, K = self.mat_cols[name]
        return self.mats_t[0:K, off:off + M]

    def mm(self, name, rhs_f, Pout, psum_w=None):
        """psum[Pout, F] = mats[name].T @ V(rhs)  (chunked, full width)"""
        nc = self.nc
        F = psum_w or rhs_f.F
        ps = self.psum_pool.tile([Pout, F], F32, tag="psA", name=f"ps_{name}_{nc.next_id()}")
        rhs = rhs_f.t[:, 0:F]
        lhsT = self.mat(name)
        for c0 in range(0, F, 512):
            w = min(512, F - c0)
            nc.tensor.matmul(ps[:, c0:c0 + w], lhsT, rhs[:, c0:c0 + w], start=True, stop=True)
        return ps

    # --- ghost prep -------------------------------------------------------
    def prep_z(self, f, bc):
        """fill z ghost rows: global BC rows (+ inter-half swap on levels 0-1)"""
        nc, d3 = self.nc, self.D3(f)
        P, zr = f.P, f.zr
        split = f.lvl <= 1
        lo = slice(0, P // 2) if split else slice(0, P)
        hi = slice(P // 2, P) if split else slice(0, P)
        if bc['z'][0] == 'n':
            nc.scalar.copy(d3[lo, 0, :], d3[lo, 1, :])
        else:
            nc.gpsimd.memset(d3[lo, 0, :], 0.0)
        if bc['z'][1] == 'n':
            nc.scalar.copy(d3[hi, zr - 1, :], d3[hi, zr - 2, :])
        else:
            nc.gpsimd.memset(d3[hi, zr - 1, :], 0.0)
        if split:
            nc.sync.dma_start(d3[lo, zr - 1, :], d3[hi, 1, :])
            nc.sync.dma_start(d3[hi, 0, :], d3[lo, zr - 2, :])

    def prep_x_bc(self, f, bc):
        """replicated levels: plain BC on both x faces"""
        nc, d3 = self.nc, self.D3(f)
        xc = f.xc
        if bc['x'][0] == 'n':
            nc.scalar.copy(d3[:, :, 0], d3[:, :, 1])
        else:
            nc.gpsimd.memset(d3[:, :, 0], 0.0)
        if bc['x'][1] == 'n':
            nc.scalar.copy(d3[:, :, xc - 1], d3[:, :, xc - 2])
        else:
            nc.gpsimd.memset(d3[:, :, xc - 1], 0.0)

    def edge_fix(self, f, bc):
        """overwrite ring-1 ghost cols on the 2 edge cores by BC, via
        per-core mask inputs (mL,nmL,mR,nmR)."""
        nc, d3 = self.nc, self.D3(f)
        P, zr, xc, gw = f.P, f.zr, f.xc, f.gw
        mL, nmL = self.masks_t[0:P, 0:1], self.masks_t[0:P, 1:2]
        mR, nmR = self.masks_t[0:P, 2:3], self.masks_t[0:P, 3:4]
        for (lo, side, m, nm) in ((True, gw - 1, mL, nmL), (False, xc - gw, mR, nmR)):
            gcol = d3[:, :, side]
            if bc['x'][0 if lo else 1] == 'd':
                nc.vector.tensor_scalar_mul(gcol, gcol, nm)
            else:
                icol = d3[:, :, gw if lo else xc - gw - 1]
                tmp = self.ebc_t[0:P, 0:zr]
                nc.vector.tensor_scalar_mul(tmp, icol, m)
                nc.vector.scalar_tensor_tensor(gcol, gcol, nm, tmp, OP.mult, OP.add)

    def exchange(self, fields_bcs, fam, fix=True):
        """Staged halo exchange.  fields_bcs: list of (Fld, bc, wd).
        Ghost-edge interior columns are packed into a contiguous staging
        tile (cheap strided engine copies), shipped through ONE contiguous
        DMA + AllGather, and the two needed neighbor slots are unpacked
        via contiguous DMAs + engine copies into the ghost columns."""
        nc = self.nc
        f0 = fields_bcs[0][0]
        P, zr = f0.P, f0.zr
        offs, W = [], 0
        for (f, bc, wd) in fields_bcs:
            offs.append(W)
            W += f.zr * wd
        pk = self.pk_t[0:P, 0:2 * W]
        # side 0 = left-edge interior (becomes left nbr's right ghost),
        # side 1 = right-edge interior (becomes right nbr's left ghost)
        for (f, bc, wd), off in zip(fields_bcs, offs):
            d3 = self.D3(f)
            gw, xc = f.gw, f.xc
            for s, c0 in ((0, gw), (1, xc - gw - wd)):
                dst = pk[:, s * W + off: s * W + off + f.zr * wd].rearrange(
                    "p (z w) -> p z w", w=wd)
                nc.scalar.copy(dst, d3[:, :, c0:c0 + wd])
        agin = self.dram.tile([2, P, W], F32, tag=f'agin_{fam}', name=f'agin{nc.next_id()}')
        agout = self.dram.tile([NC_ * 2, P, W], F32, tag=f'agout_{fam}',
                               name=f'agout{nc.next_id()}', addr_space="Shared")
        nc.sync.dma_start(agin[:, :, :].transpose([1, 0, 2]),
                          pk[:, :].rearrange("p (s w) -> p s w", s=2))
        nc.gpsimd.collective_compute(
            "AllGather", OP.bypass, replica_groups=[list(range(NC_))],
            ins=[agin.opt()], outs=[agout.opt()])
        uL = self.uL_t[0:P, 0:W]
        uR = self.uR_t[0:P, 0:W]
        nc.sync.dma_start(uL[:, :], agout[bass.ds(self.regL, 1), :, :])
        nc.sync.dma_start(uR[:, :], agout[bass.ds(self.regR, 1), :, :])
        for (f, bc, wd), off in zip(fields_bcs, offs):
            d3 = self.D3(f)
            gw, xc = f.gw, f.xc
            srcL = uL[:, off:off + f.zr * wd].rearrange("p (z w) -> p z w", w=wd)
            srcR = uR[:, off:off + f.zr * wd].rearrange("p (z w) -> p z w", w=wd)
            nc.scalar.copy(d3[:, :, gw - wd:gw], srcL)
            nc.scalar.copy(d3[:, :, xc - gw:xc - gw + wd], srcR)
            if fix:
                self.edge_fix(f, bc)

    # --- compute blocks ---------------------------------------------------
    def jacobi(self, dst, w_in, r, lvl):
        """dst = sum_taps(-wA/diag)*w_nb + (1/diag)*r ; w_in ghosts must be valid.
        z taps on gpsimd, x taps + combine on vector, y taps on PE."""
        nc, C = self.nc, self.C
        xc = w_in.xc
        ps = self.mm(f'JY{lvl}', w_in, w_in.P)
        pst = ps[:, xc: w_in.F - xc]
        gz = self.sub(self.gz0, lvl)
        nc.gpsimd.tensor_add(self.T(gz), self.T(w_in, xc), self.T(w_in, -xc))
        nc.vector.scalar_tensor_tensor(self.T(dst), self.T(w_in, 1), C['jxp'], pst, OP.mult, OP.add)
        nc.vector.scalar_tensor_tensor(self.T(dst), self.T(w_in, -1), C['jxm'], self.T(dst), OP.mult, OP.add)
        nc.vector.scalar_tensor_tensor(self.T(dst), self.T(gz), C['jzp'], self.T(dst), OP.mult, OP.add)
        nc.vector.scalar_tensor_tensor(self.T(dst), self.T(r), C['rb'], self.T(dst), OP.mult, OP.add)

    def residual(self, dst, pd, b):
        """dst = A pd - b at L0; pd ghosts valid."""
        nc, C = self.nc, self.C
        xc = pd.xc
        ps = self.mm('AY0', pd, 128)
        pst = ps[:, xc: pd.F - xc]
        gz = self.gz0
        nc.gpsimd.tensor_add(self.T(gz), self.T(pd, xc), self.T(pd, -xc))
        nc.vector.scalar_tensor_tensor(self.T(dst), self.T(pd, 1), C['wA_xp'], pst, OP.mult, OP.add)
        nc.vector.scalar_tensor_tensor(self.T(dst), self.T(pd, -1), C['wA_xm'], self.T(dst), OP.mult, OP.add)
        nc.vector.scalar_tensor_tensor(self.T(dst), self.T(b), -1.0, self.T(dst), OP.mult, OP.add)
        nc.vector.scalar_tensor_tensor(self.T(dst), self.T(gz), C['wA_zp'], self.T(dst), OP.mult, OP.add)

    def restrict(self, r_f, r_c, lf):
        """r_c (level lf+1) interior = w_res-weighted 2x2x2 sum of r_f (level lf)."""
        nc, C = self.nc, self.C
        g = GEOM[lf]
        P, zr, xc = g[0], g[2], g[3]
        F = zr * xc
        gc = GEOM[lf + 1]
        Pc = gc[0]
        ps = self.mm(f'R{lf}', r_f, Pc)
        t0 = self.gz0.t[0:Pc, 0:F]
        nc.scalar.copy(t0, ps[:, 0:F])
        t1 = self.sx0.t[0:Pc, 0:F]
        t2 = self.tx0.t[0:Pc, 0:F]
        nc.vector.tensor_add(t1[:, 0:F - 1], t0[:, 0:F - 1], t0[:, 1:F])
        nc.vector.tensor_add(t2[:, 0:F - xc - 1], t1[:, 0:F - xc - 1], t1[:, xc:F - 1])
        # strided gather: coarse cells <- fine pair sums.  For lf==0 also
        # produce the coarse x-ghost ring-1 (computable from the extended
        # fine residual) so L1 never needs its own halo exchange.
        zi = gc[2] - 2
        gwf, gwc = GEOM[lf][5], GEOM[lf + 1][5]
        d3c = self.D3(r_c)
        t23 = t2[:, 0:F].rearrange("p (z x) -> p z x", x=xc)
        if lf == 0:
            xi = gc[3] - 2 * gwc + 2        # interior + ghost ring-1 (34)
            c0, f0 = gwc - 1, gwf - 2       # coarse col 1 <- fine cols (2,3)
        else:
            xi = gc[3] - 2 * gwc
            c0, f0 = gwc, gwf
        nc.vector.tensor_scalar_mul(
            d3c[:, 1:1 + zi, c0:c0 + xi],
            t23[:, 1:1 + 2 * zi:2, f0:f0 + 2 * xi:2],
            2.0 * C['wres'])

    def prolong_mm(self, w_c, lf):
        """y-doubling matmul of level lf+1 tile -> psum [P_lf, F_{lf+1}]"""
        return self.mm(f'PR{lf}', w_c, GEOM[lf][0])

    def parity_views(self, ps, lvl_f, dst3):
        """yield (dst_quadrant, psum_quadrant) for the 4 z/x parities."""
        gf, gc = GEOM[lvl_f], GEOM[lvl_f + 1]
        zrf, xcf = gf[2], gf[3]
        zrc, xcc = gc[2], gc[3]
        ps3 = ps[:, 0:zrc * xcc].rearrange("p (z x) -> p z x", x=xcc)
        for pz in (0, 1):
            nzf = (zrf - pz + 1) // 2
            cz = 0 if pz == 0 else 1
            for px in (0, 1):
                nxf = (xcf - px + 1) // 2
                cx = 0 if px == 0 else 1
                yield (dst3[:, pz::2, px::2], ps3[:, cz:cz + nzf, cx:cx + nxf])

    def prolong_copy(self, w_c, w_f, lf):
        """w_f = prolong(w_c) including ghosts (coarse ghosts must be valid)."""
        nc = self.nc
        if lf == 1:
            # extract this core's 20-col x window (incl. both ghost rings) of
            # the replicated L2 field via a padded DRAM bounce (dynamic
            # SBUF-side DMA offsets hang on hardware), y-double via PR1, and
            # expand with per-zh coarse row offsets.  Fine cols {2k, 2k+1}
            # map to window col k.
            d3w2 = self.D3(w_c)
            win3 = self.w2win.t[:, 0:360].rearrange("p (z x) -> p z x", x=20)
            w2d = self.dram.tile([16, 18, 132], F32, tag='w2d', name=f'w2d{nc.next_id()}')
            nc.sync.dma_start(w2d[:, :, 0:130], d3w2[:, :, :])
            nc.sync.dma_start(win3[:, :, :], w2d[:, :, bass.ds(self.reg_w2, 20)])
            ps = self.mm('PR1', Fld(self.w2win.t[:, 0:360], 1), 64, psum_w=360)
            ps3 = ps[:, 0:360].rearrange("p (z x) -> p z x", x=20)
            d3 = self.D3(w_f)
            for zh in (0, 1):
                czh = 8 * zh
                psl = slice(zh * 32, (zh + 1) * 32)
                for pz in (0, 1):
                    nzf = (18 - pz + 1) // 2
                    cz = czh + (0 if pz == 0 else 1)
                    for fx0 in (0, 1):
                        nc.scalar.copy(
                            d3[psl, pz::2, fx0::2],
                            ps3[psl, cz:cz + nzf, 0:18])
            return
        ps = self.prolong_mm(w_c, lf)
        d3 = self.D3(w_f)
        for dq, pq in self.parity_views(ps, lf, d3):
            self.nc.scalar.copy(dq, pq)

    def prolong_sub(self, w_c, pd_old, pd_new, lf):
        """pd_new = pd_old - prolong(w_c), covering interior + ghost rings 1-2.
        Fine level 0 has gw=3: fine col c maps to coarse col (c-3)//2+1;
        cols {1,3,..,67} <- coarse 0..33, cols {2,4,..,68} <- coarse 0..33."""
        assert lf == 0
        ps = self.prolong_mm(w_c, lf)
        gf, gc = GEOM[lf], GEOM[lf + 1]
        zrf, xcf = gf[2], gf[3]
        zrc, xcc = gc[2], gc[3]
        ps3 = ps[:, 0:zrc * xcc].rearrange("p (z x) -> p z x", x=xcc)
        d3n, d3o = self.D3(pd_new), self.D3(pd_old)
        for pz in (0, 1):
            nzf = (zrf - pz + 1) // 2
            cz = 0 if pz == 0 else 1
            for fx0 in (0, 1):
                dq = d3n[:, pz::2, fx0::2]
                oq = d3o[:, pz::2, fx0::2]
                pq = ps3[:, cz:cz + nzf, 0:36]
                self.nc.vector.scalar_tensor_tensor(dq, pq, -1.0, oq, OP.mult, OP.add)

    def dbg_dump(self, name, f):
        if self.dbg_name == name and not self.dbg_written:
            self.nc.sync.dma_start(self.p_dbg[0:f.P, 0:f.F], self.V(f))
            self.dbg_written = True

    # --- main build -------------------------------------------------------
    def build(self):
        nc, C = self.nc, self.C
        with TileContext(nc) as tc:
            with tc.tile_pool(name="main", bufs=1) as pool, \
                 tc.tile_pool(name="psum", bufs=1, space="PSUM") as psum_pool, \
                 tc.tile_pool(name="dram", bufs=1, space="DRAM") as dram:
                self.pool, self.psum_pool, self.dram = pool, psum_pool, dram
                self._build_body(tc)
        nc.finalize()
        return nc

    def _load_fld(self, pname, name, lvl, tag=None, eng=None):
        f = self.fld(name, lvl, tag=tag)
        (eng or self.nc.sync).dma_start(self.V(f), self.p_in[pname][:, :])
        return f

    def _store_fld(self, f, ch, eng=None):
        (eng or self.nc.sync).dma_start(self.p_out[ch, :, :], self.V(f))

    def _build_body(self, tc):
        nc, C = self.nc, self.C
        pool = self.pool
        if self.stage == 'io0':
            t = self._load_fld('values_u', 'u', 0)
            for ch in range(4):
                self._store_fld(t, ch)
            return
        # constants / matrices / masks
        self.mats_t = pool.tile([128, self.MC], F32, tag="mats", name="mats_t")
        nc.sync.dma_start(self.mats_t[:, :], self.p_mats[:, :])
        self.masks_t = pool.tile([128, 4], F32, tag="masks", name="masks_t")
        nc.sync.dma_start(self.masks_t[:, :], self.p_masks[:, :])
        hoffs_t = pool.tile([1, 3], I32, tag="hoffs", name="hoffs_t")
        nc.sync.dma_start(hoffs_t[:, :], self.p_hoffs[:, :])
        # slot index registers for halo unpack
        self.regL = nc.sync.value_load(hoffs_t[0:1, 0:1], min_val=None, max_val=None)
        self.regR = nc.sync.value_load(hoffs_t[0:1, 1:2], min_val=None, max_val=None)
        self.reg_w2 = nc.sync.value_load(hoffs_t[0:1, 2:3], min_val=None, max_val=None)

        if self.stage == 'io':
            t = self._load_fld('values_u', 'u', 0)
            for ch in range(4):
                self._store_fld(t, ch)
            return

        # ---- loads (one contiguous DMA per field)
        u = self._load_fld('values_u', 'u', 0)
        v = self._load_fld('values_v', 'v', 0, eng=nc.scalar)
        w = self._load_fld('values_w', 'w', 0, eng=nc.gpsimd)
        a = self._load_fld('alpha', 'a', 0, eng=nc.scalar)
        pd0 = self._load_fld('values_pd', 'pd0', 0, eng=nc.gpsimd)

        # scratch needed by edge_fix (used inside exchange unpack)
        self.ebc_t = pool.tile([128, 34], F32, tag='ebc', name='ebc_t')
        # shared halo-exchange staging (sized for the largest exchange: ag1
        # has W = 34*(2+2+2+1+4) = 374)
        self.pk_t = pool.tile([128, 748], F32, tag='pk', name='pk_t')
        self.uL_t = pool.tile([128, 374], F32, tag='uLs', name='uL_t')
        self.uR_t = pool.tile([128, 374], F32, tag='uRs', name='uR_t')
        # scratch sized for the largest level (L0 is 34*72=2448)
        self.gz0 = Fld(pool.tile([128, 2448], F32, tag='gz0', name='gz0'), 0)
        self.sx0 = Fld(pool.tile([128, 2448], F32, tag='sx0', name='sx0'), 0)
        self.tx0 = Fld(pool.tile([128, 2448], F32, tag='tx0', name='tx0'), 0)
        if self.dbg_init:
            for t_ in (self.gz0.t, self.sx0.t, self.tx0.t, self.ebc_t):
                self.nc.vector.memset(t_[:, :], 0.0)

        # ---- merged ghost exchange: u,v,w wd2 (stars valid to ring 1,
        # killing the post-predictor exchange), alpha wd1, pd wd4.
        for f, bc in ((u, BC_U), (v, BC_V), (w, BC_W), (pd0, BC_PD)):
            self.prep_z(f, bc)
        self.exchange([(u, BC_U, 2), (v, BC_V, 2), (w, BC_W, 2),
                       (a, BC_A, 1), (pd0, BC_PD, 4)], 'ag1')

        # ---- rho chain (issued after the exchange so the pack copies get
        # the scalar engine first); all are exchange-independent except the
        # use of alpha ghost ring 1, which the STT chain below needs anyway.
        buoy = self.fld('buoy', 0)
        rP1 = self.fld('rP1', 0)
        rP2 = self.fld('rP2', 0)
        nc.vector.tensor_scalar(self.T(a), self.T(a), 0.05, 1.0, OP.max, OP.min)
        nc.vector.tensor_scalar(self.T(a), self.T(a), RHO_L - RHO_G, RHO_G, OP.mult, OP.add)
        rinv = a
        nc.vector.reciprocal(self.T(rinv), self.T(a))
        nc.vector.tensor_scalar(self.T(buoy), self.T(rinv), -DT * G_Z * RHO_L, DT * G_Z, OP.mult, OP.add)
        nc.scalar.mul(self.T(rP1), self.T(rinv), DT * C['axp'])
        nc.scalar.mul(self.T(rP2), self.T(rinv), DT)

        # scaled advection multipliers (tiles also reused later)
        ut1 = self.fld('ut1', 0)     # x-family: -DT*axp*u
        wt1 = self.fld('wt1', 0)     # z-family: -DT*azp*w
        vt2 = self.fld('vt2', 0)     # y-family: -DT*v
        nc.scalar.mul(self.T(ut1), self.T(u), -DT * C['axp'])
        nc.scalar.mul(self.T(wt1), self.T(w), -DT * C['azp'])
        nc.scalar.mul(self.T(vt2), self.T(v), -DT)

        xc = u.xc
        us, vs, ws = self.fld('us', 0), self.fld('vs', 0), self.fld('ws', 0)
        sx, tx, gz = self.sx0, self.tx0, self.gz0
        for f, bc, dst, extra in ((u, BC_U, us, None), (v, BC_V, vs, None), (w, BC_W, ws, buoy)):
            nm = 'u' if f is u else ('v' if f is v else 'w')
            # PE matmuls park eagerly into SBUF via ACT so the single PSUM
            # slot frees up and all six predictor matmuls pipeline during
            # the halo exchange (they need no x-ghosts).
            ps = self.mm('MD_' + nm, f, 128)
            nc.scalar.copy(self.T(dst), ps[:, xc: f.F - xc])
            ps2 = self.mm('DY_' + nm, f, 128)
            nc.scalar.copy(self.T(tx), ps2[:, xc: f.F - xc])
            # gpsimd: z diffusion tap sum + z advection (TT ops only on Pool)
            nc.gpsimd.tensor_add(self.T(gz), self.T(f, xc), self.T(f, -xc))
            nc.gpsimd.tensor_sub(self.T(sx), self.T(f, xc), self.T(f, -xc))
            nc.gpsimd.tensor_mul(self.T(sx), self.T(sx), self.T(wt1))
            # vector: y advection first (frees tx), then x taps/advection
            nc.vector.tensor_mul(self.T(tx), self.T(tx), self.T(vt2))
            nc.vector.tensor_add(self.T(dst), self.T(dst), self.T(tx))
            nc.vector.tensor_sub(self.T(tx), self.T(f, 1), self.T(f, -1))
            nc.vector.tensor_mul(self.T(tx), self.T(tx), self.T(ut1))
            nc.vector.tensor_add(self.T(dst), self.T(dst), self.T(tx))
            nc.vector.scalar_tensor_tensor(self.T(dst), self.T(f, 1), DT * C['wd_xp'], self.T(dst), OP.mult, OP.add)
            nc.vector.scalar_tensor_tensor(self.T(dst), self.T(f, -1), DT * C['wd_xm'], self.T(dst), OP.mult, OP.add)
            nc.vector.scalar_tensor_tensor(self.T(dst), self.T(gz), DT * C['wd_zp'], self.T(dst), OP.mult, OP.add)
            nc.vector.tensor_add(self.T(dst), self.T(dst), self.T(sx))
            if extra is not None:
                nc.vector.tensor_add(self.T(dst), self.T(dst), self.T(extra))
        self.dbg_dump('us', us)
        self.dbg_dump('vs', vs)
        self.dbg_dump('ws', ws)
        if self.stage == 'pred':
            self._store_fld(us, 0)
            self._store_fld(vs, 1)
            self._store_fld(ws, 2)
            self._store_fld(ws, 3)
            return

        # ---- divergence -> b  (stars are valid through ring 1 on interior
        # cores; edge cores get the BC ring-1 values via edge_fix)
        self.prep_z(ws, BC_W)
        for f, bc in ((us, BC_U), (vs, BC_V), (ws, BC_W)):
            self.edge_fix(f, bc)
        b = Fld(buoy.t, 0)  # buoy dead after ws
        cbx = -(DX * DX / DT) * C['axp']
        cbz = -(DX * DX / DT) * C['azp']
        ps = self.mm('DY_v', vs, 128)
        pst = ps[:, xc: vs.F - xc]
        nc.vector.tensor_scalar_mul(self.T(b), pst, -(DX * DX / DT))
        nc.vector.scalar_tensor_tensor(self.T(b), self.T(us, 1), cbx, self.T(b), OP.mult, OP.add)
        nc.vector.scalar_tensor_tensor(self.T(b), self.T(us, -1), -cbx, self.T(b), OP.mult, OP.add)
        nc.gpsimd.tensor_sub(self.T(sx), self.T(ws, xc), self.T(ws, -xc))
        nc.vector.scalar_tensor_tensor(self.T(b), self.T(sx), cbz, self.T(b), OP.mult, OP.add)
        self.exchange([(b, BC_PD, 3)], 'b', fix=False)
        self.dbg_dump('b', b)
        if self.stage == 'div':
            self._store_fld(us, 0)
            self._store_fld(vs, 1)
            self._store_fld(ws, 2)
            self._store_fld(b, 3)
            return

        # ---- multigrid: 2 V-cycles
        # coarse-level tiles live inside v/w (dead after the predictor)
        r1 = Fld(v.t[0:64, 0:648], 1)
        wp1_ = Fld(v.t[0:64, 648:1296], 1)
        wsm1_ = Fld(v.t[0:64, 1296:1944], 1)
        r2l = v.t[0:32, 1944:2072]
        self.w2win = Fld(v.t[0:16, 2072:2432], 1)
        r2 = Fld(w.t[0:16, 0:2340], 2)
        pdA = Fld(u.t, 0)    # u dead after predictor
        pdB = Fld(wt1.t, 0)  # wt1 dead after predictor
        pdC = Fld(rinv.t, 0)  # rinv(=a) dead after rP1/rP2/buoy
        r0 = Fld(vt2.t, 0)   # vt2 dead after predictor
        r3 = self.fld('r3', 3)
        r4 = self.fld('r4', 4)
        r5 = self.fld('r5', 5)
        r6 = self.fld('r6', 6)
        wts = {l: (self.fld(f'wp{l}', l), self.fld(f'wsm{l}', l)) for l in range(2, 6)}
        wts[1] = (wp1_, wsm1_)
        w6 = self.fld('w6', 6)

        pd_cur = pd0
        rot = [pdB, pdC, pdA]
        ri = 0
        for vc in range(2):
            # pre-smooth (vc0: pd0 exchanged in the merged AG up front;
            # vc1: exchange here)
            if vc > 0:
                self.prep_z(pd_cur, BC_PD)
                self.exchange([(pd_cur, BC_PD, 4)], 'pd2')
            if self.stage == 'exch1' and vc == 0:
                for ch in range(4):
                    self._store_fld(pd_cur, ch)
                return
            pd1 = rot[ri % 3]; ri += 1
            self.jacobi(pd1, pd_cur, b, 0)
            self.edge_fix(pd1, BC_PD)
            if self.stage == 'jac1' and vc == 0:
                for ch in range(4):
                    self._store_fld(pd1, ch)
                return
            # residual: pd1 ghost ring-1 is valid from the extended pre-smooth
            self.prep_z(pd1, BC_PD)
            self.residual(r0, pd1, b)
            if self.stage == 'resid' and vc == 0:
                for ch in range(4):
                    self._store_fld(pd1, ch)
                return
            # restrict chain
            self.restrict(r0, r1, 0)
            # r1 -> r2 local window, then allgather full r2
            self._restrict_r1_to_r2(r1, r2l, r2)
            self.restrict(r2, r3, 2)
            self.restrict(r3, r4, 3)
            self.restrict(r4, r5, 4)
            self.restrict(r5, r6, 5)
            # coarsest: w6 = r6 / diag
            nc.scalar.mul(self.V(w6), self.V(r6), C['rb'])
            self.prep_z(w6, BC_PD)
            self.prep_x_bc(w6, BC_PD)
            # up-leg 5..1
            wc = w6
            for l in range(5, 0, -1):
                wp, wsm = wts[l]
                self.prolong_copy(wc, wp, l)
                self.jacobi(wsm, wp, (r1 if l == 1 else (r2, r3, r4, r5)[l - 2]), l)
                self.prep_z(wsm, BC_PD)
                if l >= 2:
                    self.prep_x_bc(wsm, BC_PD)
                wc = wsm
            # correction + post-smooth
            pd2 = rot[ri % 3]; ri += 1
            self.prolong_sub(wc, pd1, pd2, 0)
            if self.stage == 'corr' and vc == 0:
                for ch in range(4):
                    self._store_fld(pd2, ch)
                return
            pd3 = rot[ri % 3]; ri += 1
            self.jacobi(pd3, pd2, b, 0)
            self.edge_fix(pd3, BC_PD)
            pd_cur = pd3
            self.dbg_dump(f'pd_vc{vc}', pd3)
            if self.stage == 'vc1' and vc == 0:
                self._store_fld(us, 0)
                self._store_fld(vs, 1)
                self._store_fld(ws, 2)
                self._store_fld(pd_cur, 3)
                return

        # ---- projection: pd ghost ring-1 already valid (extended post-smooth)
        self.prep_z(pd_cur, BC_PD)
        ps = self.mm('DY_pd', pd_cur, 128)
        pst = ps[:, xc: pd_cur.F - xc]
        nc.vector.tensor_mul(self.T(tx), pst, self.T(rP2))
        nc.vector.tensor_sub(self.T(vs), self.T(vs), self.T(tx))
        nc.vector.tensor_sub(self.T(sx), self.T(pd_cur, 1), self.T(pd_cur, -1))
        nc.vector.tensor_mul(self.T(sx), self.T(sx), self.T(rP1))
        nc.vector.tensor_sub(self.T(us), self.T(us), self.T(sx))
        nc.gpsimd.tensor_sub(self.T(gz), self.T(pd_cur, xc), self.T(pd_cur, -xc))
        nc.gpsimd.tensor_mul(self.T(gz), self.T(gz), self.T(rP1))
        nc.vector.tensor_sub(self.T(ws), self.T(ws), self.T(gz))

        # ---- store (contiguous, spread across engine DMA queues)
        self._store_fld(us, 0)
        self._store_fld(vs, 1, eng=nc.scalar)
        self._store_fld(ws, 2, eng=nc.gpsimd)
        self._store_fld(pd_cur, 3)

    def _restrict_r1_to_r2(self, r1, r2l, r2):
        """restrict sharded r1 -> local coarse window, allgather -> full r2."""
        nc, C = self.nc, self.C
        g = GEOM[1]
        P, zr, xc = g[0], g[2], g[3]
        F = zr * xc
        ps = self.mm('R1', r1, P // 2)
        t0 = self.gz0.t[0:P // 2, 0:F]
        nc.scalar.copy(t0, ps[:, 0:F])
        t1 = self.sx0.t[0:P // 2, 0:F]
        t2 = self.tx0.t[0:P // 2, 0:F]
        nc.vector.tensor_add(t1[:, 0:F - 1], t0[:, 0:F - 1], t0[:, 1:F])
        nc.vector.tensor_add(t2[:, 0:F - xc - 1], t1[:, 0:F - xc - 1], t1[:, xc:F - 1])
        t23 = t2[:, 0:F].rearrange("p (z x) -> p z x", x=xc)
        # compact local window [32p, 8z * 16x]
        r2l3 = r2l[:, 0:128].rearrange("p (z x) -> p z x", x=16)
        nc.vector.tensor_scalar_mul(r2l3[:, :, :], t23[:, 1:17:2, 2:34:2], 2.0 * C['wres'])
        agin = self.dram.tile([1, 32 * 128], F32, tag='agr2i', name=f'agr2i{nc.next_id()}')
        agout = self.dram.tile([NC_, 2, 16, 8, 16], F32, tag='agr2o', name=f'agr2o{nc.next_id()}',
                               addr_space="Shared")
        nc.sync.dma_start(agin[0:1, :], r2l[:, 0:128])
        nc.gpsimd.collective_compute(
            "AllGather", OP.bypass, replica_groups=[list(range(NC_))],
            ins=[agin.opt()], outs=[agout.opt()])
        d3 = self.D3(r2)
        engs = (nc.sync, nc.scalar, nc.gpsimd)
        for s in range(NC_):
            for zh in (0, 1):
                engs[(2 * s + zh) % 3].dma_start(
                    d3[:, 1 + zh * 8:9 + zh * 8, 1 + s * 16:17 + s * 16],
                    agout[s, zh, :, :, :])


# ---------------------------------------------------------------- entry
_CACHE = {}


def _get_nc(key, C, dbg_name=None, stage='full', dbg_init=False):
    ck = (key, dbg_name, stage, dbg_init)
    if ck not in _CACHE:
        mats_np, cols = build_mats(C)
        b = B(C, mats_np, cols, dbg_name=dbg_name, stage=stage, dbg_init=dbg_init)
        nc = b.build()
        _CACHE[ck] = (nc, mats_np)
    return _CACHE[ck]


def _pad_field(arr):
    """[64z, 64y, 64x] -> padded device tile [128, 34*72] (f32)."""
    t = np.zeros((128, 34, 72), np.float32)
    # p = zh*64 + y ; row z' = 1..32 ; col 4..67
    a = arr.reshape(2, 32, 64, 64).transpose(0, 2, 1, 3).reshape(128, 32, 64)
    t[:, 1:33, 4:68] = a
    return t.reshape(128, 34 * 72)


def _make_in_maps(fields, mats_np):
    in_maps = []
    for r in range(NC_):
        m = {}
        for nm, arr in fields.items():
            loc = np.asarray(arr, np.float32)[0, 0, :, :, r * XL:(r + 1) * XL]
            m[nm] = _pad_field(loc)
        m['mats'] = mats_np
        msk = np.zeros((128, 4), np.float32)
        msk[:, 0] = 1.0 if r == 0 else 0.0       # mL
        msk[:, 1] = 0.0 if r == 0 else 1.0       # nmL
        msk[:, 2] = 1.0 if r == NC_ - 1 else 0.0  # mR
        msk[:, 3] = 0.0 if r == NC_ - 1 else 1.0  # nmR
        m['masks'] = msk
        ho = np.zeros((1, 3), np.int32)
        rl = max(r - 1, 0)
        rr = min(r + 1, NC_ - 1)
        ho[0, 0] = rl * 2 + 1   # left ghost <- left nbr's right-edge slot
        ho[0, 1] = rr * 2 + 0   # right ghost <- right nbr's left-edge slot
        ho[0, 2] = r * 16
        m['hoffs'] = ho
        in_maps.append(m)
    return in_maps


def kernel(alpha, values_u, values_v, values_w, values_pd,
           w_diff, w_xadv, w_yadv, w_zadv, w_A, w_res, _dbg=None, _stage='full', _dbg_init=False):
    C = extract_consts(w_diff, w_xadv, w_yadv, w_zadv, w_A, w_res)
    key = tuple(sorted(C.items()))
    nc, mats_np = _get_nc(key, C, dbg_name=_dbg, stage=_stage, dbg_init=_dbg_init)
    fields = {'alpha': alpha, 'values_u': values_u, 'values_v': values_v,
              'values_w': values_w, 'values_pd': values_pd}
    in_maps = _make_in_maps(fields, mats_np)
    res = run_bass_kernel_spmd(nc, in_maps, core_ids=list(range(NC_)))
    full = np.empty((4, NZ, NY, NX), np.float32)
    for r in range(NC_):
        o = res.results[r]['out'].reshape(4, 128, 34, 72)[:, :, 1:33, 4:68]
        # [4, (zh y), z', x] -> [4, (zh z'), y, x]
        o = o.reshape(4, 2, 64, 32, 64).transpose(0, 1, 3, 2, 4).reshape(4, 64, 64, 64)
        full[:, :, :, r * XL:(r + 1) * XL] = o
    if _dbg is not None:
        kernel._dbg_res = [res.results[r].get('dbg') for r in range(NC_)]
    return full[None]  # (1, 4, 64, 64, 512)


# revision 8
# speedup vs baseline: 2.3950x; 1.3150x over previous
"""Trainium2 Bass kernel for the multiphase CFD fractional-step solver
(predictor + divergence + 2 multigrid V-cycles + projection) on a
64x64x512 grid, sharded along x across 8 NeuronCores.

Self-contained: hardcodes shapes/sharding; reads stencil coefficient
VALUES from the runtime weight inputs and compiles a specialized graph
(cached per coefficient set).

Device layout (level l):
  partitions p = zh*ny + y   (zh in {0,1} z-halves; l=6 has P=1)
  free       j = z'*xc + x   (z' in [0, zr): rows 0 and zr-1 are z-ghosts;
                              x in [0, xc): cols 0 and xc-1 are x-ghosts)
Volume passes run on the row-trimmed flat range [xc, F-xc) so all
+-1 / +-xc shifted reads stay inside the [P, F] tile.
y-axis stencil taps (partition axis) are done on the TensorEngine as
[K,M] matmuls with per-field boundary rows baked into the matrices.

Precision: fields and stencil passes run in fp16 (the DVE gets 2x/4x
element rates for packed 2-byte operands and the PE runs fp16 matmuls
at 4x the fp32 rate).  fp32 is kept where it matters: the rho/1-rho
chain (values ~1e-3 would denormal in fp16 scaling), the residual
accumulation (cancellation), and the projection corrections.  The
Jacobi/residual y-matrices are pre-scaled so the per-tap coefficient
application collapses into a single tensor_scalar, exploiting the
operator's full tap symmetry (asserted in extract_consts).

I/O: the host pre-pads each field into the device tile layout
[128, zr*xc] fp16 so every load/store is one fully-contiguous DMA.
x halos travel through AllGathers with contiguous staged pack/unpack.
One merged exchange up front carries u,v,w (wd2) + alpha (wd1) +
pd (wd4): the predictor produces stars valid through ghost ring 1 and
the post-predictor exchange disappears.  Remaining collectives:
b (wd3), replicated-r2 gather per V-cycle, pd re-exchange before
V-cycle 2.  Multigrid levels 0-1 stay sharded; levels 2-6 replicated.
"""
import sys
sys.path.insert(0, '/opt/trn_rl_repo')
import numpy as np
import concourse.bass as bass
import concourse.bacc as bacc
import concourse.mybir as mybir
from concourse.bass_utils import run_bass_kernel_spmd
from concourse.tile import TileContext

F32 = mybir.dt.float32
F16 = mybir.dt.float16
I32 = mybir.dt.int32
OP = mybir.AluOpType

DT, DX, G_Z = 0.002, 0.04, -10.0
RHO_L, RHO_G, NU = 1000.0, 1.0, 1e-3
NZ, NY, NX = 64, 64, 512
NC_ = 8
XL = NX // NC_  # 64 local x

# level: (P, ny, zr, xc, sharded, gw) -- gw = x-ghost cols per side
GEOM = {
    0: (128, 64, 34, 72, True, 4),
    1: (64, 32, 18, 36, True, 2),
    2: (16, 16, 18, 130, False, 1),
    3: (8, 8, 10, 66, False, 1),
    4: (4, 4, 6, 34, False, 1),
    5: (2, 2, 4, 18, False, 1),
    6: (1, 1, 3, 10, False, 1),
}
# BC per field: axis -> (lo, hi), 'n' neumann (ghost=adjacent), 'd' dirichlet (ghost=0)
BC_U = {'z': ('n', 'n'), 'y': ('n', 'n'), 'x': ('d', 'd')}
BC_V = {'z': ('n', 'n'), 'y': ('d', 'd'), 'x': ('n', 'n')}
BC_W = {'z': ('d', 'd'), 'y': ('n', 'n'), 'x': ('n', 'n')}
BC_PD = {'z': ('n', 'd'), 'y': ('n', 'n'), 'x': ('n', 'n')}
BC_A = {'z': ('n', 'n'), 'y': ('n', 'n'), 'x': ('n', 'n')}


# ---------------------------------------------------------------- host-side
def _yblock(ny, cm, cc, cp, bc):
    """[ny, ny] matrix M with out[y] = sum_k M[k, y] in[k]:
    tridiag with sub=cm (coeff of in[y-1]), diag=cc, super=cp (in[y+1]),
    Neumann BC folds the ghost coeff into the boundary diagonal."""
    m = np.zeros((ny, ny), np.float32)
    for y in range(ny):
        m[y, y] += cc
        if y > 0:
            m[y - 1, y] += cm
        elif bc[0] == 'n':
            m[y, y] += cm
        if y < ny - 1:
            m[y + 1, y] += cp
        elif bc[1] == 'n':
            m[y, y] += cp
    return m


def _blkdiag2(b):
    n = b.shape[0]
    m = np.zeros((2 * n, 2 * b.shape[1]), np.float32)
    m[:n, :b.shape[1]] = b
    m[n:, b.shape[1]:] = b
    return m


def _halve(ny):
    m = np.zeros((ny, ny // 2), np.float32)
    for y in range(ny):
        m[y, y // 2] = 0.5
    return m


def _double(nyc, nyf):
    m = np.zeros((nyc, nyf), np.float32)
    for y in range(nyf):
        m[y // 2, y] = 1.0
    return m


def build_mats(C):
    """Concatenated [128, sum M] lhsT matrices (fp16) + column offset map."""
    cols = {}
    parts = []
    total = 0

    def add(name, m, K):
        nonlocal total
        assert m.shape[0] == K and K <= 128 and m.shape[1] <= 128
        buf = np.zeros((128, m.shape[1]), np.float32)
        buf[:K] = m
        cols[name] = (total, m.shape[1], K)
        parts.append(buf)
        total += m.shape[1]

    # predictor diffusion y-taps + center (K=M=128, blockdiag over zh)
    for nm, bc in (('u', BC_U), ('v', BC_V), ('w', BC_W)):
        b = _yblock(64, DT * C['wd_ym'], 1.0 + DT * C['wd_c'], DT * C['wd_yp'], bc['y'])
        add('MD_' + nm, _blkdiag2(b), 128)
    # advection / gradient y-difference (raw tap values)
    for nm, bc in (('u', BC_U), ('v', BC_V), ('w', BC_W), ('pd', BC_PD)):
        b = _yblock(64, C['aym'], 0.0, C['ayp'], bc['y'])
        add('DY_' + nm, _blkdiag2(b), 128)
    # residual y-taps + center at L0, pre-divided by wA_xp so the residual
    # is accumulated in the r' = r/wA_xp basis
    b = _yblock(64, 1.0, C['wA_c'] / C['wA_xp'], 1.0, BC_PD['y'])
    add('AY0', _blkdiag2(b), 128)
    # jacobi y matrices per level 0..5, unit taps: the common factor cs
    # (= -wA_xp/diag) is applied once in a final tensor_scalar
    for l in range(6):
        P, ny = GEOM[l][0], GEOM[l][1]
        b = _yblock(ny, 1.0, 0.0, 1.0, BC_PD['y'])
        add(f'JY{l}', _blkdiag2(b) if l <= 1 else b, P)
    # restrict y-halving matrices (R{l}: level l -> l+1)
    add('R0', _blkdiag2(_halve(64)), 128)
    add('R1', _blkdiag2(_halve(32)), 64)   # stays (zh,y); zh dissolved in AG unpack
    for l in range(2, 6):
        add(f'R{l}', _halve(GEOM[l][1]), GEOM[l][0])
    # prolong y-doubling: PR{l} maps level l+1 -> l
    add('PR0', _blkdiag2(_double(32, 64)), 64)
    m = _double(16, 32)
    add('PR1', np.concatenate([m, m], axis=1), 16)  # s=1 coarse -> (zh,y) fine
    for l in range(2, 6):
        add(f'PR{l}', _double(GEOM[l + 1][1], GEOM[l][1]), GEOM[l + 1][0])

    return np.concatenate(parts, axis=1).astype(np.float16), cols


def extract_consts(w_diff, w_xadv, w_yadv, w_zadv, w_A, w_res):
    g = lambda a, i, j, k: float(np.asarray(a)[0, 0, i, j, k])
    C = {}
    C['wd_c'] = g(w_diff, 1, 1, 1)
    C['wd_zm'], C['wd_zp'] = g(w_diff, 0, 1, 1), g(w_diff, 2, 1, 1)
    C['wd_ym'], C['wd_yp'] = g(w_diff, 1, 0, 1), g(w_diff, 1, 2, 1)
    C['wd_xm'], C['wd_xp'] = g(w_diff, 1, 1, 0), g(w_diff, 1, 1, 2)
    C['wA_c'] = g(w_A, 1, 1, 1)
    C['wA_zm'], C['wA_zp'] = g(w_A, 0, 1, 1), g(w_A, 2, 1, 1)
    C['wA_ym'], C['wA_yp'] = g(w_A, 1, 0, 1), g(w_A, 1, 2, 1)
    C['wA_xm'], C['wA_xp'] = g(w_A, 1, 1, 0), g(w_A, 1, 1, 2)
    C['axp'], C['axm'] = g(w_xadv, 1, 1, 2), g(w_xadv, 1, 1, 0)
    C['ayp'], C['aym'] = g(w_yadv, 1, 2, 1), g(w_yadv, 1, 0, 1)
    C['azp'], C['azm'] = g(w_zadv, 2, 1, 1), g(w_zadv, 0, 1, 1)
    wr = np.asarray(w_res).ravel()
    assert np.allclose(wr, wr[0]), "nonuniform w_res unsupported"
    C['wres'] = float(wr[0])
    # fast paths used by the kernel
    assert abs(C['axm'] + C['axp']) < 1e-12 * max(1, abs(C['axp']))
    assert abs(C['azm'] + C['azp']) < 1e-12 * max(1, abs(C['azp']))
    # x/z/y diffusion tap symmetry
    assert abs(C['wd_zm'] - C['wd_zp']) < 1e-12 * max(1, abs(C['wd_zp']))
    assert abs(C['wd_xm'] - C['wd_xp']) < 1e-12 * max(1, abs(C['wd_xp']))
    # A-operator full tap symmetry (lets the jacobi/residual scale fold
    # into a single constant cs)
    for k in ('wA_zm', 'wA_zp', 'wA_ym', 'wA_yp', 'wA_xm'):
        assert abs(C[k] - C['wA_xp']) < 1e-12 * max(1, abs(C['wA_xp'])), k
    diag = C['wA_c']
    C['diag'] = diag
    C['jxp'] = -C['wA_xp'] / diag
    C['cs'] = C['jxp']
    C['rb'] = 1.0 / diag
    return C


# ---------------------------------------------------------------- builder
class Fld:
    def __init__(self, t, lvl):
        self.t, self.lvl = t, lvl
        P, ny, zr, xc, _, gw = GEOM[lvl]
        self.P, self.zr, self.xc, self.F, self.gw = P, zr, xc, zr * xc, gw


class B:
    """Builder context."""

    def __init__(self, C, mats_np, mat_cols, dbg_name=None, stage='full', dbg_init=False):
        self.C = C
        self.stage = stage
        self.dbg_init = dbg_init
        self.dbg_name = dbg_name
        self.nc = bacc.Bacc()
        nc = self.nc
        self.mat_cols = mat_cols
        self.MC = mats_np.shape[1]
        # params (fields are pre-padded on host into the device tile layout)
        self.p_in = {}
        for nm in ('alpha', 'values_u', 'values_v', 'values_w', 'values_pd'):
            self.p_in[nm] = nc.declare_dram_parameter(nm, [128, GEOM[0][2] * GEOM[0][3]], F16, isOutput=False)
        self.p_mats = nc.declare_dram_parameter('mats', [128, self.MC], F16, isOutput=False)
        self.p_masks = nc.declare_dram_parameter('masks', [128, 4], F32, isOutput=False)
        self.p_hoffs = nc.declare_dram_parameter('hoffs', [1, 3], I32, isOutput=False)
        self.p_out = nc.declare_dram_parameter('out', [4, 128, GEOM[0][2] * GEOM[0][3]], F16, isOutput=True)
        if dbg_name:
            self.p_dbg = nc.declare_dram_parameter('dbg', [128, GEOM[0][2] * GEOM[0][3]], F16, isOutput=True)
        self.dbg_written = False

    # --- tile helpers -----------------------------------------------------
    def fld(self, name, lvl, tag=None, dt=F16):
        g = GEOM[lvl]
        t = self.pool.tile([g[0], g[2] * g[3]], dt, tag=(tag or name), name=name)
        if self.dbg_init:
            self.nc.vector.memset(t[:, :], 0.0)
        return Fld(t, lvl)

    def sub(self, f, lvl):
        g = GEOM[lvl]
        return Fld(f.t[0:g[0], 0:g[2] * g[3]], lvl)

    def T(self, f, s=0):
        """row-trimmed shifted flat view [P, F-2*xc]"""
        return f.t[:, f.xc + s: f.F - f.xc + s]

    def V(self, f):
        return f.t[:, 0:f.F]

    def D3(self, f):
        return f.t[:, 0:f.F].rearrange("p (z x) -> p z x", x=f.xc)

    def mat(self, name):
        off, M, K = self.mat_cols[name]
        return self.mats_t[0:K, off:off + M]

    def mm(self, name, rhs_f, Pout, psum_w=None):
        """psum[Pout, F] = mats[name].T @ V(rhs)  (chunked, full width)"""
        nc = self.nc
        F = psum_w or rhs_f.F
        ps = self.psum_pool.tile([Pout, F], F32, tag="psA", name=f"ps_{name}_{nc.next_id()}")
        rhs = rhs_f.t[:, 0:F]
        lhsT = self.mat(name)
        for c0 in range(0, F, 512):
            w = min(512, F - c0)
            nc.tensor.matmul(ps[:, c0:c0 + w], lhsT, rhs[:, c0:c0 + w], start=True, stop=True)
        return ps

    # --- ghost prep -------------------------------------------------------
    def prep_z(self, f, bc):
        """fill z ghost rows: global BC rows (+ inter-half swap on levels 0-1)"""
        nc, d3 = self.nc, self.D3(f)
        P, zr = f.P, f.zr
        split = f.lvl <= 1
        lo = slice(0, P // 2) if split else slice(0, P)
        hi = slice(P // 2, P) if split else slice(0, P)
        if bc['z'][0] == 'n':
            nc.scalar.copy(d3[lo, 0, :], d3[lo, 1, :])
        else:
            nc.gpsimd.memset(d3[lo, 0, :], 0.0)
        if bc['z'][1] == 'n':
            nc.scalar.copy(d3[hi, zr - 1, :], d3[hi, zr - 2, :])
        else:
            nc.gpsimd.memset(d3[hi, zr - 1, :], 0.0)
        if split:
            nc.sync.dma_start(d3[lo, zr - 1, :], d3[hi, 1, :])
            nc.sync.dma_start(d3[hi, 0, :], d3[lo, zr - 2, :])

    def prep_x_bc(self, f, bc):
        """replicated levels: plain BC on both x faces"""
        nc, d3 = self.nc, self.D3(f)
        xc = f.xc
        if bc['x'][0] == 'n':
            nc.scalar.copy(d3[:, :, 0], d3[:, :, 1])
        else:
            nc.gpsimd.memset(d3[:, :, 0], 0.0)
        if bc['x'][1] == 'n':
            nc.scalar.copy(d3[:, :, xc - 1], d3[:, :, xc - 2])
        else:
            nc.gpsimd.memset(d3[:, :, xc - 1], 0.0)

    def edge_fix(self, f, bc):
        """overwrite ring-1 ghost cols on the 2 edge cores by BC, via
        per-core mask inputs (mL,nmL,mR,nmR)."""
        nc, d3 = self.nc, self.D3(f)
        P, zr, xc, gw = f.P, f.zr, f.xc, f.gw
        mL, nmL = self.masks_t[0:P, 0:1], self.masks_t[0:P, 1:2]
        mR, nmR = self.masks_t[0:P, 2:3], self.masks_t[0:P, 3:4]
        for (lo, side, m, nm) in ((True, gw - 1, mL, nmL), (False, xc - gw, mR, nmR)):
            gcol = d3[:, :, side]
            if bc['x'][0 if lo else 1] == 'd':
                nc.vector.tensor_scalar_mul(gcol, gcol, nm)
            else:
                icol = d3[:, :, gw if lo else xc - gw - 1]
                tmp = self.ebc_t[0:P, 0:zr]
                nc.vector.tensor_scalar_mul(tmp, icol, m)
                nc.vector.scalar_tensor_tensor(gcol, gcol, nm, tmp, OP.mult, OP.add)

    def exchange(self, fields_bcs, fam, fix=True):
        """Staged halo exchange.  fields_bcs: list of (Fld, bc, wd).
        Ghost-edge interior columns are packed into a contiguous staging
        tile (cheap strided engine copies), shipped through ONE contiguous
        DMA + AllGather, and the two needed neighbor slots are unpacked
        via contiguous DMAs + engine copies into the ghost columns."""
        nc = self.nc
        f0 = fields_bcs[0][0]
        P, zr = f0.P, f0.zr
        offs, W = [], 0
        for (f, bc, wd) in fields_bcs:
            offs.append(W)
            W += f.zr * wd
        pk = self.pk_t[0:P, 0:2 * W]
        # side 0 = left-edge interior (becomes left nbr's right ghost),
        # side 1 = right-edge interior (becomes right nbr's left ghost)
        for (f, bc, wd), off in zip(fields_bcs, offs):
            d3 = self.D3(f)
            gw, xc = f.gw, f.xc
            for s, c0 in ((0, gw), (1, xc - gw - wd)):
                dst = pk[:, s * W + off: s * W + off + f.zr * wd].rearrange(
                    "p (z w) -> p z w", w=wd)
                nc.scalar.copy(dst, d3[:, :, c0:c0 + wd])
        agin = self.dram.tile([2, P, W], F16, tag=f'agin_{fam}', name=f'agin{nc.next_id()}')
        agout = self.dram.tile([NC_ * 2, P, W], F16, tag=f'agout_{fam}',
                               name=f'agout{nc.next_id()}', addr_space="Shared")
        nc.sync.dma_start(agin[:, :, :].transpose([1, 0, 2]),
                          pk[:, :].rearrange("p (s w) -> p s w", s=2))
        nc.gpsimd.collective_compute(
            "AllGather", OP.bypass, replica_groups=[list(range(NC_))],
            ins=[agin.opt()], outs=[agout.opt()])
        uL = self.uL_t[0:P, 0:W]
        uR = self.uR_t[0:P, 0:W]
        nc.sync.dma_start(uL[:, :], agout[bass.ds(self.regL, 1), :, :])
        nc.sync.dma_start(uR[:, :], agout[bass.ds(self.regR, 1), :, :])
        for (f, bc, wd), off in zip(fields_bcs, offs):
            d3 = self.D3(f)
            gw, xc = f.gw, f.xc
            srcL = uL[:, off:off + f.zr * wd].rearrange("p (z w) -> p z w", w=wd)
            srcR = uR[:, off:off + f.zr * wd].rearrange("p (z w) -> p z w", w=wd)
            nc.scalar.copy(d3[:, :, gw - wd:gw], srcL)
            nc.scalar.copy(d3[:, :, xc - gw:xc - gw + wd], srcR)
            if fix:
                self.edge_fix(f, bc)

    # --- compute blocks ---------------------------------------------------
    def jacobi(self, dst, w_in, rr, lvl):
        """dst = cs * (x-sum + z-sum + y-sum(JY matmul) - rr), the damped
        Jacobi update in the r' = r/wA_xp scaled basis (rr = b/wA_xp at L0,
        or the scaled residual at coarse levels).  w_in ghosts valid."""
        nc, C = self.nc, self.C
        xc = w_in.xc
        ps = self.mm(f'JY{lvl}', w_in, w_in.P)
        p6 = self.sub(self.sx0, lvl)
        nc.scalar.copy(self.T(p6), ps[:, xc: w_in.F - xc])   # park PSUM -> fp16
        gz = self.sub(self.gz0, lvl)
        nc.gpsimd.tensor_add(self.T(gz), self.T(w_in, xc), self.T(w_in, -xc))
        s = self.sub(self.tx0, lvl)
        nc.vector.tensor_add(self.T(s), self.T(w_in, 1), self.T(w_in, -1))
        nc.vector.tensor_add(self.T(s), self.T(s), self.T(gz))
        nc.vector.tensor_sub(self.T(s), self.T(s), self.T(rr))
        nc.vector.tensor_add(self.T(s), self.T(s), self.T(p6))
        nc.vector.tensor_scalar_mul(self.T(dst), self.T(s), C['cs'])

    def residual(self, dst, pd, bA):
        """dst = (A pd - b)/wA_xp at L0, accumulated in fp32 (cancellation);
        pd ghosts valid, bA = b/wA_xp."""
        nc, C = self.nc, self.C
        xc = pd.xc
        ps = self.mm('AY0', pd, 128)
        pst = ps[:, xc: pd.F - xc]
        gz = self.gz0
        nc.gpsimd.tensor_add(self.T(gz), self.T(pd, xc), self.T(pd, -xc))
        s32 = self.f32s
        nc.vector.tensor_add(self.T(s32), self.T(pd, 1), self.T(pd, -1))
        nc.vector.tensor_add(self.T(s32), self.T(s32), self.T(gz))
        nc.vector.tensor_add(self.T(s32), self.T(s32), pst)
        nc.vector.tensor_sub(self.T(dst), self.T(s32), self.T(bA))

    def restrict(self, r_f, r_c, lf):
        """r_c (level lf+1) interior = w_res-weighted 2x2x2 sum of r_f (level lf)."""
        nc, C = self.nc, self.C
        g = GEOM[lf]
        P, zr, xc = g[0], g[2], g[3]
        F = zr * xc
        gc = GEOM[lf + 1]
        Pc = gc[0]
        ps = self.mm(f'R{lf}', r_f, Pc)
        t0 = self.gz0.t[0:Pc, 0:F]
        nc.scalar.copy(t0, ps[:, 0:F])
        t1 = self.sx0.t[0:Pc, 0:F]
        t2 = self.tx0.t[0:Pc, 0:F]
        nc.vector.tensor_add(t1[:, 0:F - 1], t0[:, 0:F - 1], t0[:, 1:F])
        nc.vector.tensor_add(t2[:, 0:F - xc - 1], t1[:, 0:F - xc - 1], t1[:, xc:F - 1])
        # strided gather: coarse cells <- fine pair sums.  For lf==0 also
        # produce the coarse x-ghost ring-1 (computable from the extended
        # fine residual) so L1 never needs its own halo exchange.
        zi = gc[2] - 2
        gwf, gwc = GEOM[lf][5], GEOM[lf + 1][5]
        d3c = self.D3(r_c)
        t23 = t2[:, 0:F].rearrange("p (z x) -> p z x", x=xc)
        if lf == 0:
            xi = gc[3] - 2 * gwc + 2        # interior + ghost ring-1 (34)
            c0, f0 = gwc - 1, gwf - 2       # coarse col 1 <- fine cols (2,3)
        else:
            xi = gc[3] - 2 * gwc
            c0, f0 = gwc, gwf
        nc.vector.tensor_scalar_mul(
            d3c[:, 1:1 + zi, c0:c0 + xi],
            t23[:, 1:1 + 2 * zi:2, f0:f0 + 2 * xi:2],
            2.0 * C['wres'])

    def prolong_mm(self, w_c, lf):
        """y-doubling matmul of level lf+1 tile -> psum [P_lf, F_{lf+1}]"""
        return self.mm(f'PR{lf}', w_c, GEOM[lf][0])

    def parity_views(self, ps, lvl_f, dst3):
        """yield (dst_quadrant, psum_quadrant) for the 4 z/x parities."""
        gf, gc = GEOM[lvl_f], GEOM[lvl_f + 1]
        zrf, xcf = gf[2], gf[3]
        zrc, xcc = gc[2], gc[3]
        ps3 = ps[:, 0:zrc * xcc].rearrange("p (z x) -> p z x", x=xcc)
        for pz in (0, 1):
            nzf = (zrf - pz + 1) // 2
            cz = 0 if pz == 0 else 1
            for px in (0, 1):
                nxf = (xcf - px + 1) // 2
                cx = 0 if px == 0 else 1
                yield (dst3[:, pz::2, px::2], ps3[:, cz:cz + nzf, cx:cx + nxf])

    def prolong_copy(self, w_c, w_f, lf):
        """w_f = prolong(w_c) including ghosts (coarse ghosts must be valid)."""
        nc = self.nc
        if lf == 1:
            # extract this core's 20-col x window (incl. both ghost rings) of
            # the replicated L2 field via a padded DRAM bounce (dynamic
            # SBUF-side DMA offsets hang on hardware), y-double via PR1, and
            # expand with per-zh coarse row offsets.  Fine cols {2k, 2k+1}
            # map to window col k.
            d3w2 = self.D3(w_c)
            win3 = self.w2win.t[:, 0:360].rearrange("p (z x) -> p z x", x=20)
            w2d = self.dram.tile([16, 18, 132], F16, tag='w2d', name=f'w2d{nc.next_id()}')
            nc.sync.dma_start(w2d[:, :, 0:130], d3w2[:, :, :])
            nc.sync.dma_start(win3[:, :, :], w2d[:, :, bass.ds(self.reg_w2, 20)])
            ps = self.mm('PR1', Fld(self.w2win.t[:, 0:360], 1), 64, psum_w=360)
            ps3 = ps[:, 0:360].rearrange("p (z x) -> p z x", x=20)
            d3 = self.D3(w_f)
            for zh in (0, 1):
                czh = 8 * zh
                psl = slice(zh * 32, (zh + 1) * 32)
                for pz in (0, 1):
                    nzf = (18 - pz + 1) // 2
                    cz = czh + (0 if pz == 0 else 1)
                    for fx0 in (0, 1):
                        nc.scalar.copy(
                            d3[psl, pz::2, fx0::2],
                            ps3[psl, cz:cz + nzf, 0:18])
            return
        ps = self.prolong_mm(w_c, lf)
        d3 = self.D3(w_f)
        for dq, pq in self.parity_views(ps, lf, d3):
            self.nc.scalar.copy(dq, pq)

    def prolong_sub(self, w_c, pd_old, pd_new, lf):
        """pd_new = pd_old - prolong(w_c) (w_c is true-basis), covering
        interior + ghost rings 1-2.  Fine level 0 has gw=3: fine col c maps
        to coarse col (c-3)//2+1."""
        assert lf == 0
        ps = self.prolong_mm(w_c, lf)
        gf, gc = GEOM[lf], GEOM[lf + 1]
        zrf, xcf = gf[2], gf[3]
        zrc, xcc = gc[2], gc[3]
        ps3 = ps[:, 0:zrc * xcc].rearrange("p (z x) -> p z x", x=xcc)
        d3n, d3o = self.D3(pd_new), self.D3(pd_old)
        for pz in (0, 1):
            nzf = (zrf - pz + 1) // 2
            cz = 0 if pz == 0 else 1
            for fx0 in (0, 1):
                dq = d3n[:, pz::2, fx0::2]
                oq = d3o[:, pz::2, fx0::2]
                pq = ps3[:, cz:cz + nzf, 0:36]
                self.nc.vector.scalar_tensor_tensor(
                    dq, pq, -1.0, oq, OP.mult, OP.add)

    def dbg_dump(self, name, f):
        if self.dbg_name == name and not self.dbg_written:
            self.nc.sync.dma_start(self.p_dbg[0:f.P, 0:f.F], self.V(f))
            self.dbg_written = True

    # --- main build -------------------------------------------------------
    def build(self):
        nc, C = self.nc, self.C
        with TileContext(nc) as tc:
            with tc.tile_pool(name="main", bufs=1) as pool, \
                 tc.tile_pool(name="psum", bufs=1, space="PSUM") as psum_pool, \
                 tc.tile_pool(name="dram", bufs=1, space="DRAM") as dram:
                self.pool, self.psum_pool, self.dram = pool, psum_pool, dram
                self._build_body(tc)
        nc.finalize()
        return nc

    def _load_fld(self, pname, name, lvl, tag=None, eng=None):
        f = self.fld(name, lvl, tag=tag)
        (eng or self.nc.sync).dma_start(self.V(f), self.p_in[pname][:, :])
        return f

    def _store_fld(self, f, ch, eng=None):
        (eng or self.nc.sync).dma_start(self.p_out[ch, :, :], self.V(f))

    def _build_body(self, tc):
        nc, C = self.nc, self.C
        pool = self.pool
        if self.stage == 'io0':
            t = self._load_fld('values_u', 'u', 0)
            for ch in range(4):
                self._store_fld(t, ch)
            return
        # constants / matrices / masks
        self.mats_t = pool.tile([128, self.MC], F16, tag="mats", name="mats_t")
        nc.sync.dma_start(self.mats_t[:, :], self.p_mats[:, :])
        self.masks_t = pool.tile([128, 4], F32, tag="masks", name="masks_t")
        nc.sync.dma_start(self.masks_t[:, :], self.p_masks[:, :])
        hoffs_t = pool.tile([1, 3], I32, tag="hoffs", name="hoffs_t")
        nc.sync.dma_start(hoffs_t[:, :], self.p_hoffs[:, :])
        # slot index registers for halo unpack
        self.regL = nc.sync.value_load(hoffs_t[0:1, 0:1], min_val=None, max_val=None)
        self.regR = nc.sync.value_load(hoffs_t[0:1, 1:2], min_val=None, max_val=None)
        self.reg_w2 = nc.sync.value_load(hoffs_t[0:1, 2:3], min_val=None, max_val=None)

        if self.stage == 'io':
            t = self._load_fld('values_u', 'u', 0)
            for ch in range(4):
                self._store_fld(t, ch)
            return

        # ---- loads (one contiguous DMA per field)
        u = self._load_fld('values_u', 'u', 0)
        v = self._load_fld('values_v', 'v', 0, eng=nc.scalar)
        w = self._load_fld('values_w', 'w', 0, eng=nc.gpsimd)
        a = self._load_fld('alpha', 'a', 0, eng=nc.scalar)
        pd0 = self._load_fld('values_pd', 'pd0', 0, eng=nc.gpsimd)

        # scratch needed by edge_fix (used inside exchange unpack)
        self.ebc_t = pool.tile([128, 34], F16, tag='ebc', name='ebc_t')
        # shared halo-exchange staging (sized for the largest exchange: ag1
        # has W = 34*(2+2+2+1+4) = 374)
        self.pk_t = pool.tile([128, 748], F16, tag='pk', name='pk_t')
        self.uL_t = pool.tile([128, 374], F16, tag='uLs', name='uL_t')
        self.uR_t = pool.tile([128, 374], F16, tag='uRs', name='uR_t')
        # scratch sized for the largest level (L0 is 34*72=2448)
        self.gz0 = Fld(pool.tile([128, 2448], F16, tag='gz0', name='gz0'), 0)
        self.sx0 = Fld(pool.tile([128, 2448], F16, tag='sx0', name='sx0'), 0)
        self.tx0 = Fld(pool.tile([128, 2448], F16, tag='tx0', name='tx0'), 0)
        self.f32s = Fld(pool.tile([128, 2448], F32, tag='f32s', name='f32s'), 0)
        if self.dbg_init:
            for t_ in (self.gz0.t, self.sx0.t, self.tx0.t, self.f32s.t, self.ebc_t):
                self.nc.vector.memset(t_[:, :], 0.0)

        # ---- merged ghost exchange: u,v,w wd2 (stars valid to ring 1,
        # killing the post-predictor exchange), alpha wd1, pd wd4.
        for f, bc in ((u, BC_U), (v, BC_V), (w, BC_W), (pd0, BC_PD)):
            self.prep_z(f, bc)
        self.exchange([(u, BC_U, 2), (v, BC_V, 2), (w, BC_W, 2),
                       (a, BC_A, 1), (pd0, BC_PD, 4)], 'ag1')

        # ---- rho chain in fp32 (1/rho ~ 1e-3 would lose precision in
        # fp16 scaling products); issued after the exchange packs.
        rho = self.fld('rho', 0, dt=F32)
        buoy = self.fld('buoy', 0)
        nc.scalar.copy(self.T(rho), self.T(a))      # fp16 -> fp32
        nc.vector.tensor_scalar(self.T(rho), self.T(rho), 0.05, 1.0, OP.max, OP.min)
        nc.vector.tensor_scalar(self.T(rho), self.T(rho), RHO_L - RHO_G, RHO_G, OP.mult, OP.add)
        rinv = self.fld('rinv', 0, dt=F32)
        nc.vector.reciprocal(self.T(rinv), self.T(rho))
        nc.vector.tensor_scalar(self.T(buoy), self.T(rinv), -DT * G_Z * RHO_L, DT * G_Z, OP.mult, OP.add)
        rP1 = Fld(rho.t, 0)   # rho dead after rinv
        nc.scalar.mul(self.T(rP1), self.T(rinv), DT * C['axp'])

        # combined advection+diffusion x/z multipliers (shared by u,v,w):
        #   f(+1)*axp_ + f(-1)*axm_ = DT*wd_xp*(f+1 + f-1) - DT*axp*adv*(f+1 - f-1)
        axp_ = self.fld('axp_', 0)
        axm_ = self.fld('axm_', 0)
        wtp_ = self.fld('wtp_', 0)
        wtm_ = self.fld('wtm_', 0)
        vt2 = self.fld('vt2', 0)
        nc.vector.tensor_scalar(self.T(axp_), self.T(u), -DT * C['axp'], DT * C['wd_xp'], OP.mult, OP.add)
        nc.vector.tensor_scalar(self.T(axm_), self.T(u), DT * C['axp'], DT * C['wd_xm'], OP.mult, OP.add)
        nc.vector.tensor_scalar(self.T(wtp_), self.T(w), -DT * C['azp'], DT * C['wd_zp'], OP.mult, OP.add)
        nc.vector.tensor_scalar(self.T(wtm_), self.T(w), DT * C['azp'], DT * C['wd_zm'], OP.mult, OP.add)
        nc.vector.tensor_scalar_mul(self.T(vt2), self.T(v), -DT)

        xc = u.xc
        us, vs, ws = self.fld('us', 0), self.fld('vs', 0), self.fld('ws', 0)
        sx, tx, gz = self.sx0, self.tx0, self.gz0
        for f, bc, dst, extra in ((u, BC_U, us, None), (v, BC_V, vs, None), (w, BC_W, ws, buoy)):
            nm = 'u' if f is u else ('v' if f is v else 'w')
            # PE matmuls park eagerly into SBUF via ACT so the single PSUM
            # slot frees up and all six predictor matmuls pipeline during
            # the halo exchange (they need no x-ghosts).
            ps = self.mm('MD_' + nm, f, 128)
            nc.scalar.copy(self.T(dst), ps[:, xc: f.F - xc])
            ps2 = self.mm('DY_' + nm, f, 128)
            nc.scalar.copy(self.T(tx), ps2[:, xc: f.F - xc])
            # gpsimd: z diffusion+advection terms (STT form: 0.6 eff)
            nc.gpsimd.tensor_mul(self.T(sx), self.T(f, xc), self.T(wtp_))
            nc.gpsimd.tensor_mul(self.T(gz), self.T(f, -xc), self.T(wtm_))
            nc.gpsimd.tensor_add(self.T(sx), self.T(sx), self.T(gz))
            # vector: y advection, then combined x terms
            nc.vector.tensor_mul(self.T(tx), self.T(tx), self.T(vt2))
            nc.vector.tensor_add(self.T(dst), self.T(dst), self.T(tx))
            nc.vector.tensor_mul(self.T(tx), self.T(f, 1), self.T(axp_))
            nc.vector.tensor_add(self.T(dst), self.T(dst), self.T(tx))
            nc.vector.tensor_mul(self.T(tx), self.T(f, -1), self.T(axm_))
            nc.vector.tensor_add(self.T(dst), self.T(dst), self.T(tx))
            nc.vector.tensor_add(self.T(dst), self.T(dst), self.T(sx))
            if extra is not None:
                nc.vector.tensor_add(self.T(dst), self.T(dst), self.T(extra))
        self.dbg_dump('us', us)
        self.dbg_dump('vs', vs)
        self.dbg_dump('ws', ws)
        if self.stage == 'pred':
            self._store_fld(us, 0)
            self._store_fld(vs, 1)
            self._store_fld(ws, 2)
            self._store_fld(ws, 3)
            return

        # ---- divergence -> b -> bA = b/wA_xp  (stars valid through ring 1
        # on interior cores; edge cores get BC ring-1 values via edge_fix)
        self.prep_z(ws, BC_W)
        for f, bc in ((us, BC_U), (vs, BC_V), (ws, BC_W)):
            self.edge_fix(f, bc)
        b = Fld(buoy.t, 0)  # buoy dead after ws
        cb = -(DX * DX / DT)
        cbx = cb * C['axp']
        cbz = cb * C['azp']
        ps = self.mm('DY_v', vs, 128)
        nc.scalar.mul(self.T(b), ps[:, xc: vs.F - xc], cb)
        nc.gpsimd.tensor_sub(self.T(sx), self.T(ws, xc), self.T(ws, -xc))
        nc.vector.tensor_sub(self.T(tx), self.T(us, 1), self.T(us, -1))
        nc.vector.tensor_scalar_mul(self.T(tx), self.T(tx), cbx)
        nc.vector.tensor_add(self.T(b), self.T(b), self.T(tx))
        nc.vector.tensor_scalar_mul(self.T(gz), self.T(sx), cbz)
        nc.vector.tensor_add(self.T(b), self.T(b), self.T(gz))
        self.exchange([(b, BC_PD, 3)], 'b', fix=False)
        # scaled-basis RHS (valid on ghost cols too after the exchange)
        bA = b
        nc.vector.tensor_scalar_mul(self.V(bA), self.V(b), 1.0 / C['wA_xp'])
        self.dbg_dump('b', b)
        if self.stage == 'div':
            self._store_fld(us, 0)
            self._store_fld(vs, 1)
            self._store_fld(ws, 2)
            self._store_fld(b, 3)
            return

        # ---- multigrid: 2 V-cycles
        # coarse-level tiles live inside v/w (dead after the predictor)
        r1 = Fld(v.t[0:64, 0:648], 1)
        wp1_ = Fld(v.t[0:64, 648:1296], 1)
        wsm1_ = Fld(v.t[0:64, 1296:1944], 1)
        r2l = v.t[0:32, 1944:2072]
        self.w2win = Fld(v.t[0:16, 2072:2432], 1)
        r2 = Fld(w.t[0:16, 0:2340], 2)
        pdA = Fld(u.t, 0)     # u dead after predictor
        pdB = Fld(wtp_.t, 0)  # wtp_ dead after predictor
        pdC = Fld(wtm_.t, 0)  # wtm_ dead after predictor
        r0 = Fld(vt2.t, 0)    # vt2 dead after predictor
        r3 = self.fld('r3', 3)
        r4 = self.fld('r4', 4)
        r5 = self.fld('r5', 5)
        r6 = self.fld('r6', 6)
        wts = {l: (self.fld(f'wp{l}', l), self.fld(f'wsm{l}', l)) for l in range(2, 6)}
        wts[1] = (wp1_, wsm1_)
        w6 = self.fld('w6', 6)

        pd_cur = pd0
        rot = [pdB, pdC, pdA]
        ri = 0
        for vc in range(2):
            # pre-smooth (vc0: pd0 exchanged in the merged AG up front;
            # vc1: exchange here)
            if vc > 0:
                self.prep_z(pd_cur, BC_PD)
                self.exchange([(pd_cur, BC_PD, 4)], 'pd2')
            if self.stage == 'exch1' and vc == 0:
                for ch in range(4):
                    self._store_fld(pd_cur, ch)
                return
            pd1 = rot[ri % 3]; ri += 1
            self.jacobi(pd1, pd_cur, bA, 0)
            self.edge_fix(pd1, BC_PD)
            if self.stage == 'jac1' and vc == 0:
                for ch in range(4):
                    self._store_fld(pd1, ch)
                return
            # residual: pd1 ghost ring-1 is valid from the extended pre-smooth
            self.prep_z(pd1, BC_PD)
            self.residual(r0, pd1, bA)
            if self.stage == 'resid' and vc == 0:
                for ch in range(4):
                    self._store_fld(pd1, ch)
                return
            # restrict chain
            self.restrict(r0, r1, 0)
            # r1 -> r2 local window, then allgather full r2
            self._restrict_r1_to_r2(r1, r2l, r2)
            self.restrict(r2, r3, 2)
            self.restrict(r3, r4, 3)
            self.restrict(r4, r5, 4)
            self.restrict(r5, r6, 5)
            # coarsest (r6 is scaled by 1/wA_xp, w6 is true-basis):
            # w6 = rb * r6_true = rb * wA_xp * r6' = -cs * r6'
            nc.scalar.mul(self.V(w6), self.V(r6), -C['cs'])
            self.prep_z(w6, BC_PD)
            self.prep_x_bc(w6, BC_PD)
            # up-leg 5..1
            wc = w6
            for l in range(5, 0, -1):
                wp, wsm = wts[l]
                self.prolong_copy(wc, wp, l)
                self.jacobi(wsm, wp, (r1 if l == 1 else (r2, r3, r4, r5)[l - 2]), l)
                self.prep_z(wsm, BC_PD)
                if l >= 2:
                    self.prep_x_bc(wsm, BC_PD)
                wc = wsm
            # correction + post-smooth
            pd2 = rot[ri % 3]; ri += 1
            self.prolong_sub(wc, pd1, pd2, 0)
            if self.stage == 'corr' and vc == 0:
                for ch in range(4):
                    self._store_fld(pd2, ch)
                return
            pd3 = rot[ri % 3]; ri += 1
            self.jacobi(pd3, pd2, bA, 0)
            self.edge_fix(pd3, BC_PD)
            pd_cur = pd3
            self.dbg_dump(f'pd_vc{vc}', pd3)
            if self.stage == 'vc1' and vc == 0:
                self._store_fld(us, 0)
                self._store_fld(vs, 1)
                self._store_fld(ws, 2)
                self._store_fld(pd_cur, 3)
                return

        # ---- projection in fp32-mixed: pd ghost ring-1 valid
        self.prep_z(pd_cur, BC_PD)
        rP2 = rinv
        s32 = self.f32s
        ps = self.mm('DY_pd', pd_cur, 128)
        nc.vector.tensor_scalar_mul(self.T(s32), ps[:, xc: pd_cur.F - xc], DT)
        nc.vector.tensor_mul(self.T(s32), self.T(s32), self.T(rP2))
        nc.vector.tensor_sub(self.T(vs), self.T(vs), self.T(s32))
        nc.vector.tensor_sub(self.T(tx), self.T(pd_cur, 1), self.T(pd_cur, -1))
        nc.vector.tensor_mul(self.T(s32), self.T(tx), self.T(rP1))
        nc.vector.tensor_sub(self.T(us), self.T(us), self.T(s32))
        nc.gpsimd.tensor_sub(self.T(gz), self.T(pd_cur, xc), self.T(pd_cur, -xc))
        nc.vector.tensor_mul(self.T(s32), self.T(gz), self.T(rP1))
        nc.vector.tensor_sub(self.T(ws), self.T(ws), self.T(s32))

        # ---- store (contiguous, spread across engine DMA queues)
        self._store_fld(us, 0)
        self._store_fld(vs, 1, eng=nc.scalar)
        self._store_fld(ws, 2, eng=nc.gpsimd)
        self._store_fld(pd_cur, 3)

    def _restrict_r1_to_r2(self, r1, r2l, r2):
        """restrict sharded r1 -> local coarse window, allgather -> full r2."""
        nc, C = self.nc, self.C
        g = GEOM[1]
        P, zr, xc = g[0], g[2], g[3]
        F = zr * xc
        ps = self.mm('R1', r1, P // 2)
        t0 = self.gz0.t[0:P // 2, 0:F]
        nc.scalar.copy(t0, ps[:, 0:F])
        t1 = self.sx0.t[0:P // 2, 0:F]
        t2 = self.tx0.t[0:P // 2, 0:F]
        nc.vector.tensor_add(t1[:, 0:F - 1], t0[:, 0:F - 1], t0[:, 1:F])
        nc.vector.tensor_add(t2[:, 0:F - xc - 1], t1[:, 0:F - xc - 1], t1[:, xc:F - 1])
        t23 = t2[:, 0:F].rearrange("p (z x) -> p z x", x=xc)
        # compact local window [32p, 8z * 16x]
        r2l3 = r2l[:, 0:128].rearrange("p (z x) -> p z x", x=16)
        nc.vector.tensor_scalar_mul(r2l3[:, :, :], t23[:, 1:17:2, 2:34:2], 2.0 * C['wres'])
        agin = self.dram.tile([1, 32 * 128], F16, tag='agr2i', name=f'agr2i{nc.next_id()}')
        agout = self.dram.tile([NC_, 2, 16, 8, 16], F16, tag='agr2o', name=f'agr2o{nc.next_id()}',
                               addr_space="Shared")
        nc.sync.dma_start(agin[0:1, :], r2l[:, 0:128])
        nc.gpsimd.collective_compute(
            "AllGather", OP.bypass, replica_groups=[list(range(NC_))],
            ins=[agin.opt()], outs=[agout.opt()])
        d3 = self.D3(r2)
        engs = (nc.sync, nc.scalar, nc.gpsimd)
        for s in range(NC_):
            for zh in (0, 1):
                engs[(2 * s + zh) % 3].dma_start(
                    d3[:, 1 + zh * 8:9 + zh * 8, 1 + s * 16:17 + s * 16],
                    agout[s, zh, :, :, :])


# ---------------------------------------------------------------- entry
_CACHE = {}


def _get_nc(key, C, dbg_name=None, stage='full', dbg_init=False):
    ck = (key, dbg_name, stage, dbg_init)
    if ck not in _CACHE:
        mats_np, cols = build_mats(C)
        b = B(C, mats_np, cols, dbg_name=dbg_name, stage=stage, dbg_init=dbg_init)
        nc = b.build()
        _CACHE[ck] = (nc, mats_np)
    return _CACHE[ck]


def _pad_field(arr):
    """[64z, 64y, 64x] -> padded device tile [128, 34*72] (fp16)."""
    t = np.zeros((128, 34, 72), np.float16)
    # p = zh*64 + y ; row z' = 1..32 ; col 4..67
    a = arr.reshape(2, 32, 64, 64).transpose(0, 2, 1, 3).reshape(128, 32, 64)
    t[:, 1:33, 4:68] = a
    return t.reshape(128, 34 * 72)


def _make_in_maps(fields, mats_np):
    in_maps = []
    for r in range(NC_):
        m = {}
        for nm, arr in fields.items():
            loc = np.asarray(arr, np.float32)[0, 0, :, :, r * XL:(r + 1) * XL]
            m[nm] = _pad_field(loc)
        m['mats'] = mats_np
        msk = np.zeros((128, 4), np.float32)
        msk[:, 0] = 1.0 if r == 0 else 0.0       # mL
        msk[:, 1] = 0.0 if r == 0 else 1.0       # nmL
        msk[:, 2] = 1.0 if r == NC_ - 1 else 0.0  # mR
        msk[:, 3] = 0.0 if r == NC_ - 1 else 1.0  # nmR
        m['masks'] = msk
        ho = np.zeros((1, 3), np.int32)
        rl = max(r - 1, 0)
        rr = min(r + 1, NC_ - 1)
        ho[0, 0] = rl * 2 + 1   # left ghost <- left nbr's right-edge slot
        ho[0, 1] = rr * 2 + 0   # right ghost <- right nbr's left-edge slot
        ho[0, 2] = r * 16
        m['hoffs'] = ho
        in_maps.append(m)
    return in_maps


def kernel(alpha, values_u, values_v, values_w, values_pd,
           w_diff, w_xadv, w_yadv, w_zadv, w_A, w_res, _dbg=None, _stage='full', _dbg_init=False):
    C = extract_consts(w_diff, w_xadv, w_yadv, w_zadv, w_A, w_res)
    key = tuple(sorted(C.items()))
    nc, mats_np = _get_nc(key, C, dbg_name=_dbg, stage=_stage, dbg_init=_dbg_init)
    fields = {'alpha': alpha, 'values_u': values_u, 'values_v': values_v,
              'values_w': values_w, 'values_pd': values_pd}
    in_maps = _make_in_maps(fields, mats_np)
    res = run_bass_kernel_spmd(nc, in_maps, core_ids=list(range(NC_)))
    full = np.empty((4, NZ, NY, NX), np.float32)
    for r in range(NC_):
        o = res.results[r]['out'].reshape(4, 128, 34, 72)[:, :, 1:33, 4:68].astype(np.float32)
        # [4, (zh y), z', x] -> [4, (zh z'), y, x]
        o = o.reshape(4, 2, 64, 32, 64).transpose(0, 1, 3, 2, 4).reshape(4, 64, 64, 64)
        full[:, :, :, r * XL:(r + 1) * XL] = o
    if _dbg is not None:
        kernel._dbg_res = [res.results[r].get('dbg') for r in range(NC_)]
    return full[None]  # (1, 4, 64, 64, 512)


# revision 10
# speedup vs baseline: 2.5656x; 1.0712x over previous
"""Trainium2 Bass kernel for the multiphase CFD fractional-step solver
(predictor + divergence + 2 multigrid V-cycles + projection) on a
64x64x512 grid, sharded along x across 8 NeuronCores.

Self-contained: hardcodes shapes/sharding; reads stencil coefficient
VALUES from the runtime weight inputs and compiles a specialized graph
(cached per coefficient set).

Device layout (level l):
  partitions p = zh*ny + y   (zh in {0,1} z-halves; l=6 has P=1)
  free       j = z'*xc + x   (z' in [0, zr): rows 0 and zr-1 are z-ghosts;
                              x in [0, xc): cols 0 and xc-1 are x-ghosts)
Volume passes run on the row-trimmed flat range [xc, F-xc) so all
+-1 / +-xc shifted reads stay inside the [P, F] tile.
y-axis stencil taps (partition axis) are done on the TensorEngine as
[K,M] matmuls with per-field boundary rows baked into the matrices.

Precision: fields and stencil passes run in fp16 (the DVE gets 2x/4x
element rates for packed 2-byte operands and the PE runs fp16 matmuls
at 4x the fp32 rate).  fp32 is kept where it matters: the rho/1-rho
chain (values ~1e-3 would denormal in fp16 scaling), the residual
accumulation (cancellation), and the projection corrections.  The
Jacobi/residual y-matrices are pre-scaled so the per-tap coefficient
application collapses into a single tensor_scalar, exploiting the
operator's full tap symmetry (asserted in extract_consts).

I/O: the host pre-pads each field into the device tile layout
[128, zr*xc] fp16 so every load/store is one fully-contiguous DMA.
x halos travel through AllGathers with contiguous staged pack/unpack.
One merged exchange up front carries u,v,w (wd2) + alpha (wd1) +
pd (wd4): the predictor produces stars valid through ghost ring 1 and
the post-predictor exchange disappears.  Remaining collectives:
b (wd3), replicated-r2 gather per V-cycle, pd re-exchange before
V-cycle 2.  Multigrid levels 0-1 stay sharded; levels 2-6 replicated.
"""
import sys
sys.path.insert(0, '/opt/trn_rl_repo')
import numpy as np
import concourse.bass as bass
import concourse.bacc as bacc
import concourse.mybir as mybir
from concourse.bass_utils import run_bass_kernel_spmd
from concourse.tile import TileContext

F32 = mybir.dt.float32
F16 = mybir.dt.float16
I32 = mybir.dt.int32
OP = mybir.AluOpType

DT, DX, G_Z = 0.002, 0.04, -10.0
RHO_L, RHO_G, NU = 1000.0, 1.0, 1e-3
NZ, NY, NX = 64, 64, 512
NC_ = 8
XL = NX // NC_  # 64 local x

# level: (P, ny, zr, xc, sharded, gw) -- gw = x-ghost cols per side
GEOM = {
    0: (128, 64, 34, 72, True, 4),
    1: (64, 32, 18, 36, True, 2),
    2: (16, 16, 18, 130, False, 1),
    3: (8, 8, 10, 66, False, 1),
    4: (4, 4, 6, 34, False, 1),
    5: (2, 2, 4, 18, False, 1),
    6: (1, 1, 3, 10, False, 1),
}
# BC per field: axis -> (lo, hi), 'n' neumann (ghost=adjacent), 'd' dirichlet (ghost=0)
BC_U = {'z': ('n', 'n'), 'y': ('n', 'n'), 'x': ('d', 'd')}
BC_V = {'z': ('n', 'n'), 'y': ('d', 'd'), 'x': ('n', 'n')}
BC_W = {'z': ('d', 'd'), 'y': ('n', 'n'), 'x': ('n', 'n')}
BC_PD = {'z': ('n', 'd'), 'y': ('n', 'n'), 'x': ('n', 'n')}
BC_A = {'z': ('n', 'n'), 'y': ('n', 'n'), 'x': ('n', 'n')}


# ---------------------------------------------------------------- host-side
def _yblock(ny, cm, cc, cp, bc):
    """[ny, ny] matrix M with out[y] = sum_k M[k, y] in[k]:
    tridiag with sub=cm (coeff of in[y-1]), diag=cc, super=cp (in[y+1]),
    Neumann BC folds the ghost coeff into the boundary diagonal."""
    m = np.zeros((ny, ny), np.float32)
    for y in range(ny):
        m[y, y] += cc
        if y > 0:
            m[y - 1, y] += cm
        elif bc[0] == 'n':
            m[y, y] += cm
        if y < ny - 1:
            m[y + 1, y] += cp
        elif bc[1] == 'n':
            m[y, y] += cp
    return m


def _blkdiag2(b):
    n = b.shape[0]
    m = np.zeros((2 * n, 2 * b.shape[1]), np.float32)
    m[:n, :b.shape[1]] = b
    m[n:, b.shape[1]:] = b
    return m


def _halve(ny):
    m = np.zeros((ny, ny // 2), np.float32)
    for y in range(ny):
        m[y, y // 2] = 0.5
    return m


def _double(nyc, nyf):
    m = np.zeros((nyc, nyf), np.float32)
    for y in range(nyf):
        m[y // 2, y] = 1.0
    return m


def build_mats(C):
    """Concatenated [128, sum M] lhsT matrices (fp16) + column offset map."""
    cols = {}
    parts = []
    total = 0

    def add(name, m, K):
        nonlocal total
        assert m.shape[0] == K and K <= 128 and m.shape[1] <= 128
        buf = np.zeros((128, m.shape[1]), np.float32)
        buf[:K] = m
        cols[name] = (total, m.shape[1], K)
        parts.append(buf)
        total += m.shape[1]

    # predictor diffusion y-taps + center (K=M=128, blockdiag over zh)
    for nm, bc in (('u', BC_U), ('v', BC_V), ('w', BC_W)):
        b = _yblock(64, DT * C['wd_ym'], 1.0 + DT * C['wd_c'], DT * C['wd_yp'], bc['y'])
        add('MD_' + nm, _blkdiag2(b), 128)
    # advection / gradient y-difference (raw tap values)
    for nm, bc in (('u', BC_U), ('v', BC_V), ('w', BC_W), ('pd', BC_PD)):
        b = _yblock(64, C['aym'], 0.0, C['ayp'], bc['y'])
        add('DY_' + nm, _blkdiag2(b), 128)
    # residual y-taps + center at L0, pre-divided by wA_xp so the residual
    # is accumulated in the r' = r/wA_xp basis
    b = _yblock(64, 1.0, C['wA_c'] / C['wA_xp'], 1.0, BC_PD['y'])
    add('AY0', _blkdiag2(b), 128)
    # jacobi y matrices per level 0..5, unit taps: the common factor cs
    # (= -wA_xp/diag) is applied once in a final tensor_scalar
    for l in range(6):
        P, ny = GEOM[l][0], GEOM[l][1]
        b = _yblock(ny, 1.0, 0.0, 1.0, BC_PD['y'])
        add(f'JY{l}', _blkdiag2(b) if l <= 1 else b, P)
    # identity (for PE-accumulated x/z shift taps in the residual)
    add('I0', np.eye(128, dtype=np.float32), 128)
    # restrict y-halving matrices (R{l}: level l -> l+1)
    add('R0', _blkdiag2(_halve(64)), 128)
    add('R1', _blkdiag2(_halve(32)), 64)   # stays (zh,y); zh dissolved in AG unpack
    for l in range(2, 6):
        add(f'R{l}', _halve(GEOM[l][1]), GEOM[l][0])
    # prolong y-doubling: PR{l} maps level l+1 -> l
    add('PR0', _blkdiag2(_double(32, 64)), 64)
    m = _double(16, 32)
    add('PR1', np.concatenate([m, m], axis=1), 16)  # s=1 coarse -> (zh,y) fine
    for l in range(2, 6):
        add(f'PR{l}', _double(GEOM[l + 1][1], GEOM[l][1]), GEOM[l + 1][0])

    return np.concatenate(parts, axis=1).astype(np.float16), cols


def extract_consts(w_diff, w_xadv, w_yadv, w_zadv, w_A, w_res):
    g = lambda a, i, j, k: float(np.asarray(a)[0, 0, i, j, k])
    C = {}
    C['wd_c'] = g(w_diff, 1, 1, 1)
    C['wd_zm'], C['wd_zp'] = g(w_diff, 0, 1, 1), g(w_diff, 2, 1, 1)
    C['wd_ym'], C['wd_yp'] = g(w_diff, 1, 0, 1), g(w_diff, 1, 2, 1)
    C['wd_xm'], C['wd_xp'] = g(w_diff, 1, 1, 0), g(w_diff, 1, 1, 2)
    C['wA_c'] = g(w_A, 1, 1, 1)
    C['wA_zm'], C['wA_zp'] = g(w_A, 0, 1, 1), g(w_A, 2, 1, 1)
    C['wA_ym'], C['wA_yp'] = g(w_A, 1, 0, 1), g(w_A, 1, 2, 1)
    C['wA_xm'], C['wA_xp'] = g(w_A, 1, 1, 0), g(w_A, 1, 1, 2)
    C['axp'], C['axm'] = g(w_xadv, 1, 1, 2), g(w_xadv, 1, 1, 0)
    C['ayp'], C['aym'] = g(w_yadv, 1, 2, 1), g(w_yadv, 1, 0, 1)
    C['azp'], C['azm'] = g(w_zadv, 2, 1, 1), g(w_zadv, 0, 1, 1)
    wr = np.asarray(w_res).ravel()
    assert np.allclose(wr, wr[0]), "nonuniform w_res unsupported"
    C['wres'] = float(wr[0])
    # fast paths used by the kernel
    assert abs(C['axm'] + C['axp']) < 1e-12 * max(1, abs(C['axp']))
    assert abs(C['azm'] + C['azp']) < 1e-12 * max(1, abs(C['azp']))
    # x/z/y diffusion tap symmetry
    assert abs(C['wd_zm'] - C['wd_zp']) < 1e-12 * max(1, abs(C['wd_zp']))
    assert abs(C['wd_xm'] - C['wd_xp']) < 1e-12 * max(1, abs(C['wd_xp']))
    # A-operator full tap symmetry (lets the jacobi/residual scale fold
    # into a single constant cs)
    for k in ('wA_zm', 'wA_zp', 'wA_ym', 'wA_yp', 'wA_xm'):
        assert abs(C[k] - C['wA_xp']) < 1e-12 * max(1, abs(C['wA_xp'])), k
    diag = C['wA_c']
    C['diag'] = diag
    C['jxp'] = -C['wA_xp'] / diag
    C['cs'] = C['jxp']
    C['rb'] = 1.0 / diag
    return C


# ---------------------------------------------------------------- builder
class Fld:
    def __init__(self, t, lvl):
        self.t, self.lvl = t, lvl
        P, ny, zr, xc, _, gw = GEOM[lvl]
        self.P, self.zr, self.xc, self.F, self.gw = P, zr, xc, zr * xc, gw


class B:
    """Builder context."""

    def __init__(self, C, mats_np, mat_cols, dbg_name=None, stage='full', dbg_init=False):
        self.C = C
        self.stage = stage
        self.dbg_init = dbg_init
        self.dbg_name = dbg_name
        self.nc = bacc.Bacc()
        nc = self.nc
        self.mat_cols = mat_cols
        self.MC = mats_np.shape[1]
        # params (fields are pre-padded on host into the device tile layout)
        self.p_in = {}
        for nm in ('alpha', 'values_u', 'values_v', 'values_w', 'values_pd'):
            self.p_in[nm] = nc.declare_dram_parameter(nm, [128, GEOM[0][2] * GEOM[0][3]], F16, isOutput=False)
        self.p_mats = nc.declare_dram_parameter('mats', [128, self.MC], F16, isOutput=False)
        self.p_masks = nc.declare_dram_parameter('masks', [128, 4], F32, isOutput=False)
        self.p_hoffs = nc.declare_dram_parameter('hoffs', [1, 3], I32, isOutput=False)
        self.p_out = nc.declare_dram_parameter('out', [4, 128, GEOM[0][2] * GEOM[0][3]], F16, isOutput=True)
        if dbg_name:
            self.p_dbg = nc.declare_dram_parameter('dbg', [128, GEOM[0][2] * GEOM[0][3]], F16, isOutput=True)
        self.dbg_written = False

    # --- tile helpers -----------------------------------------------------
    def fld(self, name, lvl, tag=None, dt=F16):
        g = GEOM[lvl]
        t = self.pool.tile([g[0], g[2] * g[3]], dt, tag=(tag or name), name=name)
        if self.dbg_init:
            self.nc.vector.memset(t[:, :], 0.0)
        return Fld(t, lvl)

    def sub(self, f, lvl):
        g = GEOM[lvl]
        return Fld(f.t[0:g[0], 0:g[2] * g[3]], lvl)

    def T(self, f, s=0):
        """row-trimmed shifted flat view [P, F-2*xc]"""
        return f.t[:, f.xc + s: f.F - f.xc + s]

    def V(self, f):
        return f.t[:, 0:f.F]

    def D3(self, f):
        return f.t[:, 0:f.F].rearrange("p (z x) -> p z x", x=f.xc)

    def mat(self, name):
        off, M, K = self.mat_cols[name]
        return self.mats_t[0:K, off:off + M]

    def mm(self, name, rhs_f, Pout, psum_w=None):
        """psum[Pout, F] = mats[name].T @ V(rhs)  (chunked, full width)"""
        nc = self.nc
        F = psum_w or rhs_f.F
        ps = self.psum_pool.tile([Pout, F], F32, tag="psA", name=f"ps_{name}_{nc.next_id()}")
        rhs = rhs_f.t[:, 0:F]
        lhsT = self.mat(name)
        for c0 in range(0, F, 512):
            w = min(512, F - c0)
            nc.tensor.matmul(ps[:, c0:c0 + w], lhsT, rhs[:, c0:c0 + w], start=True, stop=True)
        return ps

    # --- ghost prep -------------------------------------------------------
    def prep_z(self, f, bc):
        """fill z ghost rows: global BC rows (+ inter-half swap on levels 0-1)"""
        nc, d3 = self.nc, self.D3(f)
        P, zr = f.P, f.zr
        split = f.lvl <= 1
        lo = slice(0, P // 2) if split else slice(0, P)
        hi = slice(P // 2, P) if split else slice(0, P)
        if bc['z'][0] == 'n':
            nc.scalar.copy(d3[lo, 0, :], d3[lo, 1, :])
        else:
            nc.gpsimd.memset(d3[lo, 0, :], 0.0)
        if bc['z'][1] == 'n':
            nc.scalar.copy(d3[hi, zr - 1, :], d3[hi, zr - 2, :])
        else:
            nc.gpsimd.memset(d3[hi, zr - 1, :], 0.0)
        if split:
            nc.sync.dma_start(d3[lo, zr - 1, :], d3[hi, 1, :])
            nc.sync.dma_start(d3[hi, 0, :], d3[lo, zr - 2, :])

    def prep_x_bc(self, f, bc):
        """replicated levels: plain BC on both x faces"""
        nc, d3 = self.nc, self.D3(f)
        xc = f.xc
        if bc['x'][0] == 'n':
            nc.scalar.copy(d3[:, :, 0], d3[:, :, 1])
        else:
            nc.gpsimd.memset(d3[:, :, 0], 0.0)
        if bc['x'][1] == 'n':
            nc.scalar.copy(d3[:, :, xc - 1], d3[:, :, xc - 2])
        else:
            nc.gpsimd.memset(d3[:, :, xc - 1], 0.0)

    def edge_fix(self, f, bc):
        """overwrite ring-1 ghost cols on the 2 edge cores by BC, via
        per-core mask inputs (mL,nmL,mR,nmR)."""
        nc, d3 = self.nc, self.D3(f)
        P, zr, xc, gw = f.P, f.zr, f.xc, f.gw
        mL, nmL = self.masks_t[0:P, 0:1], self.masks_t[0:P, 1:2]
        mR, nmR = self.masks_t[0:P, 2:3], self.masks_t[0:P, 3:4]
        for (lo, side, m, nm) in ((True, gw - 1, mL, nmL), (False, xc - gw, mR, nmR)):
            gcol = d3[:, :, side]
            if bc['x'][0 if lo else 1] == 'd':
                nc.vector.tensor_scalar_mul(gcol, gcol, nm)
            else:
                icol = d3[:, :, gw if lo else xc - gw - 1]
                tmp = self.ebc_t[0:P, 0:zr]
                nc.vector.tensor_scalar_mul(tmp, icol, m)
                nc.vector.scalar_tensor_tensor(gcol, gcol, nm, tmp, OP.mult, OP.add)

    def exchange_begin(self, fields_bcs, fam):
        """Pack + allgather trigger half of the staged halo exchange.
        fields_bcs: list of (Fld, bc, wd).  Returns state for exchange_end.
        side 0 = left-edge interior (becomes left nbr's right ghost),
        side 1 = right-edge interior (becomes right nbr's left ghost)."""
        nc = self.nc
        f0 = fields_bcs[0][0]
        P = f0.P
        offs, W = [], 0
        for (f, bc, wd) in fields_bcs:
            offs.append(W)
            W += f.zr * wd
        pk = self.pk_t[0:P, 0:2 * W]
        for (f, bc, wd), off in zip(fields_bcs, offs):
            d3 = self.D3(f)
            gw, xc = f.gw, f.xc
            for s, c0 in ((0, gw), (1, xc - gw - wd)):
                dst = pk[:, s * W + off: s * W + off + f.zr * wd].rearrange(
                    "p (z w) -> p z w", w=wd)
                nc.scalar.copy(dst, d3[:, :, c0:c0 + wd])
        agin = self.dram.tile([2, P, W], F16, tag=f'agin_{fam}', name=f'agin{nc.next_id()}')
        agout = self.dram.tile([NC_ * 2, P, W], F16, tag=f'agout_{fam}',
                               name=f'agout{nc.next_id()}', addr_space="Shared")
        nc.sync.dma_start(agin[:, :, :].transpose([1, 0, 2]),
                          pk[:, :].rearrange("p (s w) -> p s w", s=2))
        nc.gpsimd.collective_compute(
            "AllGather", OP.bypass, replica_groups=[list(range(NC_))],
            ins=[agin.opt()], outs=[agout.opt()])
        return (fields_bcs, offs, W, agout)

    def exchange_end(self, st, fix=True):
        """Unpack half: contiguous DMAs of the two neighbor slots + engine
        copies into ghost columns + edge BC fix."""
        nc = self.nc
        fields_bcs, offs, W, agout = st
        P = fields_bcs[0][0].P
        uL = self.uL_t[0:P, 0:W]
        uR = self.uR_t[0:P, 0:W]
        nc.sync.dma_start(uL[:, :], agout[bass.ds(self.regL, 1), :, :])
        nc.sync.dma_start(uR[:, :], agout[bass.ds(self.regR, 1), :, :])
        for (f, bc, wd), off in zip(fields_bcs, offs):
            d3 = self.D3(f)
            gw, xc = f.gw, f.xc
            srcL = uL[:, off:off + f.zr * wd].rearrange("p (z w) -> p z w", w=wd)
            srcR = uR[:, off:off + f.zr * wd].rearrange("p (z w) -> p z w", w=wd)
            nc.scalar.copy(d3[:, :, gw - wd:gw], srcL)
            nc.scalar.copy(d3[:, :, xc - gw:xc - gw + wd], srcR)
            if fix:
                self.edge_fix(f, bc)

    def exchange(self, fields_bcs, fam, fix=True):
        self.exchange_end(self.exchange_begin(fields_bcs, fam), fix=fix)

    # --- compute blocks ---------------------------------------------------
    def jacobi(self, dst, w_in, rr, lvl):
        """dst = cs * (x-sum + z-sum + y-sum(JY matmul) - rr), the damped
        Jacobi update in the r' = r/wA_xp scaled basis (rr = b/wA_xp at L0,
        or the scaled residual at coarse levels).  w_in ghosts valid."""
        nc, C = self.nc, self.C
        xc = w_in.xc
        ps = self.mm(f'JY{lvl}', w_in, w_in.P)
        pst = ps[:, xc: w_in.F - xc]
        gz = self.sub(self.gz0, lvl)
        s = self.sub(self.tx0, lvl)
        nc.vector.tensor_add(self.T(gz), self.T(w_in, xc), self.T(w_in, -xc))
        nc.vector.tensor_add(self.T(s), self.T(w_in, 1), self.T(w_in, -1))
        nc.vector.tensor_add(self.T(s), self.T(s), self.T(gz))
        nc.vector.tensor_sub(self.T(s), self.T(s), self.T(rr))
        nc.vector.tensor_add(self.T(s), self.T(s), pst)
        nc.vector.tensor_scalar_mul(self.T(dst), self.T(s), C['cs'])

    def residual(self, dst, pd, bA):
        """dst = (A pd - b)/wA_xp at L0.  The y-taps+center matrix and the
        four x/z shift taps (identity matmuls over shifted views) accumulate
        into one fp32 PSUM group on the PE - exact fp16-product sums, no
        cancellation loss - leaving a single DVE subtract."""
        nc, C = self.nc, self.C
        xc = pd.xc
        Ft = pd.F - 2 * xc
        ps = self.psum_pool.tile([128, Ft], F32, tag="psA", name=f"ps_res_{nc.next_id()}")
        mA, mI = self.mat('AY0'), self.mat('I0')
        for c0 in range(0, Ft, 512):
            w = min(512, Ft - c0)
            taps = ((mA, 0), (mI, 1), (mI, -1), (mI, xc), (mI, -xc))
            for i, (m, sh) in enumerate(taps):
                nc.tensor.matmul(ps[:, c0:c0 + w],
                                 m, pd.t[:, xc + c0 + sh: xc + c0 + sh + w],
                                 start=(i == 0), stop=(i == len(taps) - 1))
        nc.vector.tensor_sub(self.T(dst), ps[:, 0:Ft], self.T(bA))

    def restrict(self, r_f, r_c, lf):
        """r_c (level lf+1) interior = w_res-weighted 2x2x2 sum of r_f (level lf)."""
        nc, C = self.nc, self.C
        g = GEOM[lf]
        P, zr, xc = g[0], g[2], g[3]
        F = zr * xc
        gc = GEOM[lf + 1]
        Pc = gc[0]
        ps = self.mm(f'R{lf}', r_f, Pc)
        t0 = self.gz0.t[0:Pc, 0:F]
        nc.scalar.copy(t0, ps[:, 0:F])
        t1 = self.sx0.t[0:Pc, 0:F]
        t2 = self.tx0.t[0:Pc, 0:F]
        nc.vector.tensor_add(t1[:, 0:F - 1], t0[:, 0:F - 1], t0[:, 1:F])
        nc.vector.tensor_add(t2[:, 0:F - xc - 1], t1[:, 0:F - xc - 1], t1[:, xc:F - 1])
        # strided gather: coarse cells <- fine pair sums.  For lf==0 also
        # produce the coarse x-ghost ring-1 (computable from the extended
        # fine residual) so L1 never needs its own halo exchange.
        zi = gc[2] - 2
        gwf, gwc = GEOM[lf][5], GEOM[lf + 1][5]
        d3c = self.D3(r_c)
        t23 = t2[:, 0:F].rearrange("p (z x) -> p z x", x=xc)
        if lf == 0:
            xi = gc[3] - 2 * gwc + 2        # interior + ghost ring-1 (34)
            c0, f0 = gwc - 1, gwf - 2       # coarse col 1 <- fine cols (2,3)
        else:
            xi = gc[3] - 2 * gwc
            c0, f0 = gwc, gwf
        nc.vector.tensor_scalar_mul(
            d3c[:, 1:1 + zi, c0:c0 + xi],
            t23[:, 1:1 + 2 * zi:2, f0:f0 + 2 * xi:2],
            2.0 * C['wres'])

    def prolong_mm(self, w_c, lf):
        """y-doubling matmul of level lf+1 tile -> psum [P_lf, F_{lf+1}]"""
        return self.mm(f'PR{lf}', w_c, GEOM[lf][0])

    def parity_views(self, ps, lvl_f, dst3):
        """yield (dst_quadrant, psum_quadrant) for the 4 z/x parities."""
        gf, gc = GEOM[lvl_f], GEOM[lvl_f + 1]
        zrf, xcf = gf[2], gf[3]
        zrc, xcc = gc[2], gc[3]
        ps3 = ps[:, 0:zrc * xcc].rearrange("p (z x) -> p z x", x=xcc)
        for pz in (0, 1):
            nzf = (zrf - pz + 1) // 2
            cz = 0 if pz == 0 else 1
            for px in (0, 1):
                nxf = (xcf - px + 1) // 2
                cx = 0 if px == 0 else 1
                yield (dst3[:, pz::2, px::2], ps3[:, cz:cz + nzf, cx:cx + nxf])

    def prolong_copy(self, w_c, w_f, lf):
        """w_f = prolong(w_c) including ghosts (coarse ghosts must be valid)."""
        nc = self.nc
        if lf == 1:
            # extract this core's 20-col x window (incl. both ghost rings) of
            # the replicated L2 field via a padded DRAM bounce (dynamic
            # SBUF-side DMA offsets hang on hardware), y-double via PR1, and
            # expand with per-zh coarse row offsets.  Fine cols {2k, 2k+1}
            # map to window col k.
            d3w2 = self.D3(w_c)
            win3 = self.w2win.t[:, 0:360].rearrange("p (z x) -> p z x", x=20)
            w2d = self.dram.tile([16, 18, 132], F16, tag='w2d', name=f'w2d{nc.next_id()}')
            nc.sync.dma_start(w2d[:, :, 0:130], d3w2[:, :, :])
            nc.sync.dma_start(win3[:, :, :], w2d[:, :, bass.ds(self.reg_w2, 20)])
            ps = self.mm('PR1', Fld(self.w2win.t[:, 0:360], 1), 64, psum_w=360)
            ps3 = ps[:, 0:360].rearrange("p (z x) -> p z x", x=20)
            d3 = self.D3(w_f)
            for zh in (0, 1):
                czh = 8 * zh
                psl = slice(zh * 32, (zh + 1) * 32)
                for pz in (0, 1):
                    nzf = (18 - pz + 1) // 2
                    cz = czh + (0 if pz == 0 else 1)
                    for fx0 in (0, 1):
                        nc.scalar.copy(
                            d3[psl, pz::2, fx0::2],
                            ps3[psl, cz:cz + nzf, 0:18])
            return
        ps = self.prolong_mm(w_c, lf)
        d3 = self.D3(w_f)
        for dq, pq in self.parity_views(ps, lf, d3):
            self.nc.scalar.copy(dq, pq)

    def prolong_sub(self, w_c, pd_old, pd_new, lf):
        """pd_new = pd_old - prolong(w_c) (w_c is true-basis), covering
        interior + ghost rings 1-2.  Fine level 0 has gw=3: fine col c maps
        to coarse col (c-3)//2+1."""
        assert lf == 0
        ps = self.prolong_mm(w_c, lf)
        gf, gc = GEOM[lf], GEOM[lf + 1]
        zrf, xcf = gf[2], gf[3]
        zrc, xcc = gc[2], gc[3]
        ps3 = ps[:, 0:zrc * xcc].rearrange("p (z x) -> p z x", x=xcc)
        d3n, d3o = self.D3(pd_new), self.D3(pd_old)
        for pz in (0, 1):
            nzf = (zrf - pz + 1) // 2
            cz = 0 if pz == 0 else 1
            for fx0 in (0, 1):
                dq = d3n[:, pz::2, fx0::2]
                oq = d3o[:, pz::2, fx0::2]
                pq = ps3[:, cz:cz + nzf, 0:36]
                self.nc.vector.scalar_tensor_tensor(
                    dq, pq, -1.0, oq, OP.mult, OP.add)

    def dbg_dump(self, name, f):
        if self.dbg_name == name and not self.dbg_written:
            self.nc.sync.dma_start(self.p_dbg[0:f.P, 0:f.F], self.V(f))
            self.dbg_written = True

    # --- main build -------------------------------------------------------
    def build(self):
        nc, C = self.nc, self.C
        with TileContext(nc) as tc:
            with tc.tile_pool(name="main", bufs=1) as pool, \
                 tc.tile_pool(name="psum", bufs=1, space="PSUM") as psum_pool, \
                 tc.tile_pool(name="dram", bufs=1, space="DRAM") as dram:
                self.pool, self.psum_pool, self.dram = pool, psum_pool, dram
                self._build_body(tc)
        nc.finalize()
        return nc

    def _load_fld(self, pname, name, lvl, tag=None, eng=None):
        f = self.fld(name, lvl, tag=tag)
        (eng or self.nc.sync).dma_start(self.V(f), self.p_in[pname][:, :])
        return f

    def _store_fld(self, f, ch, eng=None):
        (eng or self.nc.sync).dma_start(self.p_out[ch, :, :], self.V(f))

    def _build_body(self, tc):
        nc, C = self.nc, self.C
        pool = self.pool
        if self.stage == 'io0':
            t = self._load_fld('values_u', 'u', 0)
            for ch in range(4):
                self._store_fld(t, ch)
            return
        # constants / matrices / masks
        self.mats_t = pool.tile([128, self.MC], F16, tag="mats", name="mats_t")
        nc.sync.dma_start(self.mats_t[:, :], self.p_mats[:, :])
        self.masks_t = pool.tile([128, 4], F32, tag="masks", name="masks_t")
        nc.sync.dma_start(self.masks_t[:, :], self.p_masks[:, :])
        hoffs_t = pool.tile([1, 3], I32, tag="hoffs", name="hoffs_t")
        nc.sync.dma_start(hoffs_t[:, :], self.p_hoffs[:, :])
        # slot index registers for halo unpack
        self.regL = nc.sync.value_load(hoffs_t[0:1, 0:1], min_val=None, max_val=None)
        self.regR = nc.sync.value_load(hoffs_t[0:1, 1:2], min_val=None, max_val=None)
        self.reg_w2 = nc.sync.value_load(hoffs_t[0:1, 2:3], min_val=None, max_val=None)

        # warm-up collective: the first AllGather pays ~40us of CC-engine
        # cold-start + inter-core skew; absorb it behind the load phase
        # with a tiny dummy gather nobody reads.
        warm_s = pool.tile([1, 8], F16, tag='warm', name='warm_s')
        nc.vector.memset(warm_s[:, :], 0.0)
        win = self.dram.tile([1, 8], F16, tag='warmi', name='warmi')
        wout = self.dram.tile([NC_, 8], F16, tag='warmo', name='warmo', addr_space='Shared')
        nc.sync.dma_start(win[0:1, :], warm_s[:, :])
        nc.gpsimd.collective_compute(
            "AllGather", OP.bypass, replica_groups=[list(range(NC_))],
            ins=[win.opt()], outs=[wout.opt()])

        if self.stage == 'io':
            t = self._load_fld('values_u', 'u', 0)
            for ch in range(4):
                self._store_fld(t, ch)
            return

        # ---- loads (one contiguous DMA per field)
        u = self._load_fld('values_u', 'u', 0)
        v = self._load_fld('values_v', 'v', 0, eng=nc.scalar)
        w = self._load_fld('values_w', 'w', 0, eng=nc.gpsimd)
        a = self._load_fld('alpha', 'a', 0, eng=nc.scalar)
        pd0 = self._load_fld('values_pd', 'pd0', 0, eng=nc.gpsimd)

        # scratch needed by edge_fix (used inside exchange unpack)
        self.ebc_t = pool.tile([128, 34], F16, tag='ebc', name='ebc_t')
        # shared halo-exchange staging (sized for the largest exchange: ag1
        # has W = 34*(2+2+2+1+4) = 374)
        self.pk_t = pool.tile([128, 748], F16, tag='pk', name='pk_t')
        self.uL_t = pool.tile([128, 374], F16, tag='uLs', name='uL_t')
        self.uR_t = pool.tile([128, 374], F16, tag='uRs', name='uR_t')
        # scratch sized for the largest level (L0 is 34*72=2448)
        self.gz0 = Fld(pool.tile([128, 2448], F16, tag='gz0', name='gz0'), 0)
        self.sx0 = Fld(pool.tile([128, 2448], F16, tag='sx0', name='sx0'), 0)
        self.tx0 = Fld(pool.tile([128, 2448], F16, tag='tx0', name='tx0'), 0)
        self.f32s = Fld(pool.tile([128, 2448], F32, tag='f32s', name='f32s'), 0)
        if self.dbg_init:
            for t_ in (self.gz0.t, self.sx0.t, self.tx0.t, self.f32s.t, self.ebc_t):
                self.nc.vector.memset(t_[:, :], 0.0)

        # ---- merged ghost exchange: u,v,w wd2 (stars valid to ring 1,
        # killing the post-predictor exchange), alpha wd1, pd wd4.
        for f, bc in ((u, BC_U), (v, BC_V), (w, BC_W), (pd0, BC_PD)):
            self.prep_z(f, bc)
        ag1 = self.exchange_begin([(u, BC_U, 2), (v, BC_V, 2), (w, BC_W, 2),
                                   (a, BC_A, 1), (pd0, BC_PD, 4)], 'ag1')

        # ---- early predictor matmuls: issued BEFORE the unpack so they
        # carry no dependency on the exchange (they read the pre-exchange
        # tiles; the ghost columns of the parked results are patched by
        # tiny strip matmuls over the unpack staging after the AG lands).
        xc = u.xc
        us, vs, ws = self.fld('us', 0), self.fld('vs', 0), self.fld('ws', 0)
        tyu = self.fld('tyu', 0)
        tyv = self.fld('tyv', 0)
        tyw = self.fld('tyw', 0)
        for f, dst, ty in ((u, us, tyu), (v, vs, tyv), (w, ws, tyw)):
            nm = 'u' if f is u else ('v' if f is v else 'w')
            ps = self.mm('MD_' + nm, f, 128)
            nc.scalar.copy(self.T(dst), ps[:, xc: f.F - xc])
            ps2 = self.mm('DY_' + nm, f, 128)
            nc.scalar.copy(self.T(ty), ps2[:, xc: f.F - xc])

        self.exchange_end(ag1)

        # ---- ghost-column patch: y-matmuls of the received halo strips
        # (stored contiguously in the unpack staging tiles)
        for side, stg, c0 in ((0, self.uL_t, 2), (1, self.uR_t, 68)):
            for fi, (dst, ty, nm) in enumerate(((us, tyu, 'u'), (vs, tyv, 'v'), (ws, tyw, 'w'))):
                strip = stg[0:128, fi * 68:(fi + 1) * 68]
                for mat, tgt in (('MD_' + nm, dst), ('DY_' + nm, ty)):
                    ps = self.psum_pool.tile([128, 68], F32, tag="psA",
                                             name=f"ps_st_{nc.next_id()}")
                    nc.tensor.matmul(ps[:, 0:68], self.mat(mat), strip, start=True, stop=True)
                    nc.scalar.copy(
                        self.D3(tgt)[:, :, c0:c0 + 2],
                        ps[:, 0:68].rearrange("p (z w) -> p z w", w=2))

        # ---- rho chain in fp32 (1/rho ~ 1e-3 would lose precision in
        # fp16 scaling products); issued after the exchange packs.
        rho = self.fld('rho', 0, dt=F32)
        buoy = self.fld('buoy', 0)
        nc.scalar.copy(self.T(rho), self.T(a))      # fp16 -> fp32
        nc.vector.tensor_scalar(self.T(rho), self.T(rho), 0.05, 1.0, OP.max, OP.min)
        nc.vector.tensor_scalar(self.T(rho), self.T(rho), RHO_L - RHO_G, RHO_G, OP.mult, OP.add)
        rinv = self.fld('rinv', 0, dt=F32)
        nc.vector.reciprocal_approx_fast(self.T(rinv), self.T(rho))
        nc.vector.tensor_scalar(self.T(buoy), self.T(rinv), -DT * G_Z * RHO_L, DT * G_Z, OP.mult, OP.add)
        rP1 = Fld(rho.t, 0)   # rho dead after rinv
        nc.scalar.mul(self.T(rP1), self.T(rinv), DT * C['axp'])

        # combined advection+diffusion x/z multipliers (shared by u,v,w):
        #   f(+1)*axp_ + f(-1)*axm_ = DT*wd_xp*(f+1 + f-1) - DT*axp*adv*(f+1 - f-1)
        axp_ = self.fld('axp_', 0)
        axm_ = self.fld('axm_', 0)
        wtp_ = self.fld('wtp_', 0)
        wtm_ = self.fld('wtm_', 0)
        vt2 = self.fld('vt2', 0)
        nc.vector.tensor_scalar(self.T(axp_), self.T(u), -DT * C['axp'], DT * C['wd_xp'], OP.mult, OP.add)
        nc.vector.tensor_scalar(self.T(axm_), self.T(u), DT * C['axp'], DT * C['wd_xm'], OP.mult, OP.add)
        nc.vector.tensor_scalar(self.T(wtp_), self.T(w), -DT * C['azp'], DT * C['wd_zp'], OP.mult, OP.add)
        nc.vector.tensor_scalar(self.T(wtm_), self.T(w), DT * C['azp'], DT * C['wd_zm'], OP.mult, OP.add)
        nc.vector.tensor_scalar_mul(self.T(vt2), self.T(v), -DT)

        sx, tx, gz = self.sx0, self.tx0, self.gz0
        for fi, (f, bc, dst, ty, extra) in enumerate(
                ((u, BC_U, us, tyu, None), (v, BC_V, vs, tyv, None),
                 (w, BC_W, ws, tyw, buoy))):
            # alternate the Pool z-term scratch so field i+1's gpsimd work
            # doesn't stall on field i's DVE reads
            zsx = (self.sx0, self.tx0)[fi % 2]
            # gpsimd: z diffusion+advection terms
            nc.gpsimd.tensor_mul(self.T(zsx), self.T(f, xc), self.T(wtp_))
            nc.gpsimd.tensor_mul(self.T(gz), self.T(f, -xc), self.T(wtm_))
            nc.gpsimd.tensor_add(self.T(zsx), self.T(zsx), self.T(gz))
            # vector: y advection, then combined x terms
            nc.vector.tensor_mul(self.T(ty), self.T(ty), self.T(vt2))
            nc.vector.tensor_add(self.T(dst), self.T(dst), self.T(ty))
            nc.vector.tensor_mul(self.T(ty), self.T(f, 1), self.T(axp_))
            nc.vector.tensor_add(self.T(dst), self.T(dst), self.T(ty))
            nc.vector.tensor_mul(self.T(ty), self.T(f, -1), self.T(axm_))
            nc.vector.tensor_add(self.T(dst), self.T(dst), self.T(ty))
            nc.vector.tensor_add(self.T(dst), self.T(dst), self.T(zsx))
            if extra is not None:
                nc.vector.tensor_add(self.T(dst), self.T(dst), self.T(extra))
        self.dbg_dump('us', us)
        self.dbg_dump('vs', vs)
        self.dbg_dump('ws', ws)
        if self.stage == 'pred':
            self._store_fld(us, 0)
            self._store_fld(vs, 1)
            self._store_fld(ws, 2)
            self._store_fld(ws, 3)
            return

        # ---- divergence -> b -> bA = b/wA_xp  (stars valid through ring 1
        # on interior cores; edge cores get BC ring-1 values via edge_fix)
        self.prep_z(ws, BC_W)
        for f, bc in ((us, BC_U), (vs, BC_V), (ws, BC_W)):
            self.edge_fix(f, bc)
        b = Fld(buoy.t, 0)  # buoy dead after ws
        cb = -(DX * DX / DT)
        cbx = cb * C['axp']
        cbz = cb * C['azp']
        ps = self.mm('DY_v', vs, 128)
        nc.scalar.mul(self.T(b), ps[:, xc: vs.F - xc], cb)
        nc.gpsimd.tensor_sub(self.T(sx), self.T(ws, xc), self.T(ws, -xc))
        nc.vector.tensor_sub(self.T(tx), self.T(us, 1), self.T(us, -1))
        nc.vector.tensor_scalar_mul(self.T(tx), self.T(tx), cbx)
        nc.vector.tensor_add(self.T(b), self.T(b), self.T(tx))
        nc.vector.tensor_scalar_mul(self.T(gz), self.T(sx), cbz)
        nc.vector.tensor_add(self.T(b), self.T(b), self.T(gz))
        self.exchange([(b, BC_PD, 3)], 'b', fix=False)
        # scaled-basis RHS (valid on ghost cols too after the exchange)
        bA = b
        nc.vector.tensor_scalar_mul(self.V(bA), self.V(b), 1.0 / C['wA_xp'])
        self.dbg_dump('b', b)
        if self.stage == 'div':
            self._store_fld(us, 0)
            self._store_fld(vs, 1)
            self._store_fld(ws, 2)
            self._store_fld(b, 3)
            return

        # ---- multigrid: 2 V-cycles
        # coarse-level tiles live inside v/w (dead after the predictor)
        r1 = Fld(v.t[0:64, 0:648], 1)
        wp1_ = Fld(v.t[0:64, 648:1296], 1)
        wsm1_ = Fld(v.t[0:64, 1296:1944], 1)
        r2l = v.t[0:32, 1944:2072]
        self.w2win = Fld(v.t[0:16, 2072:2432], 1)
        r2 = Fld(w.t[0:16, 0:2340], 2)
        pdA = Fld(u.t, 0)     # u dead after predictor
        pdB = Fld(wtp_.t, 0)  # wtp_ dead after predictor
        pdC = Fld(wtm_.t, 0)  # wtm_ dead after predictor
        r0 = Fld(vt2.t, 0)    # vt2 dead after predictor
        r3 = self.fld('r3', 3)
        r4 = self.fld('r4', 4)
        r5 = self.fld('r5', 5)
        r6 = self.fld('r6', 6)
        wts = {l: (self.fld(f'wp{l}', l), self.fld(f'wsm{l}', l)) for l in range(2, 6)}
        wts[1] = (wp1_, wsm1_)
        w6 = self.fld('w6', 6)

        pd_cur = pd0
        rot = [pdB, pdC, pdA]
        ri = 0
        for vc in range(2):
            # pre-smooth (vc0: pd0 exchanged in the merged AG up front;
            # vc1: exchange here)
            if vc > 0:
                self.prep_z(pd_cur, BC_PD)
                self.exchange([(pd_cur, BC_PD, 4)], 'pd2')
            if self.stage == 'exch1' and vc == 0:
                for ch in range(4):
                    self._store_fld(pd_cur, ch)
                return
            pd1 = rot[ri % 3]; ri += 1
            self.jacobi(pd1, pd_cur, bA, 0)
            self.edge_fix(pd1, BC_PD)
            if self.stage == 'jac1' and vc == 0:
                for ch in range(4):
                    self._store_fld(pd1, ch)
                return
            # residual: pd1 ghost ring-1 is valid from the extended pre-smooth
            self.prep_z(pd1, BC_PD)
            self.residual(r0, pd1, bA)
            if self.stage == 'resid' and vc == 0:
                for ch in range(4):
                    self._store_fld(pd1, ch)
                return
            # restrict chain
            self.restrict(r0, r1, 0)
            # r1 -> r2 local window, then allgather full r2
            self._restrict_r1_to_r2(r1, r2l, r2)
            self.restrict(r2, r3, 2)
            self.restrict(r3, r4, 3)
            self.restrict(r4, r5, 4)
            self.restrict(r5, r6, 5)
            # coarsest (r6 is scaled by 1/wA_xp, w6 is true-basis):
            # w6 = rb * r6_true = rb * wA_xp * r6' = -cs * r6'
            nc.scalar.mul(self.V(w6), self.V(r6), -C['cs'])
            self.prep_z(w6, BC_PD)
            self.prep_x_bc(w6, BC_PD)
            # up-leg 5..1
            wc = w6
            for l in range(5, 0, -1):
                wp, wsm = wts[l]
                self.prolong_copy(wc, wp, l)
                self.jacobi(wsm, wp, (r1 if l == 1 else (r2, r3, r4, r5)[l - 2]), l)
                self.prep_z(wsm, BC_PD)
                if l >= 2:
                    self.prep_x_bc(wsm, BC_PD)
                wc = wsm
            # correction + post-smooth
            pd2 = rot[ri % 3]; ri += 1
            self.prolong_sub(wc, pd1, pd2, 0)
            if self.stage == 'corr' and vc == 0:
                for ch in range(4):
                    self._store_fld(pd2, ch)
                return
            pd3 = rot[ri % 3]; ri += 1
            self.jacobi(pd3, pd2, bA, 0)
            self.edge_fix(pd3, BC_PD)
            pd_cur = pd3
            self.dbg_dump(f'pd_vc{vc}', pd3)
            if self.stage == 'vc1' and vc == 0:
                self._store_fld(us, 0)
                self._store_fld(vs, 1)
                self._store_fld(ws, 2)
                self._store_fld(pd_cur, 3)
                return

        # ---- projection in fp32-mixed: pd ghost ring-1 valid
        self.prep_z(pd_cur, BC_PD)
        rP2 = rinv
        s32 = self.f32s
        ps = self.mm('DY_pd', pd_cur, 128)
        nc.vector.tensor_scalar_mul(self.T(s32), ps[:, xc: pd_cur.F - xc], DT)
        nc.vector.tensor_mul(self.T(s32), self.T(s32), self.T(rP2))
        nc.vector.tensor_sub(self.T(vs), self.T(vs), self.T(s32))
        nc.vector.tensor_sub(self.T(tx), self.T(pd_cur, 1), self.T(pd_cur, -1))
        nc.vector.tensor_mul(self.T(s32), self.T(tx), self.T(rP1))
        nc.vector.tensor_sub(self.T(us), self.T(us), self.T(s32))
        nc.gpsimd.tensor_sub(self.T(gz), self.T(pd_cur, xc), self.T(pd_cur, -xc))
        nc.vector.tensor_mul(self.T(s32), self.T(gz), self.T(rP1))
        nc.vector.tensor_sub(self.T(ws), self.T(ws), self.T(s32))

        # ---- store (contiguous, spread across engine DMA queues)
        self._store_fld(us, 0)
        self._store_fld(vs, 1, eng=nc.scalar)
        self._store_fld(ws, 2, eng=nc.gpsimd)
        self._store_fld(pd_cur, 3)

    def _restrict_r1_to_r2(self, r1, r2l, r2):
        """restrict sharded r1 -> local coarse window, allgather -> full r2."""
        nc, C = self.nc, self.C
        g = GEOM[1]
        P, zr, xc = g[0], g[2], g[3]
        F = zr * xc
        ps = self.mm('R1', r1, P // 2)
        t0 = self.gz0.t[0:P // 2, 0:F]
        nc.scalar.copy(t0, ps[:, 0:F])
        t1 = self.sx0.t[0:P // 2, 0:F]
        t2 = self.tx0.t[0:P // 2, 0:F]
        nc.vector.tensor_add(t1[:, 0:F - 1], t0[:, 0:F - 1], t0[:, 1:F])
        nc.vector.tensor_add(t2[:, 0:F - xc - 1], t1[:, 0:F - xc - 1], t1[:, xc:F - 1])
        t23 = t2[:, 0:F].rearrange("p (z x) -> p z x", x=xc)
        # compact local window [32p, 8z * 16x]
        r2l3 = r2l[:, 0:128].rearrange("p (z x) -> p z x", x=16)
        nc.vector.tensor_scalar_mul(r2l3[:, :, :], t23[:, 1:17:2, 2:34:2], 2.0 * C['wres'])
        agin = self.dram.tile([1, 32 * 128], F16, tag='agr2i', name=f'agr2i{nc.next_id()}')
        agout = self.dram.tile([NC_, 2, 16, 8, 16], F16, tag='agr2o', name=f'agr2o{nc.next_id()}',
                               addr_space="Shared")
        nc.sync.dma_start(agin[0:1, :], r2l[:, 0:128])
        nc.gpsimd.collective_compute(
            "AllGather", OP.bypass, replica_groups=[list(range(NC_))],
            ins=[agin.opt()], outs=[agout.opt()])
        d3 = self.D3(r2)
        engs = (nc.sync, nc.scalar, nc.gpsimd)
        for s in range(NC_):
            for zh in (0, 1):
                engs[(2 * s + zh) % 3].dma_start(
                    d3[:, 1 + zh * 8:9 + zh * 8, 1 + s * 16:17 + s * 16],
                    agout[s, zh, :, :, :])


# ---------------------------------------------------------------- entry
_CACHE = {}


def _get_nc(key, C, dbg_name=None, stage='full', dbg_init=False):
    ck = (key, dbg_name, stage, dbg_init)
    if ck not in _CACHE:
        mats_np, cols = build_mats(C)
        b = B(C, mats_np, cols, dbg_name=dbg_name, stage=stage, dbg_init=dbg_init)
        nc = b.build()
        _CACHE[ck] = (nc, mats_np)
    return _CACHE[ck]


def _pad_field(arr):
    """[64z, 64y, 64x] -> padded device tile [128, 34*72] (fp16)."""
    t = np.zeros((128, 34, 72), np.float16)
    # p = zh*64 + y ; row z' = 1..32 ; col 4..67
    a = arr.reshape(2, 32, 64, 64).transpose(0, 2, 1, 3).reshape(128, 32, 64)
    t[:, 1:33, 4:68] = a
    return t.reshape(128, 34 * 72)


def _make_in_maps(fields, mats_np):
    in_maps = []
    for r in range(NC_):
        m = {}
        for nm, arr in fields.items():
            loc = np.asarray(arr, np.float32)[0, 0, :, :, r * XL:(r + 1) * XL]
            m[nm] = _pad_field(loc)
        m['mats'] = mats_np
        msk = np.zeros((128, 4), np.float32)
        msk[:, 0] = 1.0 if r == 0 else 0.0       # mL
        msk[:, 1] = 0.0 if r == 0 else 1.0       # nmL
        msk[:, 2] = 1.0 if r == NC_ - 1 else 0.0  # mR
        msk[:, 3] = 0.0 if r == NC_ - 1 else 1.0  # nmR
        m['masks'] = msk
        ho = np.zeros((1, 3), np.int32)
        rl = max(r - 1, 0)
        rr = min(r + 1, NC_ - 1)
        ho[0, 0] = rl * 2 + 1   # left ghost <- left nbr's right-edge slot
        ho[0, 1] = rr * 2 + 0   # right ghost <- right nbr's left-edge slot
        ho[0, 2] = r * 16
        m['hoffs'] = ho
        in_maps.append(m)
    return in_maps


def kernel(alpha, values_u, values_v, values_w, values_pd,
           w_diff, w_xadv, w_yadv, w_zadv, w_A, w_res, _dbg=None, _stage='full', _dbg_init=False):
    C = extract_consts(w_diff, w_xadv, w_yadv, w_zadv, w_A, w_res)
    key = tuple(sorted(C.items()))
    nc, mats_np = _get_nc(key, C, dbg_name=_dbg, stage=_stage, dbg_init=_dbg_init)
    fields = {'alpha': alpha, 'values_u': values_u, 'values_v': values_v,
              'values_w': values_w, 'values_pd': values_pd}
    in_maps = _make_in_maps(fields, mats_np)
    res = run_bass_kernel_spmd(nc, in_maps, core_ids=list(range(NC_)))
    full = np.empty((4, NZ, NY, NX), np.float32)
    for r in range(NC_):
        o = res.results[r]['out'].reshape(4, 128, 34, 72)[:, :, 1:33, 4:68].astype(np.float32)
        # [4, (zh y), z', x] -> [4, (zh z'), y, x]
        o = o.reshape(4, 2, 64, 32, 64).transpose(0, 1, 3, 2, 4).reshape(4, 64, 64, 64)
        full[:, :, :, r * XL:(r + 1) * XL] = o
    if _dbg is not None:
        kernel._dbg_res = [res.results[r].get('dbg') for r in range(NC_)]
    return full[None]  # (1, 4, 64, 64, 512)


# revision 14
# speedup vs baseline: 2.6829x; 1.0457x over previous
"""Trainium2 Bass kernel for the multiphase CFD fractional-step solver
(predictor + divergence + 2 multigrid V-cycles + projection) on a
64x64x512 grid, sharded along x across 8 NeuronCores.

Self-contained: hardcodes shapes/sharding; reads stencil coefficient
VALUES from the runtime weight inputs and compiles a specialized graph
(cached per coefficient set).

Device layout (level l):
  partitions p = zh*ny + y   (zh in {0,1} z-halves; l=6 has P=1)
  free       j = z'*xc + x   (z' in [0, zr): rows 0 and zr-1 are z-ghosts;
                              x in [0, xc): cols 0 and xc-1 are x-ghosts)
Volume passes run on the row-trimmed flat range [xc, F-xc) so all
+-1 / +-xc shifted reads stay inside the [P, F] tile.
y-axis stencil taps (partition axis) are done on the TensorEngine as
[K,M] matmuls with per-field boundary rows baked into the matrices.

Precision: fields and stencil passes run in fp16 (the DVE gets 2x/4x
element rates for packed 2-byte operands and the PE runs fp16 matmuls
at 4x the fp32 rate).  fp32 is kept where it matters: the rho/1-rho
chain (values ~1e-3 would denormal in fp16 scaling), the residual
accumulation (cancellation), and the projection corrections.  The
Jacobi/residual y-matrices are pre-scaled so the per-tap coefficient
application collapses into a single tensor_scalar, exploiting the
operator's full tap symmetry (asserted in extract_consts).

I/O: the host pre-pads each field into the device tile layout
[128, zr*xc] fp16 so every load/store is one fully-contiguous DMA.
x halos travel through AllGathers with contiguous staged pack/unpack.
One merged exchange up front carries u,v,w (wd2) + alpha (wd1) +
pd (wd4): the predictor produces stars valid through ghost ring 1 and
the post-predictor exchange disappears.  Remaining collectives:
b (wd3), replicated-r2 gather per V-cycle, pd re-exchange before
V-cycle 2.  Multigrid levels 0-1 stay sharded; levels 2-6 replicated.
"""
import sys
sys.path.insert(0, '/opt/trn_rl_repo')
import numpy as np
import concourse.bass as bass
import concourse.bacc as bacc
import concourse.mybir as mybir
from concourse.bass_utils import run_bass_kernel_spmd
from concourse.tile import TileContext

F32 = mybir.dt.float32
F16 = mybir.dt.float16
I32 = mybir.dt.int32
OP = mybir.AluOpType

DT, DX, G_Z = 0.002, 0.04, -10.0
RHO_L, RHO_G, NU = 1000.0, 1.0, 1e-3
NZ, NY, NX = 64, 64, 512
NC_ = 8
XL = NX // NC_  # 64 local x

# level: (P, ny, zr, xc, sharded, gw) -- gw = x-ghost cols per side
GEOM = {
    0: (128, 64, 34, 72, True, 4),
    1: (64, 32, 18, 36, True, 2),
    2: (16, 16, 18, 130, False, 1),
    3: (8, 8, 10, 66, False, 1),
    4: (4, 4, 6, 34, False, 1),
    5: (2, 2, 4, 18, False, 1),
    6: (1, 1, 3, 10, False, 1),
}
# BC per field: axis -> (lo, hi), 'n' neumann (ghost=adjacent), 'd' dirichlet (ghost=0)
BC_U = {'z': ('n', 'n'), 'y': ('n', 'n'), 'x': ('d', 'd')}
BC_V = {'z': ('n', 'n'), 'y': ('d', 'd'), 'x': ('n', 'n')}
BC_W = {'z': ('d', 'd'), 'y': ('n', 'n'), 'x': ('n', 'n')}
BC_PD = {'z': ('n', 'd'), 'y': ('n', 'n'), 'x': ('n', 'n')}
BC_A = {'z': ('n', 'n'), 'y': ('n', 'n'), 'x': ('n', 'n')}


# ---------------------------------------------------------------- host-side
def _yblock(ny, cm, cc, cp, bc):
    """[ny, ny] matrix M with out[y] = sum_k M[k, y] in[k]:
    tridiag with sub=cm (coeff of in[y-1]), diag=cc, super=cp (in[y+1]),
    Neumann BC folds the ghost coeff into the boundary diagonal."""
    m = np.zeros((ny, ny), np.float32)
    for y in range(ny):
        m[y, y] += cc
        if y > 0:
            m[y - 1, y] += cm
        elif bc[0] == 'n':
            m[y, y] += cm
        if y < ny - 1:
            m[y + 1, y] += cp
        elif bc[1] == 'n':
            m[y, y] += cp
    return m


def _blkdiag2(b):
    n = b.shape[0]
    m = np.zeros((2 * n, 2 * b.shape[1]), np.float32)
    m[:n, :b.shape[1]] = b
    m[n:, b.shape[1]:] = b
    return m


def _halve(ny):
    m = np.zeros((ny, ny // 2), np.float32)
    for y in range(ny):
        m[y, y // 2] = 0.5
    return m


def _double(nyc, nyf):
    m = np.zeros((nyc, nyf), np.float32)
    for y in range(nyf):
        m[y // 2, y] = 1.0
    return m


def build_mats(C):
    """Concatenated [128, sum M] lhsT matrices (fp16) + column offset map."""
    cols = {}
    parts = []
    total = 0

    def add(name, m, K):
        nonlocal total
        assert m.shape[0] == K and K <= 128 and m.shape[1] <= 128
        buf = np.zeros((128, m.shape[1]), np.float32)
        buf[:K] = m
        cols[name] = (total, m.shape[1], K)
        parts.append(buf)
        total += m.shape[1]

    # predictor diffusion y-taps + center (K=M=128, blockdiag over zh)
    for nm, bc in (('u', BC_U), ('v', BC_V), ('w', BC_W)):
        b = _yblock(64, DT * C['wd_ym'], 1.0 + DT * C['wd_c'], DT * C['wd_yp'], bc['y'])
        add('MD_' + nm, _blkdiag2(b), 128)
    # advection / gradient y-difference (raw tap values)
    for nm, bc in (('u', BC_U), ('v', BC_V), ('w', BC_W), ('pd', BC_PD)):
        b = _yblock(64, C['aym'], 0.0, C['ayp'], bc['y'])
        add('DY_' + nm, _blkdiag2(b), 128)
    # residual y-taps + center at L0, pre-divided by wA_xp so the residual
    # is accumulated in the r' = r/wA_xp basis
    b = _yblock(64, 1.0, C['wA_c'] / C['wA_xp'], 1.0, BC_PD['y'])
    add('AY0', _blkdiag2(b), 128)
    # jacobi y matrices per level 0..5, unit taps: the common factor cs
    # (= -wA_xp/diag) is applied once in a final tensor_scalar
    for l in range(6):
        P, ny = GEOM[l][0], GEOM[l][1]
        b = _yblock(ny, 1.0, 0.0, 1.0, BC_PD['y'])
        add(f'JY{l}', _blkdiag2(b) if l <= 1 else b, P)
    # identity (for PE-accumulated x/z shift taps in the residual)
    add('I0', np.eye(128, dtype=np.float32), 128)
    # restrict y-halving matrices (R{l}: level l -> l+1)
    add('R0', _blkdiag2(_halve(64)), 128)
    add('R1', _blkdiag2(_halve(32)), 64)   # stays (zh,y); zh dissolved in AG unpack
    for l in range(2, 6):
        add(f'R{l}', _halve(GEOM[l][1]), GEOM[l][0])
    # prolong y-doubling: PR{l} maps level l+1 -> l
    add('PR0', _blkdiag2(_double(32, 64)), 64)
    m = _double(16, 32)
    add('PR1', np.concatenate([m, m], axis=1), 16)  # s=1 coarse -> (zh,y) fine
    for l in range(2, 6):
        add(f'PR{l}', _double(GEOM[l + 1][1], GEOM[l][1]), GEOM[l + 1][0])

    return np.concatenate(parts, axis=1).astype(np.float16), cols


def extract_consts(w_diff, w_xadv, w_yadv, w_zadv, w_A, w_res):
    g = lambda a, i, j, k: float(np.asarray(a)[0, 0, i, j, k])
    C = {}
    C['wd_c'] = g(w_diff, 1, 1, 1)
    C['wd_zm'], C['wd_zp'] = g(w_diff, 0, 1, 1), g(w_diff, 2, 1, 1)
    C['wd_ym'], C['wd_yp'] = g(w_diff, 1, 0, 1), g(w_diff, 1, 2, 1)
    C['wd_xm'], C['wd_xp'] = g(w_diff, 1, 1, 0), g(w_diff, 1, 1, 2)
    C['wA_c'] = g(w_A, 1, 1, 1)
    C['wA_zm'], C['wA_zp'] = g(w_A, 0, 1, 1), g(w_A, 2, 1, 1)
    C['wA_ym'], C['wA_yp'] = g(w_A, 1, 0, 1), g(w_A, 1, 2, 1)
    C['wA_xm'], C['wA_xp'] = g(w_A, 1, 1, 0), g(w_A, 1, 1, 2)
    C['axp'], C['axm'] = g(w_xadv, 1, 1, 2), g(w_xadv, 1, 1, 0)
    C['ayp'], C['aym'] = g(w_yadv, 1, 2, 1), g(w_yadv, 1, 0, 1)
    C['azp'], C['azm'] = g(w_zadv, 2, 1, 1), g(w_zadv, 0, 1, 1)
    wr = np.asarray(w_res).ravel()
    assert np.allclose(wr, wr[0]), "nonuniform w_res unsupported"
    C['wres'] = float(wr[0])
    # fast paths used by the kernel
    assert abs(C['axm'] + C['axp']) < 1e-12 * max(1, abs(C['axp']))
    assert abs(C['azm'] + C['azp']) < 1e-12 * max(1, abs(C['azp']))
    # x/z/y diffusion tap symmetry
    assert abs(C['wd_zm'] - C['wd_zp']) < 1e-12 * max(1, abs(C['wd_zp']))
    assert abs(C['wd_xm'] - C['wd_xp']) < 1e-12 * max(1, abs(C['wd_xp']))
    # A-operator full tap symmetry (lets the jacobi/residual scale fold
    # into a single constant cs)
    for k in ('wA_zm', 'wA_zp', 'wA_ym', 'wA_yp', 'wA_xm'):
        assert abs(C[k] - C['wA_xp']) < 1e-12 * max(1, abs(C['wA_xp'])), k
    diag = C['wA_c']
    C['diag'] = diag
    C['jxp'] = -C['wA_xp'] / diag
    C['cs'] = C['jxp']
    C['rb'] = 1.0 / diag
    return C


# ---------------------------------------------------------------- builder
class Fld:
    def __init__(self, t, lvl):
        self.t, self.lvl = t, lvl
        P, ny, zr, xc, _, gw = GEOM[lvl]
        self.P, self.zr, self.xc, self.F, self.gw = P, zr, xc, zr * xc, gw


class B:
    """Builder context."""

    def __init__(self, C, mats_np, mat_cols, dbg_name=None, stage='full', dbg_init=False):
        self.C = C
        self.stage = stage
        self.dbg_init = dbg_init
        self.dbg_name = dbg_name
        self.nc = bacc.Bacc()
        nc = self.nc
        self.mat_cols = mat_cols
        self.MC = mats_np.shape[1]
        # params (fields are pre-padded on host into the device tile layout)
        self.p_in = {}
        for nm in ('alpha', 'values_u', 'values_v', 'values_w', 'values_pd'):
            self.p_in[nm] = nc.declare_dram_parameter(nm, [128, GEOM[0][2] * GEOM[0][3]], F16, isOutput=False)
        self.p_mats = nc.declare_dram_parameter('mats', [128, self.MC], F16, isOutput=False)
        self.p_masks = nc.declare_dram_parameter('masks', [128, 4], F32, isOutput=False)
        self.p_hoffs = nc.declare_dram_parameter('hoffs', [1, 3], I32, isOutput=False)
        self.p_out = nc.declare_dram_parameter('out', [4, 128, GEOM[0][2] * GEOM[0][3]], F16, isOutput=True)
        if dbg_name:
            self.p_dbg = nc.declare_dram_parameter('dbg', [128, GEOM[0][2] * GEOM[0][3]], F16, isOutput=True)
        self.dbg_written = False

    # --- tile helpers -----------------------------------------------------
    def fld(self, name, lvl, tag=None, dt=F16):
        g = GEOM[lvl]
        t = self.pool.tile([g[0], g[2] * g[3]], dt, tag=(tag or name), name=name)
        if self.dbg_init:
            self.nc.vector.memset(t[:, :], 0.0)
        return Fld(t, lvl)

    def sub(self, f, lvl):
        g = GEOM[lvl]
        return Fld(f.t[0:g[0], 0:g[2] * g[3]], lvl)

    def T(self, f, s=0):
        """row-trimmed shifted flat view [P, F-2*xc]"""
        return f.t[:, f.xc + s: f.F - f.xc + s]

    def V(self, f):
        return f.t[:, 0:f.F]

    def D3(self, f):
        return f.t[:, 0:f.F].rearrange("p (z x) -> p z x", x=f.xc)

    def mat(self, name):
        off, M, K = self.mat_cols[name]
        return self.mats_t[0:K, off:off + M]

    def mm(self, name, rhs_f, Pout, psum_w=None):
        """psum[Pout, F] = mats[name].T @ V(rhs)  (chunked, full width)"""
        nc = self.nc
        F = psum_w or rhs_f.F
        ps = self.psum_pool.tile([Pout, F], F32, tag="psA", name=f"ps_{name}_{nc.next_id()}")
        rhs = rhs_f.t[:, 0:F]
        lhsT = self.mat(name)
        for c0 in range(0, F, 512):
            w = min(512, F - c0)
            nc.tensor.matmul(ps[:, c0:c0 + w], lhsT, rhs[:, c0:c0 + w], start=True, stop=True)
        return ps

    # --- ghost prep -------------------------------------------------------
    def prep_z(self, f, bc):
        """fill z ghost rows: global BC rows (+ inter-half swap on levels 0-1)"""
        nc, d3 = self.nc, self.D3(f)
        P, zr = f.P, f.zr
        split = f.lvl <= 1
        lo = slice(0, P // 2) if split else slice(0, P)
        hi = slice(P // 2, P) if split else slice(0, P)
        if bc['z'][0] == 'n':
            nc.scalar.copy(d3[lo, 0, :], d3[lo, 1, :])
        else:
            nc.gpsimd.memset(d3[lo, 0, :], 0.0)
        if bc['z'][1] == 'n':
            nc.scalar.copy(d3[hi, zr - 1, :], d3[hi, zr - 2, :])
        else:
            nc.gpsimd.memset(d3[hi, zr - 1, :], 0.0)
        if split:
            nc.sync.dma_start(d3[lo, zr - 1, :], d3[hi, 1, :])
            nc.sync.dma_start(d3[hi, 0, :], d3[lo, zr - 2, :])

    def prep_x_bc(self, f, bc):
        """replicated levels: plain BC on both x faces"""
        nc, d3 = self.nc, self.D3(f)
        xc = f.xc
        if bc['x'][0] == 'n':
            nc.scalar.copy(d3[:, :, 0], d3[:, :, 1])
        else:
            nc.gpsimd.memset(d3[:, :, 0], 0.0)
        if bc['x'][1] == 'n':
            nc.scalar.copy(d3[:, :, xc - 1], d3[:, :, xc - 2])
        else:
            nc.gpsimd.memset(d3[:, :, xc - 1], 0.0)

    def edge_fix(self, f, bc):
        """overwrite ring-1 ghost cols on the 2 edge cores by BC, via
        per-core mask inputs (mL,nmL,mR,nmR)."""
        nc, d3 = self.nc, self.D3(f)
        P, zr, xc, gw = f.P, f.zr, f.xc, f.gw
        mL, nmL = self.masks_t[0:P, 0:1], self.masks_t[0:P, 1:2]
        mR, nmR = self.masks_t[0:P, 2:3], self.masks_t[0:P, 3:4]
        for (lo, side, m, nm) in ((True, gw - 1, mL, nmL), (False, xc - gw, mR, nmR)):
            gcol = d3[:, :, side]
            if bc['x'][0 if lo else 1] == 'd':
                nc.vector.tensor_scalar_mul(gcol, gcol, nm)
            else:
                icol = d3[:, :, gw if lo else xc - gw - 1]
                tmp = self.ebc_t[0:P, 0:zr]
                nc.vector.tensor_scalar_mul(tmp, icol, m)
                nc.vector.scalar_tensor_tensor(gcol, gcol, nm, tmp, OP.mult, OP.add)

    def exchange_begin(self, fields_bcs, fam):
        """Pack + allgather trigger half of the staged halo exchange.
        fields_bcs: list of (Fld, bc, wd).  Returns state for exchange_end.
        side 0 = left-edge interior (becomes left nbr's right ghost),
        side 1 = right-edge interior (becomes right nbr's left ghost)."""
        nc = self.nc
        f0 = fields_bcs[0][0]
        P = f0.P
        offs, W = [], 0
        for (f, bc, wd) in fields_bcs:
            offs.append(W)
            W += f.zr * wd
        pk = self.pk_t[0:P, 0:2 * W]
        for (f, bc, wd), off in zip(fields_bcs, offs):
            d3 = self.D3(f)
            gw, xc = f.gw, f.xc
            for s, c0 in ((0, gw), (1, xc - gw - wd)):
                dst = pk[:, s * W + off: s * W + off + f.zr * wd].rearrange(
                    "p (z w) -> p z w", w=wd)
                nc.scalar.copy(dst, d3[:, :, c0:c0 + wd])
        agin = self.dram.tile([2, P, W], F16, tag=f'agin_{fam}', name=f'agin{nc.next_id()}')
        agout = self.dram.tile([NC_ * 2, P, W], F16, tag=f'agout_{fam}',
                               name=f'agout{nc.next_id()}', addr_space="Shared")
        nc.sync.dma_start(agin[:, :, :].transpose([1, 0, 2]),
                          pk[:, :].rearrange("p (s w) -> p s w", s=2))
        nc.gpsimd.collective_compute(
            "AllGather", OP.bypass, replica_groups=[list(range(NC_))],
            ins=[agin.opt()], outs=[agout.opt()])
        return (fields_bcs, offs, W, agout)

    def exchange_end(self, st, fix=True):
        """Unpack half: contiguous DMAs of the two neighbor slots + engine
        copies into ghost columns + edge BC fix."""
        nc = self.nc
        fields_bcs, offs, W, agout = st
        P = fields_bcs[0][0].P
        uL = self.uL_t[0:P, 0:W]
        uR = self.uR_t[0:P, 0:W]
        nc.sync.dma_start(uL[:, :], agout[bass.ds(self.regL, 1), :, :])
        nc.sync.dma_start(uR[:, :], agout[bass.ds(self.regR, 1), :, :])
        for (f, bc, wd), off in zip(fields_bcs, offs):
            d3 = self.D3(f)
            gw, xc = f.gw, f.xc
            srcL = uL[:, off:off + f.zr * wd].rearrange("p (z w) -> p z w", w=wd)
            srcR = uR[:, off:off + f.zr * wd].rearrange("p (z w) -> p z w", w=wd)
            nc.scalar.copy(d3[:, :, gw - wd:gw], srcL)
            nc.scalar.copy(d3[:, :, xc - gw:xc - gw + wd], srcR)
            if fix:
                self.edge_fix(f, bc)

    def exchange(self, fields_bcs, fam, fix=True):
        self.exchange_end(self.exchange_begin(fields_bcs, fam), fix=fix)

    # --- compute blocks ---------------------------------------------------
    def jacobi(self, dst, w_in, rr, lvl):
        """dst = cs * (x-sum + z-sum + y-sum(JY matmul) - rr), the damped
        Jacobi update in the r' = r/wA_xp scaled basis (rr = b/wA_xp at L0,
        or the scaled residual at coarse levels).  w_in ghosts valid."""
        nc, C = self.nc, self.C
        xc = w_in.xc
        ps = self.mm(f'JY{lvl}', w_in, w_in.P)
        pst = ps[:, xc: w_in.F - xc]
        gz = self.sub(self.gz0, lvl)
        s = self.sub(self.tx0, lvl)
        nc.vector.tensor_add(self.T(gz), self.T(w_in, xc), self.T(w_in, -xc))
        nc.vector.tensor_add(self.T(s), self.T(w_in, 1), self.T(w_in, -1))
        nc.vector.tensor_add(self.T(s), self.T(s), self.T(gz))
        nc.vector.tensor_sub(self.T(s), self.T(s), self.T(rr))
        nc.vector.tensor_add(self.T(s), self.T(s), pst)
        nc.vector.tensor_scalar_mul(self.T(dst), self.T(s), C['cs'])

    def residual(self, dst, pd, bA):
        """dst = (A pd - b)/wA_xp at L0.  The y-taps+center matrix and the
        four x/z shift taps (identity matmuls over shifted views) accumulate
        into one fp32 PSUM group on the PE - exact fp16-product sums, no
        cancellation loss - leaving a single DVE subtract."""
        nc, C = self.nc, self.C
        xc = pd.xc
        Ft = pd.F - 2 * xc
        ps = self.psum_pool.tile([128, Ft], F32, tag="psA", name=f"ps_res_{nc.next_id()}")
        mA, mI = self.mat('AY0'), self.mat('I0')
        for c0 in range(0, Ft, 512):
            w = min(512, Ft - c0)
            taps = ((mA, 0), (mI, 1), (mI, -1), (mI, xc), (mI, -xc))
            for i, (m, sh) in enumerate(taps):
                nc.tensor.matmul(ps[:, c0:c0 + w],
                                 m, pd.t[:, xc + c0 + sh: xc + c0 + sh + w],
                                 start=(i == 0), stop=(i == len(taps) - 1))
        nc.vector.tensor_sub(self.T(dst), ps[:, 0:Ft], self.T(bA))

    def restrict(self, r_f, r_c, lf):
        """r_c (level lf+1) interior = w_res-weighted 2x2x2 sum of r_f (level lf)."""
        nc, C = self.nc, self.C
        g = GEOM[lf]
        P, zr, xc = g[0], g[2], g[3]
        F = zr * xc
        gc = GEOM[lf + 1]
        Pc = gc[0]
        # 2x2 (x,z) pair sums BEFORE the y-halving matmul (same element
        # count - cost is free-size only - but kills the PSUM park)
        s1 = self.sx0.t[0:P, 0:F]
        s2 = Fld(self.tx0.t[0:P, 0:F], r_f.lvl)
        rt = r_f.t
        nc.vector.tensor_add(s1[:, 0:F - 1], rt[:, 0:F - 1], rt[:, 1:F])
        nc.vector.tensor_add(s2.t[:, 0:F - xc - 1], s1[:, 0:F - xc - 1], s1[:, xc:F - 1])
        ps = self.mm(f'R{lf}', s2, Pc, psum_w=F)
        # strided gather: coarse cells <- fine pair sums.  For lf==0 also
        # produce the coarse x-ghost ring-1 (computable from the extended
        # fine residual) so L1 never needs its own halo exchange.
        zi = gc[2] - 2
        gwf, gwc = GEOM[lf][5], GEOM[lf + 1][5]
        d3c = self.D3(r_c)
        t23 = ps[:, 0:F].rearrange("p (z x) -> p z x", x=xc)
        if lf == 0:
            xi = gc[3] - 2 * gwc + 2        # interior + ghost ring-1 (34)
            c0, f0 = gwc - 1, gwf - 2       # coarse col 1 <- fine cols (2,3)
        else:
            xi = gc[3] - 2 * gwc
            c0, f0 = gwc, gwf
        nc.vector.tensor_scalar_mul(
            d3c[:, 1:1 + zi, c0:c0 + xi],
            t23[:, 1:1 + 2 * zi:2, f0:f0 + 2 * xi:2],
            2.0 * C['wres'])

    def prolong_mm(self, w_c, lf):
        """y-doubling matmul of level lf+1 tile -> psum [P_lf, F_{lf+1}]"""
        return self.mm(f'PR{lf}', w_c, GEOM[lf][0])

    def parity_views(self, ps, lvl_f, dst3):
        """yield (dst_quadrant, psum_quadrant) for the 4 z/x parities."""
        gf, gc = GEOM[lvl_f], GEOM[lvl_f + 1]
        zrf, xcf = gf[2], gf[3]
        zrc, xcc = gc[2], gc[3]
        ps3 = ps[:, 0:zrc * xcc].rearrange("p (z x) -> p z x", x=xcc)
        for pz in (0, 1):
            nzf = (zrf - pz + 1) // 2
            cz = 0 if pz == 0 else 1
            for px in (0, 1):
                nxf = (xcf - px + 1) // 2
                cx = 0 if px == 0 else 1
                yield (dst3[:, pz::2, px::2], ps3[:, cz:cz + nzf, cx:cx + nxf])

    def prolong_copy(self, w_c, w_f, lf):
        """w_f = prolong(w_c) including ghosts (coarse ghosts must be valid)."""
        nc = self.nc
        if lf == 1:
            # extract this core's 20-col x window (incl. both ghost rings) of
            # the replicated L2 field via a padded DRAM bounce (dynamic
            # SBUF-side DMA offsets hang on hardware), y-double via PR1, and
            # expand with per-zh coarse row offsets.  Fine cols {2k, 2k+1}
            # map to window col k.
            d3w2 = self.D3(w_c)
            win3 = self.w2win.t[:, 0:360].rearrange("p (z x) -> p z x", x=20)
            w2d = self.dram.tile([16, 18, 132], F16, tag='w2d', name=f'w2d{nc.next_id()}')
            nc.sync.dma_start(w2d[:, :, 0:130], d3w2[:, :, :])
            nc.sync.dma_start(win3[:, :, :], w2d[:, :, bass.ds(self.reg_w2, 20)])
            ps = self.mm('PR1', Fld(self.w2win.t[:, 0:360], 1), 64, psum_w=360)
            ps3 = ps[:, 0:360].rearrange("p (z x) -> p z x", x=20)
            d3 = self.D3(w_f)
            for zh in (0, 1):
                czh = 8 * zh
                psl = slice(zh * 32, (zh + 1) * 32)
                for pz in (0, 1):
                    nzf = (18 - pz + 1) // 2
                    cz = czh + (0 if pz == 0 else 1)
                    for fx0 in (0, 1):
                        eng = nc.scalar if (pz + fx0) % 2 == 0 else nc.vector
                        if eng is nc.scalar:
                            eng.copy(d3[psl, pz::2, fx0::2],
                                     ps3[psl, cz:cz + nzf, 0:18])
                        else:
                            eng.tensor_copy(d3[psl, pz::2, fx0::2],
                                            ps3[psl, cz:cz + nzf, 0:18])
            return
        ps = self.prolong_mm(w_c, lf)
        d3 = self.D3(w_f)
        for dq, pq in self.parity_views(ps, lf, d3):
            self.nc.scalar.copy(dq, pq)

    def prolong_sub(self, w_c, pd_old, pd_new, lf):
        """pd_new = pd_old - prolong(w_c) (w_c is true-basis), covering
        interior + ghost rings 1-2.  Fine level 0 has gw=3: fine col c maps
        to coarse col (c-3)//2+1."""
        assert lf == 0
        ps = self.prolong_mm(w_c, lf)
        gf, gc = GEOM[lf], GEOM[lf + 1]
        zrf, xcf = gf[2], gf[3]
        zrc, xcc = gc[2], gc[3]
        ps3 = ps[:, 0:zrc * xcc].rearrange("p (z x) -> p z x", x=xcc)
        d3n, d3o = self.D3(pd_new), self.D3(pd_old)
        for pz in (0, 1):
            nzf = (zrf - pz + 1) // 2
            cz = 0 if pz == 0 else 1
            for fx0 in (0, 1):
                dq = d3n[:, pz::2, fx0::2]
                oq = d3o[:, pz::2, fx0::2]
                pq = ps3[:, cz:cz + nzf, 0:36]
                self.nc.vector.scalar_tensor_tensor(
                    dq, pq, -1.0, oq, OP.mult, OP.add)

    def dbg_dump(self, name, f):
        if self.dbg_name == name and not self.dbg_written:
            self.nc.sync.dma_start(self.p_dbg[0:f.P, 0:f.F], self.V(f))
            self.dbg_written = True

    # --- main build -------------------------------------------------------
    def build(self):
        nc, C = self.nc, self.C
        with TileContext(nc) as tc:
            with tc.tile_pool(name="main", bufs=1) as pool, \
                 tc.tile_pool(name="psum", bufs=1, space="PSUM") as psum_pool, \
                 tc.tile_pool(name="dram", bufs=1, space="DRAM") as dram:
                self.pool, self.psum_pool, self.dram = pool, psum_pool, dram
                self._build_body(tc)
        nc.finalize()
        return nc

    def _load_fld(self, pname, name, lvl, tag=None, eng=None):
        f = self.fld(name, lvl, tag=tag)
        (eng or self.nc.sync).dma_start(self.V(f), self.p_in[pname][:, :])
        return f

    def _store_fld(self, f, ch, eng=None):
        (eng or self.nc.sync).dma_start(self.p_out[ch, :, :], self.V(f))

    def _build_body(self, tc):
        nc, C = self.nc, self.C
        pool = self.pool
        if self.stage == 'io0':
            t = self._load_fld('values_u', 'u', 0)
            for ch in range(4):
                self._store_fld(t, ch)
            return
        # warm-up collective FIRST: the first collective pays ~50us of
        # CC-engine cold-init; start it immediately (the gather reads
        # whatever is in DRAM - nobody consumes the output).
        win = self.dram.tile([1, 8], F16, tag='warmi', name='warmi')
        wout = self.dram.tile([NC_, 8], F16, tag='warmo', name='warmo', addr_space='Shared')
        nc.gpsimd.collective_compute(
            "AllGather", OP.bypass, replica_groups=[list(range(NC_))],
            ins=[win.opt()], outs=[wout.opt()])

        # constants / matrices / masks
        self.mats_t = pool.tile([128, self.MC], F16, tag="mats", name="mats_t")
        nc.sync.dma_start(self.mats_t[:, :], self.p_mats[:, :])
        self.masks_t = pool.tile([128, 4], F32, tag="masks", name="masks_t")
        nc.sync.dma_start(self.masks_t[:, :], self.p_masks[:, :])
        hoffs_t = pool.tile([1, 3], I32, tag="hoffs", name="hoffs_t")
        nc.sync.dma_start(hoffs_t[:, :], self.p_hoffs[:, :])
        # slot index registers for halo unpack
        self.regL = nc.sync.value_load(hoffs_t[0:1, 0:1], min_val=None, max_val=None)
        self.regR = nc.sync.value_load(hoffs_t[0:1, 1:2], min_val=None, max_val=None)
        self.reg_w2 = nc.sync.value_load(hoffs_t[0:1, 2:3], min_val=None, max_val=None)

        if self.stage == 'io':
            t = self._load_fld('values_u', 'u', 0)
            for ch in range(4):
                self._store_fld(t, ch)
            return

        # ---- loads (one contiguous DMA per field)
        u = self._load_fld('values_u', 'u', 0)
        v = self._load_fld('values_v', 'v', 0, eng=nc.scalar)
        w = self._load_fld('values_w', 'w', 0, eng=nc.gpsimd)
        a = self._load_fld('alpha', 'a', 0, eng=nc.scalar)
        pd0 = self._load_fld('values_pd', 'pd0', 0, eng=nc.gpsimd)

        # scratch needed by edge_fix (used inside exchange unpack)
        self.ebc_t = pool.tile([128, 34], F16, tag='ebc', name='ebc_t')
        # shared halo-exchange staging (sized for the largest exchange: ag1
        # has W = 34*(2+2+2+1+4) = 374)
        self.pk_t = pool.tile([128, 748], F16, tag='pk', name='pk_t')
        self.uL_t = pool.tile([128, 374], F16, tag='uLs', name='uL_t')
        self.uR_t = pool.tile([128, 374], F16, tag='uRs', name='uR_t')
        # scratch sized for the largest level (L0 is 34*72=2448)
        self.gz0 = Fld(pool.tile([128, 2448], F16, tag='gz0', name='gz0'), 0)
        self.sx0 = Fld(pool.tile([128, 2448], F16, tag='sx0', name='sx0'), 0)
        self.tx0 = Fld(pool.tile([128, 2448], F16, tag='tx0', name='tx0'), 0)
        if self.dbg_init:
            for t_ in (self.gz0.t, self.sx0.t, self.tx0.t, self.ebc_t):
                self.nc.vector.memset(t_[:, :], 0.0)

        # ---- merged ghost exchange: u,v,w wd2 (stars valid to ring 1,
        # killing the post-predictor exchange), alpha wd1, pd wd4.
        for f, bc in ((u, BC_U), (v, BC_V), (w, BC_W), (pd0, BC_PD)):
            self.prep_z(f, bc)
        ag1 = self.exchange_begin([(u, BC_U, 2), (v, BC_V, 2), (w, BC_W, 2),
                                   (a, BC_A, 1), (pd0, BC_PD, 4)], 'ag1')

        # ---- early predictor matmuls: issued BEFORE the unpack so they
        # carry no dependency on the exchange (they read the pre-exchange
        # tiles; the ghost columns of the parked results are patched by
        # tiny strip matmuls over the unpack staging after the AG lands).
        xc = u.xc
        us, vs, ws = self.fld('us', 0), self.fld('vs', 0), self.fld('ws', 0)
        tyu = self.fld('tyu', 0)
        tyv = self.fld('tyv', 0)
        tyw = self.fld('tyw', 0)
        for f, dst, ty in ((u, us, tyu), (v, vs, tyv), (w, ws, tyw)):
            nm = 'u' if f is u else ('v' if f is v else 'w')
            ps = self.mm('MD_' + nm, f, 128)
            nc.scalar.copy(self.T(dst), ps[:, xc: f.F - xc])
            ps2 = self.mm('DY_' + nm, f, 128)
            nc.scalar.copy(self.T(ty), ps2[:, xc: f.F - xc])

        self.exchange_end(ag1)

        # ---- ghost-column patch: y-matmuls of the received halo strips
        # (stored contiguously in the unpack staging tiles)
        for side, stg, c0 in ((0, self.uL_t, 2), (1, self.uR_t, 68)):
            for fi, (dst, ty, nm) in enumerate(((us, tyu, 'u'), (vs, tyv, 'v'), (ws, tyw, 'w'))):
                strip = stg[0:128, fi * 68:(fi + 1) * 68]
                for mat, tgt in (('MD_' + nm, dst), ('DY_' + nm, ty)):
                    ps = self.psum_pool.tile([128, 68], F32, tag="psA",
                                             name=f"ps_st_{nc.next_id()}")
                    nc.tensor.matmul(ps[:, 0:68], self.mat(mat), strip, start=True, stop=True)
                    nc.scalar.copy(
                        self.D3(tgt)[:, :, c0:c0 + 2],
                        ps[:, 0:68].rearrange("p (z w) -> p z w", w=2))

        # ---- rho chain in fp32 (1/rho ~ 1e-3 would lose precision in
        # fp16 scaling products); issued after the exchange packs.
        rho = self.fld('rho', 0, dt=F32)
        buoy = self.fld('buoy', 0)
        nc.scalar.copy(self.T(rho), self.T(a))      # fp16 -> fp32
        nc.vector.tensor_scalar(self.T(rho), self.T(rho), 0.05, 1.0, OP.max, OP.min)
        nc.vector.tensor_scalar(self.T(rho), self.T(rho), RHO_L - RHO_G, RHO_G, OP.mult, OP.add)
        rinv = self.fld('rinv', 0, dt=F32)
        nc.vector.reciprocal_approx_fast(self.T(rinv), self.T(rho))
        nc.vector.tensor_scalar(self.T(buoy), self.T(rinv), -DT * G_Z * RHO_L, DT * G_Z, OP.mult, OP.add)
        # projection 1/rho factors, pre-scaled by 1024 to stay in fp16
        # normal range (rinv*DT ~ 2e-6 would denormal); the 1/1024 is
        # re-applied after the gradient product.
        rp1s = self.fld('rp1s', 0)
        rp2s = self.fld('rp2s', 0)
        nc.scalar.mul(self.T(rp1s), self.T(rinv), DT * C['axp'] * 1024.0)
        nc.scalar.mul(self.T(rp2s), self.T(rinv), DT * 1024.0)

        # combined advection+diffusion x/z multipliers (shared by u,v,w):
        #   f(+1)*axp_ + f(-1)*axm_ = DT*wd_xp*(f+1 + f-1) - DT*axp*adv*(f+1 - f-1)
        axp_ = self.fld('axp_', 0)
        axm_ = self.fld('axm_', 0)
        wtp_ = self.fld('wtp_', 0)
        wtm_ = self.fld('wtm_', 0)
        vt2 = self.fld('vt2', 0)
        nc.vector.tensor_scalar(self.T(axp_), self.T(u), -DT * C['axp'], DT * C['wd_xp'], OP.mult, OP.add)
        nc.vector.tensor_scalar(self.T(axm_), self.T(u), DT * C['axp'], DT * C['wd_xm'], OP.mult, OP.add)
        nc.vector.tensor_scalar(self.T(wtp_), self.T(w), -DT * C['azp'], DT * C['wd_zp'], OP.mult, OP.add)
        nc.vector.tensor_scalar(self.T(wtm_), self.T(w), DT * C['azp'], DT * C['wd_zm'], OP.mult, OP.add)
        nc.vector.tensor_scalar_mul(self.T(vt2), self.T(v), -DT)

        sx, tx, gz = self.sx0, self.tx0, self.gz0
        for fi, (f, bc, dst, ty, extra) in enumerate(
                ((u, BC_U, us, tyu, None), (v, BC_V, vs, tyv, None),
                 (w, BC_W, ws, tyw, buoy))):
            # alternate the Pool z-term scratch so field i+1's gpsimd work
            # doesn't stall on field i's DVE reads
            zsx = (self.sx0, self.tx0)[fi % 2]
            # gpsimd: z diffusion+advection terms
            nc.gpsimd.tensor_mul(self.T(zsx), self.T(f, xc), self.T(wtp_))
            nc.gpsimd.tensor_mul(self.T(gz), self.T(f, -xc), self.T(wtm_))
            nc.gpsimd.tensor_add(self.T(zsx), self.T(zsx), self.T(gz))
            # vector: y advection, then combined x terms
            nc.vector.tensor_mul(self.T(ty), self.T(ty), self.T(vt2))
            nc.vector.tensor_add(self.T(dst), self.T(dst), self.T(ty))
            nc.vector.tensor_mul(self.T(ty), self.T(f, 1), self.T(axp_))
            nc.vector.tensor_add(self.T(dst), self.T(dst), self.T(ty))
            nc.vector.tensor_mul(self.T(ty), self.T(f, -1), self.T(axm_))
            nc.vector.tensor_add(self.T(dst), self.T(dst), self.T(ty))
            nc.vector.tensor_add(self.T(dst), self.T(dst), self.T(zsx))
            if extra is not None:
                nc.vector.tensor_add(self.T(dst), self.T(dst), self.T(extra))
        self.dbg_dump('us', us)
        self.dbg_dump('vs', vs)
        self.dbg_dump('ws', ws)
        if self.stage == 'pred':
            self._store_fld(us, 0)
            self._store_fld(vs, 1)
            self._store_fld(ws, 2)
            self._store_fld(ws, 3)
            return

        # ---- divergence -> b -> bA = b/wA_xp  (stars valid through ring 1
        # on interior cores; edge cores get BC ring-1 values via edge_fix)
        self.prep_z(ws, BC_W)
        for f, bc in ((us, BC_U), (vs, BC_V), (ws, BC_W)):
            self.edge_fix(f, bc)
        b = Fld(buoy.t, 0)  # buoy dead after ws
        cb = -(DX * DX / DT)
        cbx = cb * C['axp']
        cbz = cb * C['azp']
        ps = self.mm('DY_v', vs, 128)
        nc.scalar.mul(self.T(b), ps[:, xc: vs.F - xc], cb)
        nc.gpsimd.tensor_sub(self.T(sx), self.T(ws, xc), self.T(ws, -xc))
        nc.vector.tensor_sub(self.T(tx), self.T(us, 1), self.T(us, -1))
        nc.vector.tensor_scalar_mul(self.T(tx), self.T(tx), cbx)
        nc.vector.tensor_add(self.T(b), self.T(b), self.T(tx))
        nc.vector.tensor_scalar_mul(self.T(gz), self.T(sx), cbz)
        nc.vector.tensor_add(self.T(b), self.T(b), self.T(gz))
        self.exchange([(b, BC_PD, 3)], 'b', fix=False)
        # scaled-basis RHS (valid on ghost cols too after the exchange)
        bA = b
        nc.vector.tensor_scalar_mul(self.V(bA), self.V(b), 1.0 / C['wA_xp'])
        self.dbg_dump('b', b)
        if self.stage == 'div':
            self._store_fld(us, 0)
            self._store_fld(vs, 1)
            self._store_fld(ws, 2)
            self._store_fld(b, 3)
            return

        # ---- multigrid: 2 V-cycles
        # coarse-level tiles live inside v/w (dead after the predictor)
        r1 = Fld(v.t[0:64, 0:648], 1)
        wp1_ = Fld(v.t[0:64, 648:1296], 1)
        wsm1_ = Fld(v.t[0:64, 1296:1944], 1)
        r2l = v.t[0:32, 1944:2072]
        self.w2win = Fld(v.t[0:16, 2072:2432], 1)
        r2 = Fld(w.t[0:16, 0:2340], 2)
        pdA = Fld(u.t, 0)     # u dead after predictor
        pdB = Fld(wtp_.t, 0)  # wtp_ dead after predictor
        pdC = Fld(wtm_.t, 0)  # wtm_ dead after predictor
        r0 = Fld(vt2.t, 0)    # vt2 dead after predictor
        r3 = self.fld('r3', 3)
        r4 = self.fld('r4', 4)
        r5 = self.fld('r5', 5)
        r6 = self.fld('r6', 6)
        wts = {l: (self.fld(f'wp{l}', l), self.fld(f'wsm{l}', l)) for l in range(2, 6)}
        wts[1] = (wp1_, wsm1_)
        w6 = self.fld('w6', 6)

        pd_cur = pd0
        rot = [pdB, pdC, pdA]
        ri = 0
        for vc in range(2):
            # pre-smooth (vc0: pd0 exchanged in the merged AG up front;
            # vc1: exchange here)
            if vc > 0:
                self.prep_z(pd_cur, BC_PD)
                self.exchange([(pd_cur, BC_PD, 4)], 'pd2')
            if self.stage == 'exch1' and vc == 0:
                for ch in range(4):
                    self._store_fld(pd_cur, ch)
                return
            pd1 = rot[ri % 3]; ri += 1
            self.jacobi(pd1, pd_cur, bA, 0)
            self.edge_fix(pd1, BC_PD)
            if self.stage == 'jac1' and vc == 0:
                for ch in range(4):
                    self._store_fld(pd1, ch)
                return
            # residual: pd1 ghost ring-1 is valid from the extended pre-smooth
            self.prep_z(pd1, BC_PD)
            self.residual(r0, pd1, bA)
            if self.stage == 'resid' and vc == 0:
                for ch in range(4):
                    self._store_fld(pd1, ch)
                return
            # restrict chain
            self.restrict(r0, r1, 0)
            # r1 -> r2 local window, then allgather full r2
            self._restrict_r1_to_r2(r1, r2l, r2)
            self.restrict(r2, r3, 2)
            self.restrict(r3, r4, 3)
            self.restrict(r4, r5, 4)
            self.restrict(r5, r6, 5)
            # coarsest (r6 is scaled by 1/wA_xp, w6 is true-basis):
            # w6 = rb * r6_true = rb * wA_xp * r6' = -cs * r6'
            nc.scalar.mul(self.V(w6), self.V(r6), -C['cs'])
            self.prep_z(w6, BC_PD)
            self.prep_x_bc(w6, BC_PD)
            # up-leg 5..1
            wc = w6
            for l in range(5, 0, -1):
                wp, wsm = wts[l]
                self.prolong_copy(wc, wp, l)
                self.jacobi(wsm, wp, (r1 if l == 1 else (r2, r3, r4, r5)[l - 2]), l)
                self.prep_z(wsm, BC_PD)
                if l >= 2:
                    self.prep_x_bc(wsm, BC_PD)
                wc = wsm
            # correction + post-smooth
            pd2 = rot[ri % 3]; ri += 1
            self.prolong_sub(wc, pd1, pd2, 0)
            if self.stage == 'corr' and vc == 0:
                for ch in range(4):
                    self._store_fld(pd2, ch)
                return
            pd3 = rot[ri % 3]; ri += 1
            self.jacobi(pd3, pd2, bA, 0)
            self.edge_fix(pd3, BC_PD)
            pd_cur = pd3
            self.dbg_dump(f'pd_vc{vc}', pd3)
            if self.stage == 'vc1' and vc == 0:
                self._store_fld(us, 0)
                self._store_fld(vs, 1)
                self._store_fld(ws, 2)
                self._store_fld(pd_cur, 3)
                return

        # ---- projection (fp16 with x1024 pre-scaled rho factors)
        self.prep_z(pd_cur, BC_PD)
        K1 = 1.0 / 1024.0
        ps = self.mm('DY_pd', pd_cur, 128)
        tp = self.tx0
        nc.scalar.copy(self.T(tp), ps[:, xc: pd_cur.F - xc])
        nc.vector.tensor_mul(self.T(tp), self.T(tp), self.T(rp2s))
        nc.vector.tensor_scalar_mul(self.T(tp), self.T(tp), K1)
        nc.vector.tensor_sub(self.T(vs), self.T(vs), self.T(tp))
        nc.vector.tensor_sub(self.T(tp), self.T(pd_cur, 1), self.T(pd_cur, -1))
        nc.vector.tensor_mul(self.T(tp), self.T(tp), self.T(rp1s))
        nc.vector.tensor_scalar_mul(self.T(tp), self.T(tp), K1)
        nc.vector.tensor_sub(self.T(us), self.T(us), self.T(tp))
        nc.vector.tensor_sub(self.T(gz), self.T(pd_cur, xc), self.T(pd_cur, -xc))
        nc.vector.tensor_mul(self.T(gz), self.T(gz), self.T(rp1s))
        nc.vector.tensor_scalar_mul(self.T(gz), self.T(gz), K1)
        nc.vector.tensor_sub(self.T(ws), self.T(ws), self.T(gz))

        # ---- store (contiguous, spread across engine DMA queues)
        self._store_fld(us, 0)
        self._store_fld(vs, 1, eng=nc.scalar)
        self._store_fld(ws, 2, eng=nc.gpsimd)
        self._store_fld(pd_cur, 3)

    def _restrict_r1_to_r2(self, r1, r2l, r2):
        """restrict sharded r1 -> local coarse window, allgather -> full r2."""
        nc, C = self.nc, self.C
        g = GEOM[1]
        P, zr, xc = g[0], g[2], g[3]
        F = zr * xc
        s1 = self.sx0.t[0:P, 0:F]
        s2 = Fld(self.tx0.t[0:P, 0:F], 1)
        rt = r1.t
        nc.vector.tensor_add(s1[:, 0:F - 1], rt[:, 0:F - 1], rt[:, 1:F])
        nc.vector.tensor_add(s2.t[:, 0:F - xc - 1], s1[:, 0:F - xc - 1], s1[:, xc:F - 1])
        ps = self.mm('R1', s2, P // 2, psum_w=F)
        t23 = ps[:, 0:F].rearrange("p (z x) -> p z x", x=xc)
        # compact local window [32p, 8z * 16x]
        r2l3 = r2l[:, 0:128].rearrange("p (z x) -> p z x", x=16)
        nc.vector.tensor_scalar_mul(r2l3[:, :, :], t23[:, 1:17:2, 2:34:2], 2.0 * C['wres'])
        agin = self.dram.tile([1, 32 * 128], F16, tag='agr2i', name=f'agr2i{nc.next_id()}')
        agout = self.dram.tile([NC_, 2, 16, 8, 16], F16, tag='agr2o', name=f'agr2o{nc.next_id()}',
                               addr_space="Shared")
        nc.sync.dma_start(agin[0:1, :], r2l[:, 0:128])
        nc.gpsimd.collective_compute(
            "AllGather", OP.bypass, replica_groups=[list(range(NC_))],
            ins=[agin.opt()], outs=[agout.opt()])
        d3 = self.D3(r2)
        engs = (nc.sync, nc.scalar, nc.gpsimd)
        for s in range(NC_):
            for zh in (0, 1):
                engs[(2 * s + zh) % 3].dma_start(
                    d3[:, 1 + zh * 8:9 + zh * 8, 1 + s * 16:17 + s * 16],
                    agout[s, zh, :, :, :])


# ---------------------------------------------------------------- entry
_CACHE = {}


def _get_nc(key, C, dbg_name=None, stage='full', dbg_init=False):
    ck = (key, dbg_name, stage, dbg_init)
    if ck not in _CACHE:
        mats_np, cols = build_mats(C)
        b = B(C, mats_np, cols, dbg_name=dbg_name, stage=stage, dbg_init=dbg_init)
        nc = b.build()
        _CACHE[ck] = (nc, mats_np)
    return _CACHE[ck]


def _pad_field(arr):
    """[64z, 64y, 64x] -> padded device tile [128, 34*72] (fp16)."""
    t = np.zeros((128, 34, 72), np.float16)
    # p = zh*64 + y ; row z' = 1..32 ; col 4..67
    a = arr.reshape(2, 32, 64, 64).transpose(0, 2, 1, 3).reshape(128, 32, 64)
    t[:, 1:33, 4:68] = a
    return t.reshape(128, 34 * 72)


def _make_in_maps(fields, mats_np):
    in_maps = []
    for r in range(NC_):
        m = {}
        for nm, arr in fields.items():
            loc = np.asarray(arr, np.float32)[0, 0, :, :, r * XL:(r + 1) * XL]
            m[nm] = _pad_field(loc)
        m['mats'] = mats_np
        msk = np.zeros((128, 4), np.float32)
        msk[:, 0] = 1.0 if r == 0 else 0.0       # mL
        msk[:, 1] = 0.0 if r == 0 else 1.0       # nmL
        msk[:, 2] = 1.0 if r == NC_ - 1 else 0.0  # mR
        msk[:, 3] = 0.0 if r == NC_ - 1 else 1.0  # nmR
        m['masks'] = msk
        ho = np.zeros((1, 3), np.int32)
        rl = max(r - 1, 0)
        rr = min(r + 1, NC_ - 1)
        ho[0, 0] = rl * 2 + 1   # left ghost <- left nbr's right-edge slot
        ho[0, 1] = rr * 2 + 0   # right ghost <- right nbr's left-edge slot
        ho[0, 2] = r * 16
        m['hoffs'] = ho
        in_maps.append(m)
    return in_maps


def kernel(alpha, values_u, values_v, values_w, values_pd,
           w_diff, w_xadv, w_yadv, w_zadv, w_A, w_res, _dbg=None, _stage='full', _dbg_init=False):
    C = extract_consts(w_diff, w_xadv, w_yadv, w_zadv, w_A, w_res)
    key = tuple(sorted(C.items()))
    nc, mats_np = _get_nc(key, C, dbg_name=_dbg, stage=_stage, dbg_init=_dbg_init)
    fields = {'alpha': alpha, 'values_u': values_u, 'values_v': values_v,
              'values_w': values_w, 'values_pd': values_pd}
    in_maps = _make_in_maps(fields, mats_np)
    res = run_bass_kernel_spmd(nc, in_maps, core_ids=list(range(NC_)))
    full = np.empty((4, NZ, NY, NX), np.float32)
    for r in range(NC_):
        o = res.results[r]['out'].reshape(4, 128, 34, 72)[:, :, 1:33, 4:68].astype(np.float32)
        # [4, (zh y), z', x] -> [4, (zh z'), y, x]
        o = o.reshape(4, 2, 64, 32, 64).transpose(0, 1, 3, 2, 4).reshape(4, 64, 64, 64)
        full[:, :, :, r * XL:(r + 1) * XL] = o
    if _dbg is not None:
        kernel._dbg_res = [res.results[r].get('dbg') for r in range(NC_)]
    return full[None]  # (1, 4, 64, 64, 512)
